# revision 30
# baseline (speedup 1.0000x reference)
"""Multi-head causal attention (B=4, T=2048, D=1024, H=16) on 8 Trainium2 cores.

Sharding: core c = (b, g) with b = c//2 (batch), g = c%2 (head-group of 8 heads).
Each core: Q/K/V projections for its 8 heads (column-parallel), causal attention,
row-parallel partial output projection. Host sums the g=0/g=1 partials + bias.

v4 design (fp8 DoubleRow + streaming AV; cost model: matmul = out-free-rows x
0.4167ns x cpr, fp8e4 DoubleRow cpr=0.5 contracting 2x128/instr; ACT exp at
0.8333ns/free-elem = ~147us busy is the floor):
  - Q/K proj: fp8 DR, x8 moving [128,2,512], w8 stationary [128,2,128] in 4
    col-groups (t=pair-half, h=dim-half) so psum partitions land as
    (beta=2*(pr%2)+u, dd) blocks of 32 -> qT8/kT8 [32-blocks, 2h, T] fp8 for
    2x32-contraction DR score matmuls. w quantized x16 (e4m3 subnormal
    dodge), unscaled in the DVE psum->fp8 copy; score scale 1/8 folded as
    qT8 = e4m3(q/2) + exp(scale=0.25). Sub-128 DR needs explicit
    tile_position (base-96 slices break base_partition inference).
  - scores: fp8 DR per (pr,u,kj): out ss [128 keys, 2u, 512-c0] psum.
  - exp: ACT psum -> sbuf, one instr per kj covering both heads. Off-diag
    chunks -> fp8 pt pairs [128, 2kj, 2u, 512]; diag chunks -> bf16 pt
    (pt-quant error only bites concentrated near-diagonal rows) with
    post-exp mask01 multiply on DVE (off ACT's critical path).
  - AV streaming into ctx psum [96, 512] per (pr,u), accumulated across the
    span: off-diag = single-fp8 DR (lhsT = Vbh [128, 2kj, 96]: 64 v-dims +
    ones col at 64 -> Z at psum row 64 + zero pad -- dual-fp8 ldweights
    needs cols%32==0, >=64); diag = bf16 non-DR (lhsT = Vbb [128, 65]).
    Diffuse off-diag rows average out single-fp8 V error; vonly ~1e-3.
  - evict per (pr,u): DVE reciprocal of Z row -> bf16 rz at partition 0
    (cross-partition psum read, hw-verified), gpsimd partition_broadcast ->
    rzbc sbuf, DVE mul ctx*rzbc -> ctx_sb f16 [128=(u,vd), 512] (u1 written
    cross-partition to rows 64:128). Evicts deferred into the next pair's /
    span's kj loop so the PE never stalls on them.
  - outproj: f16, 4x128-contraction per [128q, 512od] psum group; DVE f16
    stage -> DMA (psum DMA is forbidden); host sums g-partials + bias f32.
  - V proj: f16 (value-path precision), psum -> Vbh fp8 + Vbb bf16 copies,
    emitted just-in-time at pr0's first kj iterations of its own span.
  - schedule: filler pacing: qk-proj(s+1) paced through attention(s);
    outproj(0)@s2, outproj(1,2)@s3, outproj(3) tail. DMAs ride SP + ACT
    queues (Pool queue holds block partition_broadcast; DVE holds block
    copies).
Measured: 202801 ns (TimelineSim), rel err 1.20e-2 (gate 2e-2). Baseline
v3 was 240443 ns. ACT exp busy 146.9us (72%), PE 129us, DVE 118us.
"""

import os
import sys

try:
    import concourse.bass  # noqa: F401
except ImportError:  # pragma: no cover
    sys.path.insert(0, "/opt/trn_rl_repo")

import numpy as np

B, T, D = 4, 2048, 1024
H, HD = 16, 64
NCORES = 8
NPAIR = 4
NSPAN = 4
SPAN = 512
KC = 128
P = 128
LAG = 3

_CACHE = {}


def _build():
    import concourse.bacc as bacc
    import concourse.mybir as mybir
    import concourse.tile as tile

    f32 = mybir.dt.float32
    f16 = mybir.dt.float16
    bf16 = mybir.dt.bfloat16
    fp8 = mybir.dt.float8e4
    u8 = mybir.dt.uint8
    Exp = mybir.ActivationFunctionType.Exp

    DR = mybir.MatmulPerfMode.DoubleRow

    nc = bacc.Bacc("TRN2", target_bir_lowering=False, debug=False,
                   num_devices=NCORES)

    x8_h = nc.dram_tensor("x8", (D, T), u8, kind="ExternalInput")
    xh_h = nc.dram_tensor("xh", (D, T), f16, kind="ExternalInput")
    wq8_h = nc.dram_tensor("wq8", (D, 512), u8, kind="ExternalInput")
    wk8_h = nc.dram_tensor("wk8", (D, 512), u8, kind="ExternalInput")
    wvh_h = nc.dram_tensor("wvh", (D, 512), f16, kind="ExternalInput")
    woh_h = nc.dram_tensor("woh", (512, D), f16, kind="ExternalInput")
    out_h = nc.dram_tensor("out", (T, D), f16, kind="ExternalOutput")

    # x8: D = (dcp 4, k 2, p 128); per span slice on T
    x8_d = x8_h.ap().rearrange("(dcp k p) t -> p dcp k t", p=P, k=2)
    xh_d = xh_h.ap().rearrange("(dc p) t -> p dc t", p=P)
    wq8_d = wq8_h.ap().rearrange("(dcp k p) (t h m) -> p dcp k t h m",
                                 p=P, k=2, t=2, h=2)
    wk8_d = wk8_h.ap().rearrange("(dcp k p) (t h m) -> p dcp k t h m",
                                 p=P, k=2, t=2, h=2)
    wvh_d = wvh_h.ap().rearrange("(dc p) f -> p dc f", p=P)
    woh_d = woh_h.ap().rearrange("(pc p) f -> p pc f", p=P)

    with tile.TileContext(nc) as tc:
        with (
            tc.tile_pool(name="persist", bufs=1) as persist,
            tc.tile_pool(name="x8p", bufs=2) as x8pool,
            tc.tile_pool(name="xhp", bufs=2) as xhpool,
            tc.tile_pool(name="qp", bufs=4) as qpool,
            tc.tile_pool(name="ptp", bufs=6) as ptpool,
            tc.tile_pool(name="ptb", bufs=5) as ptbpool,
            tc.tile_pool(name="rzp", bufs=8) as rzpool,
            tc.tile_pool(name="cp", bufs=8) as cpool,
            tc.tile_pool(name="stg", bufs=6) as stgpool,
            tc.tile_pool(name="psS", bufs=2, space="PSUM") as psS,
            tc.tile_pool(name="psC", bufs=2, space="PSUM") as psC,
            tc.tile_pool(name="psT", bufs=2, space="PSUM") as psT,
        ):
            wq8 = persist.tile([P, 4, 2, 2, 2, P], u8, tag="wq8", name="wq8")
            wk8 = persist.tile([P, 4, 2, 2, 2, P], u8, tag="wk8", name="wk8")
            wvt = persist.tile([P, 8, 512], f16, tag="wvt", name="wvt")
            wot = persist.tile([P, 4, D], f16, tag="wot", name="wot")
            kT8 = [persist.tile([P, 2, T], fp8, tag=f"kT8{t}", name=f"kT8{t}")
                   for t in range(2)]
            # Vb: [p, kjp 8, kj 2, pr 4, u 2, 96]: 64 v-dims + ones col at
            # 64 + zero pad to 96 (dual-fp8 ldweights needs cols % 32 == 0,
            # >= 64; psum rows 65:96 are dead)
            VW = 96
            Vbh = persist.tile([P, 8, 2, NPAIR, 2, VW], fp8, tag="Vbh",
                               name="Vbh")
            # diag-chunk AV runs in bf16 (pt quantization error bites only
            # concentrated near-diagonal attention rows): [p, kc, pr, u, 65]
            Vbb = persist.tile([P, 16, NPAIR, 2, HD + 1], bf16, tag="Vbb",
                               name="Vbb")
            mask01 = persist.tile([P, KC], bf16, tag="mask01", name="mask01")
            one = nc.const_aps.tensor(1.0, (P, 1))

            # ---- initial DMAs ----
            x8ts = {0: x8pool.tile([P, 4, 2, SPAN], u8, tag="x8t", name="x8t0")}
            xhts = {0: xhpool.tile([P, 8, SPAN], f16, tag="xht", name="xht0")}
            nc.sync.dma_start(wq8[:], wq8_d)
            nc.scalar.dma_start(x8ts[0][:], x8_d[:, :, :, 0:SPAN])
            nc.sync.dma_start(wk8[:], wk8_d)
            nc.scalar.dma_start(xhts[0][:, 0:4], xh_d[:, 0:4, 0:SPAN])
            nc.sync.dma_start(wvt[:], wvh_d)
            nc.scalar.dma_start(xhts[0][:, 4:8], xh_d[:, 4:8, 0:SPAN])
            nc.sync.dma_start(wot[:], woh_d)


            # mask01[p, f] = 1 if p <= f else 0 (post-exp diag pt mask)
            nc.gpsimd.memset(mask01[:], 1.0)
            nc.gpsimd.affine_select(
                out=mask01[:], in_=mask01[:],
                compare_op=mybir.AluOpType.is_ge, fill=0.0,
                base=0, channel_multiplier=-1, pattern=[[1, KC]],
            )
            # Vbh pad cols [65:96] must be zero (read by every off-diag
            # AV); data cols are always written before first read
            nslots = 8 * 2 * NPAIR * 2
            nc.gpsimd.memset(
                Vbh[:].rearrange("p a b c d e -> p (a b c d) e")
                [:, :, HD + 1:], 0.0)
            nc.vector.tensor_copy(
                Vbh[:].rearrange("p a b c d e -> p (a b c d) e")
                [:, :, HD:HD + 1],
                one.to_broadcast((P, nslots, 1)))
            nc.vector.tensor_copy(
                Vbb[:].rearrange("p a c d e -> p (a c d) e")
                [:, :, HD:HD + 1],
                one.to_broadcast((P, 16 * NPAIR * 2, 1)))

            qts = {}    # (sp, t) -> [P, 2, SPAN] fp8 tile
            ctxs = {}   # (sp, pr) -> [P, SPAN] f16 tile

            # ---------- emission helpers ----------
            def proj_qk(w8, pr_half, h, sp, x8t, scale, isq):
                t = pr_half

                def emit():
                    ps = psT.tile([P, SPAN], f32, tag="tr", name="psqk")
                    for dcp in range(4):
                        nc.tensor.matmul(
                            ps[:], w8[:, dcp, :, t, h, :].bitcast(fp8),
                            x8t[:, dcp, :, :].bitcast(fp8),
                            start=(dcp == 0), stop=(dcp == 3), perf_mode=DR)
                    if isq:
                        if (sp, t) not in qts:
                            qts[(sp, t)] = qpool.tile(
                                [P, 2, SPAN], fp8, tag=f"qT{t}",
                                name=f"qT{t}_{sp}")
                        nc.vector.tensor_scalar_mul(
                            qts[(sp, t)][:, h, :], ps[:], scale)
                    else:
                        nc.vector.tensor_scalar_mul(
                            kT8[t][:, h, sp * SPAN:(sp + 1) * SPAN], ps[:],
                            scale)
                return emit

            def proj_v(sp, tb, xht):
                def emit():
                    ps = psT.tile([P, SPAN], f32, tag="tr", name="psv")
                    for dc in range(8):
                        nc.tensor.matmul(
                            ps[:], xht[:, dc, tb * P:(tb + 1) * P],
                            wvt[:, dc, :],
                            start=(dc == 0), stop=(dc == 7))
                    kc = sp * 4 + tb
                    psv = ps[:].rearrange("p (pr u v) -> p pr u v",
                                          pr=NPAIR, v=HD)
                    nc.vector.tensor_copy(
                        Vbh[:, kc // 2, kc % 2, :, :, 0:HD], psv)
                    nc.vector.tensor_copy(Vbb[:, kc, :, :, 0:HD], psv)
                return emit

            def qk_groups(sp, x8t):
                gs = []
                for t in range(2):
                    for h in range(2):
                        gs.append(proj_qk(wq8, t, h, sp, x8t, 1.0 / 32, True))
                        gs.append(proj_qk(wk8, t, h, sp, x8t, 1.0 / 16, False))
                return gs

            def v_groups(sp, xht):
                return [proj_v(sp, tb, xht) for tb in range(4)]

            def outproj_group(sp, tb, os_, dma_q):
                def emit():
                    ps = psT.tile([P, SPAN], f32, tag="tr", name="pso")
                    for pc in range(NPAIR):
                        nc.tensor.matmul(
                            ps[:],
                            ctxs[(sp, pc)][:, (tb - sp * 4) * P:
                                           (tb - sp * 4 + 1) * P],
                            wot[:, pc, os_ * SPAN:(os_ + 1) * SPAN],
                            start=(pc == 0), stop=(pc == NPAIR - 1))
                    stage = stgpool.tile([P, SPAN], f16, tag="st", name="stage")
                    nc.vector.tensor_copy(stage[:], ps[:])
                    dma_q.dma_start(
                        out_h.ap()[tb * P:(tb + 1) * P,
                                   os_ * SPAN:(os_ + 1) * SPAN], stage[:])
                return emit

            def outproj_groups(sp):
                return [outproj_group(sp, tb, os_,
                                      nc.sync if os_ == 0 else nc.gpsimd)
                        for tb in range(sp * 4, (sp + 1) * 4)
                        for os_ in range(2)]

            # ---------- attention for one span ----------
            deferred = []

            def attn_span(s, fillers, vgs=()):
                K = 4 * (s + 1)
                nslot = (K + 4) * NPAIR
                state = {"slot": 0, "fi": 0}

                def pace():
                    tgt = min(len(fillers),
                              len(fillers) * (state["slot"] + 1) // nslot)
                    while state["fi"] < tgt:
                        fillers[state["fi"]]()
                        state["fi"] += 1

                def tick():
                    state["slot"] += 1
                    pace()

                for pr in range(NPAIR):
                    t = pr // 2
                    ctxp = [psC.tile([96, SPAN], f32, tag="ctx",
                                     name=f"ctx{u}") for u in range(2)]
                    ct = cpool.tile([P, SPAN], f16, tag=f"cT{pr}",
                                    name=f"cT{pr}_{s}")
                    ctxs[(s, pr)] = ct
                    pts = {}
                    qt = qts[(s, t)]

                    avn = [0, 0]  # AV matmuls emitted per u (K total each)

                    def emit_qk(kj, pr=pr, t=t, pts=pts, qt=qt):
                        m = kj - 4 * s
                        ss = psS.tile([P, 2, SPAN], f32, tag="psS", name="ss")
                        c0 = 0 if m < 0 else m * KC
                        for u in range(2):
                            b32 = 32 * (2 * (pr % 2) + u)
                            r = slice(b32, b32 + 32)
                            nc.tensor.matmul(
                                ss[:, u, c0:],
                                kT8[t][r, :, kj * KC:(kj + 1) * KC],
                                qt[r, :, c0:],
                                start=True, stop=True, perf_mode=DR,
                                tile_position=(b32, 0))
                        if m < 0:
                            # off-diagonal: fp8 pt pair for DR AV
                            kjp, sl = kj // 2, kj % 2
                            if sl == 0:
                                pts[kjp] = ptpool.tile([P, 2, 2, SPAN], fp8,
                                                       tag="pt", name="pt")
                            nc.scalar.activation(pts[kjp][:, sl, :, :],
                                                 ss[:, :, :], Exp, scale=0.25)
                        else:
                            # diagonal: bf16 pt (exact-ish attention weights
                            # for concentrated rows) + post-exp triangle mask
                            ptb = ptbpool.tile([P, 2, SPAN], bf16,
                                               tag="ptb", name="ptb")
                            pts[("d", kj)] = ptb
                            nc.scalar.activation(ptb[:, :, c0:],
                                                 ss[:, :, c0:], Exp,
                                                 scale=0.25)
                            nc.vector.tensor_mul(
                                ptb[:, :, c0:c0 + KC], ptb[:, :, c0:c0 + KC],
                                mask01[:].rearrange("p (u f) -> p u f", u=1)
                                .to_broadcast((P, 2, KC)))

                    NU = 2 * s + 4  # AV matmuls per u-chain

                    def emit_av8(kjp, pr=pr, pts=pts, ctxp=ctxp, NU=NU):
                        pt = pts.pop(kjp)
                        for u in range(2):
                            nc.tensor.matmul(
                                ctxp[u][:],
                                Vbh[:, kjp, :, pr, u, :],
                                pt[:, :, u, :],
                                start=(avn[u] == 0),
                                stop=(avn[u] == NU - 1),
                                perf_mode=DR, skip_group_check=True)
                            avn[u] += 1

                    def emit_avd(kj, pr=pr, pts=pts, ctxp=ctxp, NU=NU):
                        ptb = pts.pop(("d", kj))
                        c0 = (kj - 4 * s) * KC
                        for u in range(2):
                            nc.tensor.matmul(
                                ctxp[u][0:HD + 1, c0:],
                                Vbb[:, kj, pr, u, :],
                                ptb[:, u, c0:],
                                start=(avn[u] == 0), stop=(avn[u] == NU - 1),
                                skip_group_check=True)
                            avn[u] += 1

                    def evict(pr=pr, ctxp=ctxp, ct=ct, s=s):
                        rz = rzpool.tile([P, 2, SPAN], bf16, tag="rz",
                                         name="rz")
                        rzbc = rzpool.tile([P, 2, SPAN], bf16, tag="rz",
                                           name="rzbc")

                        def fin(u):
                            def run():
                                # 1/Z row: psum row 64 -> sbuf row 0
                                # (cross-partition DVE write, hw-verified)
                                with nc.allow_low_precision(
                                        reason="1/Z bf16: 0.4% on ctx"):
                                    nc.vector.reciprocal(
                                        rz[0:1, u, :], ctxp[u][64:65, :])
                                nc.gpsimd.partition_broadcast(
                                    rzbc[:, u, :], rz[0:1, u, :])
                                nc.vector.tensor_mul(
                                    ct[u * HD:(u + 1) * HD, :],
                                    ctxp[u][0:HD, :], rzbc[0:HD, u, :])
                            return run
                        deferred.append(fin(0))
                        deferred.append(fin(1))

                    # AV work units: (ready_kj, emit_fn); off-diag kjp ready
                    # at its odd kj, diag kj ready at kj
                    units = []
                    for kj2 in range(K):
                        if kj2 < 4 * s:
                            if kj2 % 2 == 1:
                                units.append((kj2, kj2 // 2, emit_av8))
                        else:
                            units.append((kj2, kj2, emit_avd))
                    ui = [0]
                    for kj in range(K):
                        if pr == 0 and kj < len(vgs):
                            # span s's V tiles feed its own diag AV, LAG
                            # slots later -- just-in-time, not paced
                            vgs[kj]()
                        emit_qk(kj)
                        if deferred:
                            deferred.pop(0)()
                        tick()
                        while (ui[0] < len(units)
                               and units[ui[0]][0] + LAG <= kj):
                            units[ui[0]][2](units[ui[0]][1])
                            ui[0] += 1
                    while ui[0] < len(units):
                        if deferred:
                            deferred.pop(0)()
                        tick()
                        units[ui[0]][2](units[ui[0]][1])
                        ui[0] += 1
                    evict()
                while state["fi"] < len(fillers):
                    fillers[state["fi"]]()
                    state["fi"] += 1

            # ---------- main schedule ----------
            # V(s) runs inside span s itself (its first consumer is span s's
            # own diag AV, ~LAG kjs in) -- keeps spans 0/1 off the PE
            for g in qk_groups(0, x8ts[0]):
                g()
            for s in range(NSPAN):
                vgs = v_groups(s, xhts[s])
                fillers = []
                if s + 1 < NSPAN:
                    x8t = x8pool.tile([P, 4, 2, SPAN], u8, tag="x8t",
                                      name=f"x8t{s + 1}")
                    xht = xhpool.tile([P, 8, SPAN], f16, tag="xht",
                                      name=f"xht{s + 1}")
                    x8ts[s + 1], xhts[s + 1] = x8t, xht
                    sl = slice((s + 1) * SPAN, (s + 2) * SPAN)
                    nc.sync.dma_start(x8t[:], x8_d[:, :, :, sl])
                    nc.scalar.dma_start(xht[:, 0:4], xh_d[:, 0:4, sl])
                    nc.scalar.dma_start(xht[:, 4:8], xh_d[:, 4:8, sl])
                    fillers += qk_groups(s + 1, x8t)
                if s == 2:
                    fillers += outproj_groups(0)
                elif s == 3:
                    fillers += outproj_groups(1) + outproj_groups(2)
                attn_span(s, fillers, vgs)
            while deferred:
                deferred.pop(0)()
            for g in outproj_groups(3):
                g()

    nc.compile()
    return nc


def get_nc():
    if "nc" not in _CACHE:
        _CACHE["nc"] = _build()
    return _CACHE["nc"]


def _perm512():
    perm = np.empty(512, np.int64)
    i = 0
    for t in range(2):
        for h in range(2):
            for beta in range(4):
                pr = 2 * t + beta // 2
                u = beta % 2
                for dd in range(32):
                    perm[i] = pr * 128 + u * 64 + h * 32 + dd
                    i += 1
    return perm


def kernel(x, Wq, Wk, Wv, Wo, bo):
    import ml_dtypes
    from concourse import bass_utils

    e4 = ml_dtypes.float8_e4m3

    x = np.asarray(x, dtype=np.float32)
    Wq, Wk, Wv = (np.asarray(w, dtype=np.float32) for w in (Wq, Wk, Wv))
    Wo = np.asarray(Wo, dtype=np.float32)
    bo = np.asarray(bo, dtype=np.float32)
    perm = _perm512()

    in_maps = []
    for c in range(NCORES):
        b, g = c // 2, c % 2
        gsl = slice(g * 512, (g + 1) * 512)
        xT = np.ascontiguousarray(x[b].T)
        in_maps.append({
            "x8": xT.astype(e4).view(np.uint8),
            "xh": xT.astype(np.float16),
            # w quantized x16 to dodge e4m3 subnormals; unscaled on-chip
            "wq8": np.ascontiguousarray((16.0 * Wq[gsl].T)[:, perm])
            .astype(e4).view(np.uint8),
            "wk8": np.ascontiguousarray((16.0 * Wk[gsl].T)[:, perm])
            .astype(e4).view(np.uint8),
            "wvh": np.ascontiguousarray(Wv[gsl].T).astype(np.float16),
            "woh": np.ascontiguousarray(Wo[:, gsl].T).astype(np.float16),
        })

    nc = get_nc()
    res = bass_utils.run_bass_kernel_spmd(nc, in_maps,
                                          core_ids=list(range(NCORES)))
    parts = [res.results[c]["out"].astype(np.float32) for c in range(NCORES)]
    out = np.stack([parts[2 * b] + parts[2 * b + 1] + bo for b in range(B)])
    return out.astype(np.float32)


# revision 42
# speedup vs baseline: 1.0101x; 1.0101x over previous
"""Multi-head causal attention (B=4, T=2048, D=1024, H=16) on 8 Trainium2 cores.

Sharding: core c = (b, g) with b = c//2 (batch), g = c%2 (head-group of 8 heads).
Each core: Q/K/V projections for its 8 heads (column-parallel), causal attention,
row-parallel partial output projection. Host sums the g=0/g=1 partials + bias.

v4 design (fp8 DoubleRow + streaming AV; cost model: matmul = out-free-rows x
0.4167ns x cpr, fp8e4 DoubleRow cpr=0.5 contracting 2x128/instr; ACT exp at
0.8333ns/free-elem = ~147us busy is the floor):
  - Q/K proj: fp8 DR, x8 moving [128,2,512], w8 stationary [128,2,128] in 4
    col-groups (t=pair-half, h=dim-half) so psum partitions land as
    (beta=2*(pr%2)+u, dd) blocks of 32 -> qT8/kT8 [32-blocks, 2h, T] fp8 for
    2x32-contraction DR score matmuls. w quantized x16 (e4m3 subnormal
    dodge), unscaled in the DVE psum->fp8 copy; score scale 1/8 folded as
    qT8 = e4m3(q/2) + exp(scale=0.25). Sub-128 DR needs explicit
    tile_position (base-96 slices break base_partition inference).
  - scores: fp8 DR per (pr,u,kj): out ss [128 keys, 2u, 512-c0] psum.
  - exp: ACT psum -> sbuf, one instr per kj covering both heads. Off-diag
    chunks -> fp8 pt pairs [128, 2kj, 2u, 512]; diag chunks -> bf16 pt
    (pt-quant error only bites concentrated near-diagonal rows) with
    post-exp mask01 multiply on DVE (off ACT's critical path).
  - AV streaming into ctx psum [96, 512] per (pr,u), accumulated across the
    span: off-diag = single-fp8 DR (lhsT = Vbh [128, 2kj, 96]: 64 v-dims +
    ones col at 64 -> Z at psum row 64 + zero pad -- dual-fp8 ldweights
    needs cols%32==0, >=64); diag = bf16 non-DR (lhsT = Vbb [128, 65]).
    Diffuse off-diag rows average out single-fp8 V error; vonly ~1e-3.
  - evict per (pr,u): DVE reciprocal of Z row -> bf16 rz at partition 0
    (cross-partition psum read, hw-verified), gpsimd partition_broadcast ->
    rzbc sbuf, DVE mul ctx*rzbc -> ctx_sb f16 [128=(u,vd), 512] (u1 written
    cross-partition to rows 64:128). Evicts deferred into the next pair's /
    span's kj loop so the PE never stalls on them.
  - outproj: f16, 4x128-contraction per [128q, 512od] psum group; DVE f16
    stage -> DMA (psum DMA is forbidden); host sums g-partials + bias f32.
  - V proj: f16 (value-path precision), psum -> Vbh fp8 + Vbb bf16 copies,
    emitted just-in-time at pr0's first kj iterations of its own span.
  - schedule: filler pacing: qk-proj(s+1) paced through attention(s);
    outproj(0)@s2, outproj(1,2)@s3, outproj(3) tail. DMAs ride SP + ACT
    queues (Pool queue holds block partition_broadcast; DVE holds block
    copies).
Measured: 202801 ns (TimelineSim), rel err 1.20e-2 (gate 2e-2). Baseline
v3 was 240443 ns. ACT exp busy 146.9us (72%), PE 129us, DVE 118us.
"""

import os
import sys

try:
    import concourse.bass  # noqa: F401
except ImportError:  # pragma: no cover
    sys.path.insert(0, "/opt/trn_rl_repo")

import numpy as np

B, T, D = 4, 2048, 1024
H, HD = 16, 64
NCORES = 8
NPAIR = 4
NSPAN = 4
SPAN = 512
KC = 128
P = 128
LAG = 3

_CACHE = {}


def _build():
    import concourse.bacc as bacc
    import concourse.mybir as mybir
    import concourse.tile as tile

    f32 = mybir.dt.float32
    f16 = mybir.dt.float16
    bf16 = mybir.dt.bfloat16
    fp8 = mybir.dt.float8e4
    u8 = mybir.dt.uint8
    Exp = mybir.ActivationFunctionType.Exp

    DR = mybir.MatmulPerfMode.DoubleRow

    nc = bacc.Bacc("TRN2", target_bir_lowering=False, debug=False,
                   num_devices=NCORES)

    x8_h = nc.dram_tensor("x8", (D, T), u8, kind="ExternalInput")
    xh_h = nc.dram_tensor("xh", (D, T), f16, kind="ExternalInput")
    wq8_h = nc.dram_tensor("wq8", (D, 512), u8, kind="ExternalInput")
    wk8_h = nc.dram_tensor("wk8", (D, 512), u8, kind="ExternalInput")
    wvh_h = nc.dram_tensor("wvh", (D, 512), f16, kind="ExternalInput")
    woh_h = nc.dram_tensor("woh", (512, D), f16, kind="ExternalInput")
    out_h = nc.dram_tensor("out", (T, D), f16, kind="ExternalOutput")

    # x8: D = (dcp 4, k 2, p 128); per span slice on T
    x8_d = x8_h.ap().rearrange("(dcp k p) t -> p dcp k t", p=P, k=2)
    xh_d = xh_h.ap().rearrange("(dc p) t -> p dc t", p=P)
    wq8_d = wq8_h.ap().rearrange("(dcp k p) (t h m) -> p dcp k t h m",
                                 p=P, k=2, t=2, h=2)
    wk8_d = wk8_h.ap().rearrange("(dcp k p) (t h m) -> p dcp k t h m",
                                 p=P, k=2, t=2, h=2)
    wvh_d = wvh_h.ap().rearrange("(dc p) f -> p dc f", p=P)
    woh_d = woh_h.ap().rearrange("(pc p) f -> p pc f", p=P)

    with tile.TileContext(nc) as tc:
        with (
            tc.tile_pool(name="persist", bufs=1) as persist,
            tc.tile_pool(name="x8p", bufs=2) as x8pool,
            tc.tile_pool(name="xhp", bufs=2) as xhpool,
            tc.tile_pool(name="qp", bufs=4) as qpool,
            tc.tile_pool(name="ptp", bufs=6) as ptpool,
            tc.tile_pool(name="ptb", bufs=5) as ptbpool,
            tc.tile_pool(name="rzp", bufs=8) as rzpool,
            tc.tile_pool(name="cp", bufs=8) as cpool,
            tc.tile_pool(name="stg", bufs=6) as stgpool,
            tc.tile_pool(name="psS", bufs=2, space="PSUM") as psS,
            tc.tile_pool(name="psC", bufs=2, space="PSUM") as psC,
            tc.tile_pool(name="psT", bufs=2, space="PSUM") as psT,
        ):
            wq8 = persist.tile([P, 4, 2, 2, 2, P], u8, tag="wq8", name="wq8")
            wk8 = persist.tile([P, 4, 2, 2, 2, P], u8, tag="wk8", name="wk8")
            wvt = persist.tile([P, 8, 512], f16, tag="wvt", name="wvt")
            wot = persist.tile([P, 4, D], f16, tag="wot", name="wot")
            kT8 = [persist.tile([P, 2, T], fp8, tag=f"kT8{t}", name=f"kT8{t}")
                   for t in range(2)]
            # Vb: [p, kjp 8, kj 2, pr 4, u 2, 96]: 64 v-dims + ones col at
            # 64 + zero pad to 96 (dual-fp8 ldweights needs cols % 32 == 0,
            # >= 64; psum rows 65:96 are dead)
            VW = 96
            Vbh = persist.tile([P, 8, 2, NPAIR, 2, VW], fp8, tag="Vbh",
                               name="Vbh")
            # diag-chunk AV runs in bf16 (pt quantization error bites only
            # concentrated near-diagonal attention rows): [p, kc, pr, u, 65]
            Vbb = persist.tile([P, 16, NPAIR, 2, HD + 1], bf16, tag="Vbb",
                               name="Vbb")
            mask01 = persist.tile([P, KC], bf16, tag="mask01", name="mask01")
            one = nc.const_aps.tensor(1.0, (P, 1))

            # ---- initial DMAs ----
            x8ts = {0: x8pool.tile([P, 4, 2, SPAN], u8, tag="x8t", name="x8t0")}
            xhts = {0: xhpool.tile([P, 8, SPAN], f16, tag="xht", name="xht0")}
            nc.sync.dma_start(wq8[:], wq8_d)
            nc.sync.dma_start(x8ts[0][:], x8_d[:, :, :, 0:SPAN])
            nc.sync.dma_start(wk8[:], wk8_d)
            nc.sync.dma_start(xhts[0][:, 0:4], xh_d[:, 0:4, 0:SPAN])
            nc.sync.dma_start(wvt[:], wvh_d)
            nc.sync.dma_start(xhts[0][:, 4:8], xh_d[:, 4:8, 0:SPAN])
            nc.sync.dma_start(wot[:], woh_d)


            # mask01[p, f] = 1 if p <= f else 0 (post-exp diag pt mask)
            nc.gpsimd.memset(mask01[:], 1.0)
            nc.gpsimd.affine_select(
                out=mask01[:], in_=mask01[:],
                compare_op=mybir.AluOpType.is_ge, fill=0.0,
                base=0, channel_multiplier=-1, pattern=[[1, KC]],
            )
            # Vbh pad cols [65:96] must be zero (read by every off-diag
            # AV); data cols are always written before first read
            nslots = 8 * 2 * NPAIR * 2
            nc.gpsimd.memset(
                Vbh[:].rearrange("p a b c d e -> p (a b c d) e")
                [:, :, HD + 1:], 0.0)
            nc.vector.tensor_copy(
                Vbh[:].rearrange("p a b c d e -> p (a b c d) e")
                [:, :, HD:HD + 1],
                one.to_broadcast((P, nslots, 1)))
            nc.vector.tensor_copy(
                Vbb[:].rearrange("p a c d e -> p (a c d) e")
                [:, :, HD:HD + 1],
                one.to_broadcast((P, 16 * NPAIR * 2, 1)))

            qts = {}    # (sp, t) -> [P, 2, SPAN] fp8 tile
            ctxs = {}   # (sp, pr) -> [P, SPAN] f16 tile

            # ---------- emission helpers ----------
            def proj_qk(w8, pr_half, h, sp, x8t, scale, isq):
                t = pr_half

                def emit():
                    ps = psT.tile([P, SPAN], f32, tag="tr", name="psqk")
                    for dcp in range(4):
                        nc.tensor.matmul(
                            ps[:], w8[:, dcp, :, t, h, :].bitcast(fp8),
                            x8t[:, dcp, :, :].bitcast(fp8),
                            start=(dcp == 0), stop=(dcp == 3), perf_mode=DR)
                    if isq:
                        if (sp, t) not in qts:
                            qts[(sp, t)] = qpool.tile(
                                [P, 2, SPAN], fp8, tag=f"qT{t}",
                                name=f"qT{t}_{sp}")
                        nc.vector.tensor_scalar_mul(
                            qts[(sp, t)][:, h, :], ps[:], scale)
                    else:
                        nc.vector.tensor_scalar_mul(
                            kT8[t][:, h, sp * SPAN:(sp + 1) * SPAN], ps[:],
                            scale)
                return emit

            def proj_v(sp, tb, xht):
                def emit():
                    ps = psT.tile([P, SPAN], f32, tag="tr", name="psv")
                    for dc in range(8):
                        nc.tensor.matmul(
                            ps[:], xht[:, dc, tb * P:(tb + 1) * P],
                            wvt[:, dc, :],
                            start=(dc == 0), stop=(dc == 7))
                    kc = sp * 4 + tb
                    psv = ps[:].rearrange("p (pr u v) -> p pr u v",
                                          pr=NPAIR, v=HD)
                    nc.vector.tensor_copy(
                        Vbh[:, kc // 2, kc % 2, :, :, 0:HD], psv)
                    nc.vector.tensor_copy(Vbb[:, kc, :, :, 0:HD], psv)
                return emit

            def q_groups(sp, x8t):
                return [proj_qk(wq8, t, h, sp, x8t, 1.0 / 32, True)
                        for t in range(2) for h in range(2)]

            def k_groups(sp, x8t):
                return [proj_qk(wk8, t, h, sp, x8t, 1.0 / 16, False)
                        for t in range(2) for h in range(2)]

            def v_groups(sp, xht):
                return [proj_v(sp, tb, xht) for tb in range(4)]

            def outproj_group(sp, tb, os_, dma_q):
                def emit():
                    ps = psT.tile([P, SPAN], f32, tag="tr", name="pso")
                    for pc in range(NPAIR):
                        nc.tensor.matmul(
                            ps[:],
                            ctxs[(sp, pc)][:, (tb - sp * 4) * P:
                                           (tb - sp * 4 + 1) * P],
                            wot[:, pc, os_ * SPAN:(os_ + 1) * SPAN],
                            start=(pc == 0), stop=(pc == NPAIR - 1))
                    stage = stgpool.tile([P, SPAN], f16, tag="st", name="stage")
                    nc.vector.tensor_copy(stage[:], ps[:])
                    dma_q.dma_start(
                        out_h.ap()[tb * P:(tb + 1) * P,
                                   os_ * SPAN:(os_ + 1) * SPAN], stage[:])
                return emit

            def outproj_groups(sp):
                return [outproj_group(sp, tb, os_,
                                      nc.sync if os_ == 0 else nc.gpsimd)
                        for tb in range(sp * 4, (sp + 1) * 4)
                        for os_ in range(2)]

            # ---------- attention for one span ----------
            deferred = []

            def attn_span(s, fillers, vgs=(), kgs=()):
                K = 4 * (s + 1)
                nslot = (K + 4) * NPAIR
                state = {"slot": 0, "fi": 0}

                def pace():
                    tgt = min(len(fillers),
                              len(fillers) * (state["slot"] + 1) // nslot)
                    while state["fi"] < tgt:
                        fillers[state["fi"]]()
                        state["fi"] += 1

                def tick():
                    state["slot"] += 1
                    pace()

                for pr in range(NPAIR):
                    t = pr // 2
                    ctxp = [psC.tile([96, SPAN], f32, tag="ctx",
                                     name=f"ctx{u}") for u in range(2)]
                    ct = cpool.tile([P, SPAN], f16, tag=f"cT{pr}",
                                    name=f"cT{pr}_{s}")
                    ctxs[(s, pr)] = ct
                    pts = {}
                    qt = qts[(s, t)]

                    avn = [0, 0]  # AV matmuls emitted per u (K total each)

                    def emit_qk(kj, pr=pr, t=t, pts=pts, qt=qt):
                        m = kj - 4 * s
                        ss = psS.tile([P, 2, SPAN], f32, tag="psS", name="ss")
                        c0 = 0 if m < 0 else m * KC
                        for u in range(2):
                            b32 = 32 * (2 * (pr % 2) + u)
                            r = slice(b32, b32 + 32)
                            nc.tensor.matmul(
                                ss[:, u, c0:],
                                kT8[t][r, :, kj * KC:(kj + 1) * KC],
                                qt[r, :, c0:],
                                start=True, stop=True, perf_mode=DR,
                                tile_position=(b32, 0))
                        if m < 0:
                            # off-diagonal: fp8 pt pair for DR AV
                            kjp, sl = kj // 2, kj % 2
                            if sl == 0:
                                pts[kjp] = ptpool.tile([P, 2, 2, SPAN], fp8,
                                                       tag="pt", name="pt")
                            nc.scalar.activation(pts[kjp][:, sl, :, :],
                                                 ss[:, :, :], Exp, scale=0.25)
                        else:
                            # diagonal: bf16 pt (exact-ish attention weights
                            # for concentrated rows) + post-exp triangle mask
                            ptb = ptbpool.tile([P, 2, SPAN], bf16,
                                               tag="ptb", name="ptb")
                            pts[("d", kj)] = ptb
                            nc.scalar.activation(ptb[:, :, c0:],
                                                 ss[:, :, c0:], Exp,
                                                 scale=0.25)
                            nc.vector.tensor_mul(
                                ptb[:, :, c0:c0 + KC], ptb[:, :, c0:c0 + KC],
                                mask01[:].rearrange("p (u f) -> p u f", u=1)
                                .to_broadcast((P, 2, KC)))

                    NU = 2 * s + 4  # AV matmuls per u-chain

                    def emit_av8(kjp, pr=pr, pts=pts, ctxp=ctxp, NU=NU):
                        pt = pts.pop(kjp)
                        for u in range(2):
                            nc.tensor.matmul(
                                ctxp[u][:],
                                Vbh[:, kjp, :, pr, u, :],
                                pt[:, :, u, :],
                                start=(avn[u] == 0),
                                stop=(avn[u] == NU - 1),
                                perf_mode=DR, skip_group_check=True)
                            avn[u] += 1

                    def emit_avd(kj, pr=pr, pts=pts, ctxp=ctxp, NU=NU):
                        ptb = pts.pop(("d", kj))
                        c0 = (kj - 4 * s) * KC
                        for u in range(2):
                            nc.tensor.matmul(
                                ctxp[u][0:HD + 1, c0:],
                                Vbb[:, kj, pr, u, :],
                                ptb[:, u, c0:],
                                start=(avn[u] == 0), stop=(avn[u] == NU - 1),
                                skip_group_check=True)
                            avn[u] += 1

                    def evict(pr=pr, ctxp=ctxp, ct=ct, s=s):
                        rz = rzpool.tile([P, 2, SPAN], bf16, tag="rz",
                                         name="rz")
                        rzbc = rzpool.tile([P, 2, SPAN], bf16, tag="rz",
                                           name="rzbc")

                        def fin(u):
                            def run():
                                # 1/Z row: psum row 64 -> sbuf row 0
                                # (cross-partition DVE write, hw-verified)
                                with nc.allow_low_precision(
                                        reason="1/Z bf16: 0.4% on ctx"):
                                    nc.vector.reciprocal(
                                        rz[0:1, u, :], ctxp[u][64:65, :])
                                nc.gpsimd.partition_broadcast(
                                    rzbc[:, u, :], rz[0:1, u, :])
                                nc.vector.tensor_mul(
                                    ct[u * HD:(u + 1) * HD, :],
                                    ctxp[u][0:HD, :], rzbc[0:HD, u, :])
                            return run
                        deferred.append(fin(0))
                        deferred.append(fin(1))

                    # AV work units: (ready_kj, emit_fn); off-diag kjp ready
                    # at its odd kj, diag kj ready at kj
                    units = []
                    for kj2 in range(K):
                        if kj2 < 4 * s:
                            if kj2 % 2 == 1:
                                units.append((kj2, kj2 // 2, emit_av8))
                        else:
                            units.append((kj2, kj2, emit_avd))
                    # JIT placement: V(s) spread across pr0's whole kj
                    # range (consumers: pr0 diag AV); K(s) t0-groups in pr0,
                    # t1-groups in pr1 (consumers: pr0/pr2 diag QK)
                    jit = {}
                    if pr == 0:
                        for i in range(len(vgs)):
                            jit.setdefault(max(i * K // 4, i), []).append(
                                vgs[i])
                        # K(s) t0 keys must precede the diag QK at kj=4s
                        # (jit runs after emit_qk in the same iteration)
                        for i in (0, 1) if kgs else ():
                            jit.setdefault(min(1 + i * (K // 3 + 1),
                                               4 * s - 1), []).append(kgs[i])
                    elif pr == 1:
                        for i in (2, 3) if kgs else ():
                            jit.setdefault(1 + (i - 2) * (K // 3 + 1),
                                           []).append(kgs[i])
                    ui = [0]
                    for kj in range(K):
                        emit_qk(kj)
                        for g in jit.get(kj, ()):
                            g()
                        if deferred:
                            deferred.pop(0)()
                        tick()
                        while (ui[0] < len(units)
                               and units[ui[0]][0] + LAG <= kj):
                            units[ui[0]][2](units[ui[0]][1])
                            ui[0] += 1
                    while ui[0] < len(units):
                        if deferred:
                            deferred.pop(0)()
                        tick()
                        units[ui[0]][2](units[ui[0]][1])
                        ui[0] += 1
                    evict()
                while state["fi"] < len(fillers):
                    fillers[state["fi"]]()
                    state["fi"] += 1

            # ---------- main schedule ----------
            # V(s) and K(s) run inside span s itself (their first consumers
            # are span s's own diag AV / diag QK) -- keeps earlier spans off
            # the PE; only Q(s+1) must finish during span s
            # pr0's attention needs (q,k) t0 groups first: interleave
            qg0, kg0 = q_groups(0, x8ts[0]), k_groups(0, x8ts[0])
            for i in range(4):
                qg0[i]()
                kg0[i]()
            for s in range(NSPAN):
                vgs = v_groups(s, xhts[s])
                kgs = k_groups(s, x8ts[s]) if s >= 1 else ()
                fillers = []
                if s + 1 < NSPAN:
                    x8t = x8pool.tile([P, 4, 2, SPAN], u8, tag="x8t",
                                      name=f"x8t{s + 1}")
                    xht = xhpool.tile([P, 8, SPAN], f16, tag="xht",
                                      name=f"xht{s + 1}")
                    x8ts[s + 1], xhts[s + 1] = x8t, xht
                    sl = slice((s + 1) * SPAN, (s + 2) * SPAN)
                    nc.sync.dma_start(x8t[:], x8_d[:, :, :, sl])
                    nc.sync.dma_start(xht[:, 0:4], xh_d[:, 0:4, sl])
                    nc.sync.dma_start(xht[:, 4:8], xh_d[:, 4:8, sl])
                    fillers += q_groups(s + 1, x8t)
                if s == 2:
                    fillers += outproj_groups(0)
                elif s == 3:
                    fillers += outproj_groups(1) + outproj_groups(2)
                attn_span(s, fillers, vgs, kgs)
            while deferred:
                deferred.pop(0)()
            for g in outproj_groups(3):
                g()

    nc.compile()
    return nc


def get_nc():
    if "nc" not in _CACHE:
        _CACHE["nc"] = _build()
    return _CACHE["nc"]


def _perm512():
    perm = np.empty(512, np.int64)
    i = 0
    for t in range(2):
        for h in range(2):
            for beta in range(4):
                pr = 2 * t + beta // 2
                u = beta % 2
                for dd in range(32):
                    perm[i] = pr * 128 + u * 64 + h * 32 + dd
                    i += 1
    return perm


def kernel(x, Wq, Wk, Wv, Wo, bo):
    import ml_dtypes
    from concourse import bass_utils

    e4 = ml_dtypes.float8_e4m3

    x = np.asarray(x, dtype=np.float32)
    Wq, Wk, Wv = (np.asarray(w, dtype=np.float32) for w in (Wq, Wk, Wv))
    Wo = np.asarray(Wo, dtype=np.float32)
    bo = np.asarray(bo, dtype=np.float32)
    perm = _perm512()

    in_maps = []
    for c in range(NCORES):
        b, g = c // 2, c % 2
        gsl = slice(g * 512, (g + 1) * 512)
        xT = np.ascontiguousarray(x[b].T)
        in_maps.append({
            "x8": xT.astype(e4).view(np.uint8),
            "xh": xT.astype(np.float16),
            # w quantized x16 to dodge e4m3 subnormals; unscaled on-chip
            "wq8": np.ascontiguousarray((16.0 * Wq[gsl].T)[:, perm])
            .astype(e4).view(np.uint8),
            "wk8": np.ascontiguousarray((16.0 * Wk[gsl].T)[:, perm])
            .astype(e4).view(np.uint8),
            "wvh": np.ascontiguousarray(Wv[gsl].T).astype(np.float16),
            "woh": np.ascontiguousarray(Wo[:, gsl].T).astype(np.float16),
        })

    nc = get_nc()
    res = bass_utils.run_bass_kernel_spmd(nc, in_maps,
                                          core_ids=list(range(NCORES)))
    parts = [res.results[c]["out"].astype(np.float32) for c in range(NCORES)]
    out = np.stack([parts[2 * b] + parts[2 * b + 1] + bo for b in range(B)])
    return out.astype(np.float32)


# revision 48
# speedup vs baseline: 1.0303x; 1.0201x over previous
"""Multi-head causal attention (B=4, T=2048, D=1024, H=16) on 8 Trainium2 cores.

Sharding: core c = (b, g) with b = c//2 (batch), g = c%2 (head-group of 8 heads).
Each core: Q/K/V projections for its 8 heads (column-parallel), causal attention,
row-parallel partial output projection. Host sums the g=0/g=1 partials + bias.

v4 design (fp8 DoubleRow + streaming AV; cost model: matmul = out-free-rows x
0.4167ns x cpr, fp8e4 DoubleRow cpr=0.5 contracting 2x128/instr; ACT exp at
0.8333ns/free-elem = ~147us busy is the floor):
  - Q/K proj: fp8 DR, x8 moving [128,2,512], w8 stationary [128,2,128] in 4
    col-groups (t=pair-half, h=dim-half) so psum partitions land as
    (beta=2*(pr%2)+u, dd) blocks of 32 -> qT8/kT8 [32-blocks, 2h, T] fp8 for
    2x32-contraction DR score matmuls. w quantized x16 (e4m3 subnormal
    dodge), unscaled in the DVE psum->fp8 copy; score scale 1/8 folded as
    qT8 = e4m3(q/2) + exp(scale=0.25). Sub-128 DR needs explicit
    tile_position (base-96 slices break base_partition inference).
  - scores: fp8 DR per (pr,u,kj): out ss [128 keys, 2u, 512-c0] psum.
  - exp: ACT psum -> sbuf, one instr per kj covering both heads. Off-diag
    chunks -> fp8 pt pairs [128, 2kj, 2u, 512]; diag chunks -> bf16 pt
    (pt-quant error only bites concentrated near-diagonal rows) with
    post-exp mask01 multiply on DVE (off ACT's critical path).
  - AV streaming into ctx psum [96, 512] per (pr,u), accumulated across the
    span: off-diag = single-fp8 DR (lhsT = Vbh [128, 2kj, 96]: 64 v-dims +
    ones col at 64 -> Z at psum row 64 + zero pad -- dual-fp8 ldweights
    needs cols%32==0, >=64); diag = bf16 non-DR (lhsT = Vbb [128, 65]).
    Diffuse off-diag rows average out single-fp8 V error; vonly ~1e-3.
  - evict per (pr,u): DVE reciprocal of Z row -> bf16 rz at partition 0
    (cross-partition psum read, hw-verified), gpsimd partition_broadcast ->
    rzbc sbuf, DVE mul ctx*rzbc -> ctx_sb f16 [128=(u,vd), 512] (u1 written
    cross-partition to rows 64:128). Evicts deferred into the next pair's /
    span's kj loop so the PE never stalls on them.
  - outproj: f16, 4x128-contraction per [128q, 512od] psum group; DVE f16
    stage -> DMA (psum DMA is forbidden); host sums g-partials + bias f32.
  - V proj: f16 (value-path precision), psum -> Vbh fp8 + Vbb bf16 copies.
  - schedule: only Q(s+1) is paced as filler through attention(s); K(s) and
    V(s) are JIT-emitted inside span s itself (first consumers are its own
    diag QK/AV), spread across pr0/pr1 kj iterations AFTER each emit_qk --
    K keys must land strictly before kj=4s or the diag QK reads stale kT
    (caused a NaN once). outproj(0)@s2, outproj(1,2)@s3, outproj(3) tail.
    All input DMAs on the SP queue (the DMA bus serializes transfers at
    ~1.45us each; ACT-queue DMAs would block exp dispatch); out DMAs split
    SP/Pool. Evicts defer across pair/span boundaries.
Measured: 200778 ns (TimelineSim), rel err 1.20e-2 (gate 2e-2). Baseline
v3 was 240443 ns. ACT exp busy 146.9us (73%), PE 129us, DVE 118us.
Remaining idle: ~10us startup (DMA-bus chain + proj pipe), ~6us span-0/1
boundary (ss-pool bufs=2 serializes the exp chain across spans; psum is
full: psS 4 + psC 2 + psT 2 banks), small scattered bubbles.
"""

import os
import sys

try:
    import concourse.bass  # noqa: F401
except ImportError:  # pragma: no cover
    sys.path.insert(0, "/opt/trn_rl_repo")

import numpy as np

B, T, D = 4, 2048, 1024
H, HD = 16, 64
NCORES = 8
NPAIR = 4
NSPAN = 4
SPAN = 512
KC = 128
P = 128
LAG = 3

_CACHE = {}


def _build():
    import concourse.bacc as bacc
    import concourse.mybir as mybir
    import concourse.tile as tile

    f32 = mybir.dt.float32
    f16 = mybir.dt.float16
    bf16 = mybir.dt.bfloat16
    fp8 = mybir.dt.float8e4
    u8 = mybir.dt.uint8
    Exp = mybir.ActivationFunctionType.Exp

    DR = mybir.MatmulPerfMode.DoubleRow

    nc = bacc.Bacc("TRN2", target_bir_lowering=False, debug=False,
                   num_devices=NCORES)

    x8_h = nc.dram_tensor("x8", (D, T), u8, kind="ExternalInput")
    xh_h = nc.dram_tensor("xh", (D, T), f16, kind="ExternalInput")
    wq8_h = nc.dram_tensor("wq8", (D, 512), u8, kind="ExternalInput")
    wk8_h = nc.dram_tensor("wk8", (D, 512), u8, kind="ExternalInput")
    wvh_h = nc.dram_tensor("wvh", (D, 512), f16, kind="ExternalInput")
    woh_h = nc.dram_tensor("woh", (512, D), f16, kind="ExternalInput")
    out_h = nc.dram_tensor("out", (T, D), f16, kind="ExternalOutput")

    # x8: D = (dcp 4, k 2, p 128); per span slice on T
    x8_d = x8_h.ap().rearrange("(dcp k p) t -> p dcp k t", p=P, k=2)
    xh_d = xh_h.ap().rearrange("(dc p) t -> p dc t", p=P)
    wq8_d = wq8_h.ap().rearrange("(dcp k p) (t h m) -> p dcp k t h m",
                                 p=P, k=2, t=2, h=2)
    wk8_d = wk8_h.ap().rearrange("(dcp k p) (t h m) -> p dcp k t h m",
                                 p=P, k=2, t=2, h=2)
    wvh_d = wvh_h.ap().rearrange("(dc p) f -> p dc f", p=P)
    woh_d = woh_h.ap().rearrange("(pc p) f -> p pc f", p=P)

    with tile.TileContext(nc) as tc:
        with (
            tc.tile_pool(name="persist", bufs=1) as persist,
            tc.tile_pool(name="x8p", bufs=2) as x8pool,
            tc.tile_pool(name="xhp", bufs=2) as xhpool,
            tc.tile_pool(name="qp", bufs=4) as qpool,
            tc.tile_pool(name="ptp", bufs=6) as ptpool,
            tc.tile_pool(name="ptb", bufs=5) as ptbpool,
            tc.tile_pool(name="rzp", bufs=8) as rzpool,
            tc.tile_pool(name="cp", bufs=8) as cpool,
            tc.tile_pool(name="stg", bufs=6) as stgpool,
            tc.tile_pool(name="psS", bufs=2, space="PSUM") as psS,
            tc.tile_pool(name="psC", bufs=2, space="PSUM") as psC,
            tc.tile_pool(name="psT", bufs=2, space="PSUM") as psT,
        ):
            wq8 = persist.tile([P, 4, 2, 2, 2, P], u8, tag="wq8", name="wq8")
            wk8 = persist.tile([P, 4, 2, 2, 2, P], u8, tag="wk8", name="wk8")
            wvt = persist.tile([P, 8, 512], f16, tag="wvt", name="wvt")
            wot = persist.tile([P, 4, D], f16, tag="wot", name="wot")
            kT8 = [persist.tile([P, 2, T], fp8, tag=f"kT8{t}", name=f"kT8{t}")
                   for t in range(2)]
            # Vb: [p, kjp 8, kj 2, pr 4, u 2, 96]: 64 v-dims + ones col at
            # 64 + zero pad to 96 (dual-fp8 ldweights needs cols % 32 == 0,
            # >= 64; psum rows 65:96 are dead)
            VW = 96
            Vbh = persist.tile([P, 8, 2, NPAIR, 2, VW], fp8, tag="Vbh",
                               name="Vbh")
            # diag-chunk AV runs in bf16 (pt quantization error bites only
            # concentrated near-diagonal attention rows): [p, kc, pr, u, 65]
            Vbb = persist.tile([P, 16, NPAIR, 2, HD + 1], bf16, tag="Vbb",
                               name="Vbb")
            mask01 = persist.tile([P, KC], bf16, tag="mask01", name="mask01")
            one = nc.const_aps.tensor(1.0, (P, 1))

            # ---- initial DMAs ----
            x8ts = {0: x8pool.tile([P, 4, 2, SPAN], u8, tag="x8t", name="x8t0")}
            xhts = {0: xhpool.tile([P, 8, SPAN], f16, tag="xht", name="xht0")}
            nc.sync.dma_start(wq8[:], wq8_d)
            nc.sync.dma_start(x8ts[0][:], x8_d[:, :, :, 0:SPAN])
            nc.sync.dma_start(wk8[:], wk8_d)
            nc.sync.dma_start(xhts[0][:, 0:4], xh_d[:, 0:4, 0:SPAN])
            nc.sync.dma_start(wvt[:], wvh_d)
            nc.sync.dma_start(xhts[0][:, 4:8], xh_d[:, 4:8, 0:SPAN])
            nc.sync.dma_start(wot[:], woh_d)


            # mask01[p, f] = 1 if p <= f else 0 (post-exp diag pt mask)
            nc.gpsimd.memset(mask01[:], 1.0)
            nc.gpsimd.affine_select(
                out=mask01[:], in_=mask01[:],
                compare_op=mybir.AluOpType.is_ge, fill=0.0,
                base=0, channel_multiplier=-1, pattern=[[1, KC]],
            )
            # Vbh pad cols [65:96] must be zero (read by every off-diag
            # AV); data cols are always written before first read
            nslots = 8 * 2 * NPAIR * 2
            nc.gpsimd.memset(
                Vbh[:].rearrange("p a b c d e -> p (a b c d) e")
                [:, :, HD + 1:], 0.0)
            nc.vector.tensor_copy(
                Vbh[:].rearrange("p a b c d e -> p (a b c d) e")
                [:, :, HD:HD + 1],
                one.to_broadcast((P, nslots, 1)))
            nc.vector.tensor_copy(
                Vbb[:].rearrange("p a c d e -> p (a c d) e")
                [:, :, HD:HD + 1],
                one.to_broadcast((P, 16 * NPAIR * 2, 1)))

            qts = {}    # (sp, t) -> [P, 2, SPAN] fp8 tile
            ctxs = {}   # (sp, pr) -> [P, SPAN] f16 tile

            # ---------- emission helpers ----------
            def proj_qk(w8, pr_half, h, sp, x8t, scale, isq, scratch=False):
                t = pr_half

                def emit():
                    if scratch:
                        # startup only: borrow an idle psS slot so the four
                        # critical span-0 groups get independent psums
                        ps = psS.tile([P, 2, SPAN], f32, tag="psS",
                                      name="psqk0")[:, 0, :]
                    else:
                        ps = psT.tile([P, SPAN], f32, tag="tr", name="psqk")
                    for dcp in range(4):
                        nc.tensor.matmul(
                            ps[:], w8[:, dcp, :, t, h, :].bitcast(fp8),
                            x8t[:, dcp, :, :].bitcast(fp8),
                            start=(dcp == 0), stop=(dcp == 3), perf_mode=DR)
                    if isq:
                        if (sp, t) not in qts:
                            qts[(sp, t)] = qpool.tile(
                                [P, 2, SPAN], fp8, tag=f"qT{t}",
                                name=f"qT{t}_{sp}")
                        nc.vector.tensor_scalar_mul(
                            qts[(sp, t)][:, h, :], ps[:], scale)
                    else:
                        nc.vector.tensor_scalar_mul(
                            kT8[t][:, h, sp * SPAN:(sp + 1) * SPAN], ps[:],
                            scale)
                return emit

            def proj_v(sp, tb, xht):
                def emit():
                    ps = psT.tile([P, SPAN], f32, tag="tr", name="psv")
                    for dc in range(8):
                        nc.tensor.matmul(
                            ps[:], xht[:, dc, tb * P:(tb + 1) * P],
                            wvt[:, dc, :],
                            start=(dc == 0), stop=(dc == 7))
                    kc = sp * 4 + tb
                    psv = ps[:].rearrange("p (pr u v) -> p pr u v",
                                          pr=NPAIR, v=HD)
                    nc.vector.tensor_copy(
                        Vbh[:, kc // 2, kc % 2, :, :, 0:HD], psv)
                    nc.vector.tensor_copy(Vbb[:, kc, :, :, 0:HD], psv)
                return emit

            def q_groups(sp, x8t):
                return [proj_qk(wq8, t, h, sp, x8t, 1.0 / 32, True)
                        for t in range(2) for h in range(2)]

            def k_groups(sp, x8t):
                return [proj_qk(wk8, t, h, sp, x8t, 1.0 / 16, False)
                        for t in range(2) for h in range(2)]

            def v_groups(sp, xht):
                return [proj_v(sp, tb, xht) for tb in range(4)]

            def outproj_group(sp, tb, os_, dma_q, ps_ap=None, act_copy=False):
                def emit():
                    if ps_ap is None:
                        ps = psT.tile([P, SPAN], f32, tag="tr", name="pso")[:]
                    else:
                        ps = ps_ap
                    for pc in range(NPAIR):
                        nc.tensor.matmul(
                            ps,
                            ctxs[(sp, pc)][:, (tb - sp * 4) * P:
                                           (tb - sp * 4 + 1) * P],
                            wot[:, pc, os_ * SPAN:(os_ + 1) * SPAN],
                            start=(pc == 0), stop=(pc == NPAIR - 1))
                    stage = stgpool.tile([P, SPAN], f16, tag="st", name="stage")
                    if act_copy:
                        nc.scalar.activation(
                            stage[:], ps, mybir.ActivationFunctionType.Copy)
                    else:
                        nc.vector.tensor_copy(stage[:], ps)
                    dma_q.dma_start(
                        out_h.ap()[tb * P:(tb + 1) * P,
                                   os_ * SPAN:(os_ + 1) * SPAN], stage[:])
                return emit

            def outproj_groups(sp):
                return [outproj_group(sp, tb, os_,
                                      nc.sync if os_ == 0 else nc.gpsimd)
                        for tb in range(sp * 4, (sp + 1) * 4)
                        for os_ in range(2)]

            # ---------- attention for one span ----------
            deferred = []

            def attn_span(s, fillers, vgs=(), kgs=()):
                K = 4 * (s + 1)
                nslot = (K + 4) * NPAIR
                state = {"slot": 0, "fi": 0}

                def pace():
                    tgt = min(len(fillers),
                              len(fillers) * (state["slot"] + 1) // nslot)
                    while state["fi"] < tgt:
                        fillers[state["fi"]]()
                        state["fi"] += 1

                def tick():
                    state["slot"] += 1
                    pace()

                for pr in range(NPAIR):
                    t = pr // 2
                    ctxp = [psC.tile([96, SPAN], f32, tag="ctx",
                                     name=f"ctx{u}") for u in range(2)]
                    ct = cpool.tile([P, SPAN], f16, tag=f"cT{pr}",
                                    name=f"cT{pr}_{s}")
                    ctxs[(s, pr)] = ct
                    pts = {}
                    qt = qts[(s, t)]

                    avn = [0, 0]  # AV matmuls emitted per u (K total each)

                    def emit_qk(kj, pr=pr, t=t, pts=pts, qt=qt):
                        m = kj - 4 * s
                        ss = psS.tile([P, 2, SPAN], f32, tag="psS", name="ss")
                        c0 = 0 if m < 0 else m * KC
                        for u in range(2):
                            b32 = 32 * (2 * (pr % 2) + u)
                            r = slice(b32, b32 + 32)
                            nc.tensor.matmul(
                                ss[:, u, c0:],
                                kT8[t][r, :, kj * KC:(kj + 1) * KC],
                                qt[r, :, c0:],
                                start=True, stop=True, perf_mode=DR,
                                tile_position=(b32, 0))
                        if m < 0:
                            # off-diagonal: fp8 pt pair for DR AV
                            kjp, sl = kj // 2, kj % 2
                            if sl == 0:
                                pts[kjp] = ptpool.tile([P, 2, 2, SPAN], fp8,
                                                       tag="pt", name="pt")
                            nc.scalar.activation(pts[kjp][:, sl, :, :],
                                                 ss[:, :, :], Exp, scale=0.25)
                        else:
                            # diagonal: bf16 pt (exact-ish attention weights
                            # for concentrated rows) + post-exp triangle mask
                            ptb = ptbpool.tile([P, 2, SPAN], bf16,
                                               tag="ptb", name="ptb")
                            pts[("d", kj)] = ptb
                            nc.scalar.activation(ptb[:, :, c0:],
                                                 ss[:, :, c0:], Exp,
                                                 scale=0.25)
                            nc.vector.tensor_mul(
                                ptb[:, :, c0:c0 + KC], ptb[:, :, c0:c0 + KC],
                                mask01[:].rearrange("p (u f) -> p u f", u=1)
                                .to_broadcast((P, 2, KC)))

                    NU = 2 * s + 4  # AV matmuls per u-chain

                    def emit_av8(kjp, pr=pr, pts=pts, ctxp=ctxp, NU=NU):
                        pt = pts.pop(kjp)
                        for u in range(2):
                            nc.tensor.matmul(
                                ctxp[u][:],
                                Vbh[:, kjp, :, pr, u, :],
                                pt[:, :, u, :],
                                start=(avn[u] == 0),
                                stop=(avn[u] == NU - 1),
                                perf_mode=DR, skip_group_check=True)
                            avn[u] += 1

                    def emit_avd(kj, pr=pr, pts=pts, ctxp=ctxp, NU=NU):
                        ptb = pts.pop(("d", kj))
                        c0 = (kj - 4 * s) * KC
                        for u in range(2):
                            nc.tensor.matmul(
                                ctxp[u][0:HD + 1, c0:],
                                Vbb[:, kj, pr, u, :],
                                ptb[:, u, c0:],
                                start=(avn[u] == 0), stop=(avn[u] == NU - 1),
                                skip_group_check=True)
                            avn[u] += 1

                    def evict(pr=pr, ctxp=ctxp, ct=ct, s=s):
                        rz = rzpool.tile([P, 2, SPAN], bf16, tag="rz",
                                         name="rz")
                        rzbc = rzpool.tile([P, 2, SPAN], bf16, tag="rz",
                                           name="rzbc")

                        def fin(u):
                            def run():
                                # 1/Z row: psum row 64 -> sbuf row 0
                                # (cross-partition DVE write, hw-verified)
                                with nc.allow_low_precision(
                                        reason="1/Z bf16: 0.4% on ctx"):
                                    nc.vector.reciprocal(
                                        rz[0:1, u, :], ctxp[u][64:65, :])
                                nc.gpsimd.partition_broadcast(
                                    rzbc[:, u, :], rz[0:1, u, :])
                                nc.vector.tensor_mul(
                                    ct[u * HD:(u + 1) * HD, :],
                                    ctxp[u][0:HD, :], rzbc[0:HD, u, :])
                            return run
                        deferred.append(fin(0))
                        deferred.append(fin(1))

                    # AV work units: (ready_kj, emit_fn); off-diag kjp ready
                    # at its odd kj, diag kj ready at kj
                    units = []
                    for kj2 in range(K):
                        if kj2 < 4 * s:
                            if kj2 % 2 == 1:
                                units.append((kj2, kj2 // 2, emit_av8))
                        else:
                            units.append((kj2, kj2, emit_avd))
                    # JIT placement: V(s) spread across pr0's whole kj
                    # range (consumers: pr0 diag AV); K(s) t0-groups in pr0,
                    # t1-groups in pr1 (consumers: pr0/pr2 diag QK)
                    jit = {}
                    if pr == 0:
                        for i in range(len(vgs)):
                            jit.setdefault(max(i * K // 4, i), []).append(
                                vgs[i])
                        # K(s) t0 keys must precede the diag QK at kj=4s
                        # (jit runs after emit_qk in the same iteration)
                        for i in (0, 1) if kgs else ():
                            jit.setdefault(min(1 + i * (K // 3 + 1),
                                               4 * s - 1), []).append(kgs[i])
                    elif pr == 1:
                        for i in (2, 3) if kgs else ():
                            jit.setdefault(1 + (i - 2) * (K // 3 + 1),
                                           []).append(kgs[i])
                    ui = [0]
                    for kj in range(K):
                        emit_qk(kj)
                        for g in jit.get(kj, ()):
                            g()
                        if deferred:
                            deferred.pop(0)()
                        tick()
                        while (ui[0] < len(units)
                               and units[ui[0]][0] + LAG <= kj):
                            units[ui[0]][2](units[ui[0]][1])
                            ui[0] += 1
                    while ui[0] < len(units):
                        if deferred:
                            deferred.pop(0)()
                        tick()
                        units[ui[0]][2](units[ui[0]][1])
                        ui[0] += 1
                    evict()
                while state["fi"] < len(fillers):
                    fillers[state["fi"]]()
                    state["fi"] += 1

            # ---------- main schedule ----------
            # V(s) and K(s) run inside span s itself (their first consumers
            # are span s's own diag AV / diag QK) -- keeps earlier spans off
            # the PE; only Q(s+1) must finish during span s
            # pr0's attention needs (q,k) t0 groups first: interleave
            qg0, kg0 = q_groups(0, x8ts[0]), k_groups(0, x8ts[0])
            for i in range(4):
                qg0[i]()
                kg0[i]()
            for s in range(NSPAN):
                vgs = v_groups(s, xhts[s])
                kgs = k_groups(s, x8ts[s]) if s >= 1 else ()
                fillers = []
                if s + 1 < NSPAN:
                    x8t = x8pool.tile([P, 4, 2, SPAN], u8, tag="x8t",
                                      name=f"x8t{s + 1}")
                    xht = xhpool.tile([P, 8, SPAN], f16, tag="xht",
                                      name=f"xht{s + 1}")
                    x8ts[s + 1], xhts[s + 1] = x8t, xht
                    sl = slice((s + 1) * SPAN, (s + 2) * SPAN)
                    nc.sync.dma_start(x8t[:], x8_d[:, :, :, sl])
                    nc.sync.dma_start(xht[:, 0:4], xh_d[:, 0:4, sl])
                    nc.sync.dma_start(xht[:, 4:8], xh_d[:, 4:8, sl])
                    fillers += q_groups(s + 1, x8t)
                if s == 2:
                    fillers += outproj_groups(0)
                elif s == 3:
                    fillers += outproj_groups(1) + outproj_groups(2)
                attn_span(s, fillers, vgs, kgs)
            while deferred:
                deferred.pop(0)()
            # tail outproj(3): attention is done, so psS's 4 banks are free
            # scratch -- 6 parallel psums let all 32 matmuls run back-to-back
            # (no psT-rotation stalls, PE stays ramped); stage copies split
            # DVE/ACT (both idle at the tail)
            sc = []
            for i in range(2):
                t = psS.tile([P, 2, SPAN], f32, tag="psS", name=f"osc{i}")
                sc += [t[:, 0, :], t[:, 1, :]]
            tails = [(tb, os_) for tb in range(12, 16) for os_ in range(2)]
            for gi, (tb, os_) in enumerate(tails):
                ps_ap = sc[gi - 2] if 2 <= gi < 6 else None
                outproj_group(3, tb, os_,
                              nc.sync if os_ == 0 else nc.gpsimd,
                              ps_ap=ps_ap, act_copy=(gi % 2 == 1))()

    nc.compile()
    return nc


def get_nc():
    if "nc" not in _CACHE:
        _CACHE["nc"] = _build()
    return _CACHE["nc"]


def _perm512():
    perm = np.empty(512, np.int64)
    i = 0
    for t in range(2):
        for h in range(2):
            for beta in range(4):
                pr = 2 * t + beta // 2
                u = beta % 2
                for dd in range(32):
                    perm[i] = pr * 128 + u * 64 + h * 32 + dd
                    i += 1
    return perm


def kernel(x, Wq, Wk, Wv, Wo, bo):
    import ml_dtypes
    from concourse import bass_utils

    e4 = ml_dtypes.float8_e4m3

    x = np.asarray(x, dtype=np.float32)
    Wq, Wk, Wv = (np.asarray(w, dtype=np.float32) for w in (Wq, Wk, Wv))
    Wo = np.asarray(Wo, dtype=np.float32)
    bo = np.asarray(bo, dtype=np.float32)
    perm = _perm512()

    in_maps = []
    for c in range(NCORES):
        b, g = c // 2, c % 2
        gsl = slice(g * 512, (g + 1) * 512)
        xT = np.ascontiguousarray(x[b].T)
        in_maps.append({
            "x8": xT.astype(e4).view(np.uint8),
            "xh": xT.astype(np.float16),
            # w quantized x16 to dodge e4m3 subnormals; unscaled on-chip
            "wq8": np.ascontiguousarray((16.0 * Wq[gsl].T)[:, perm])
            .astype(e4).view(np.uint8),
            "wk8": np.ascontiguousarray((16.0 * Wk[gsl].T)[:, perm])
            .astype(e4).view(np.uint8),
            "wvh": np.ascontiguousarray(Wv[gsl].T).astype(np.float16),
            "woh": np.ascontiguousarray(Wo[:, gsl].T).astype(np.float16),
        })

    nc = get_nc()
    res = bass_utils.run_bass_kernel_spmd(nc, in_maps,
                                          core_ids=list(range(NCORES)))
    parts = [res.results[c]["out"].astype(np.float32) for c in range(NCORES)]
    out = np.stack([parts[2 * b] + parts[2 * b + 1] + bo for b in range(B)])
    return out.astype(np.float32)


# revision 49
# speedup vs baseline: 1.0340x; 1.0035x over previous
"""Multi-head causal attention (B=4, T=2048, D=1024, H=16) on 8 Trainium2 cores.

Sharding: core c = (b, g) with b = c//2 (batch), g = c%2 (head-group of 8 heads).
Each core: Q/K/V projections for its 8 heads (column-parallel), causal attention,
row-parallel partial output projection. Host sums the g=0/g=1 partials + bias.

v4 design (fp8 DoubleRow + streaming AV; cost model: matmul = out-free-rows x
0.4167ns x cpr, fp8e4 DoubleRow cpr=0.5 contracting 2x128/instr; ACT exp at
0.8333ns/free-elem = ~147us busy is the floor):
  - Q/K proj: fp8 DR, x8 moving [128,2,512], w8 stationary [128,2,128] in 4
    col-groups (t=pair-half, h=dim-half) so psum partitions land as
    (beta=2*(pr%2)+u, dd) blocks of 32 -> qT8/kT8 [32-blocks, 2h, T] fp8 for
    2x32-contraction DR score matmuls. w quantized x16 (e4m3 subnormal
    dodge), unscaled in the DVE psum->fp8 copy; score scale 1/8 folded as
    qT8 = e4m3(q/2) + exp(scale=0.25). Sub-128 DR needs explicit
    tile_position (base-96 slices break base_partition inference).
  - scores: fp8 DR per (pr,u,kj): out ss [128 keys, 2u, 512-c0] psum.
  - exp: ACT psum -> sbuf, one instr per kj covering both heads. Off-diag
    chunks -> fp8 pt pairs [128, 2kj, 2u, 512]; diag chunks -> bf16 pt
    (pt-quant error only bites concentrated near-diagonal rows) with
    post-exp mask01 multiply on DVE (off ACT's critical path).
  - AV streaming into ctx psum [96, 512] per (pr,u), accumulated across the
    span: off-diag = single-fp8 DR (lhsT = Vbh [128, 2kj, 96]: 64 v-dims +
    ones col at 64 -> Z at psum row 64 + zero pad -- dual-fp8 ldweights
    needs cols%32==0, >=64); diag = bf16 non-DR (lhsT = Vbb [128, 65]).
    Diffuse off-diag rows average out single-fp8 V error; vonly ~1e-3.
  - evict per (pr,u): DVE reciprocal of Z row -> bf16 rz at partition 0
    (cross-partition psum read, hw-verified), gpsimd partition_broadcast ->
    rzbc sbuf, DVE mul ctx*rzbc -> ctx_sb f16 [128=(u,vd), 512] (u1 written
    cross-partition to rows 64:128). Evicts deferred into the next pair's /
    span's kj loop so the PE never stalls on them.
  - outproj: f16, 4x128-contraction per [128q, 512od] psum group; DVE f16
    stage -> DMA (psum DMA is forbidden); host sums g-partials + bias f32.
  - V proj: f16 (value-path precision), psum -> Vbh fp8 + Vbb bf16 copies.
  - schedule: only Q(s+1) is paced as filler through attention(s); K(s) and
    V(s) are JIT-emitted inside span s itself (first consumers are its own
    diag QK/AV), spread across pr0/pr1 kj iterations AFTER each emit_qk --
    K keys must land strictly before kj=4s or the diag QK reads stale kT
    (caused a NaN once). outproj(0)@s2, outproj(1,2)@s3, outproj(3) tail.
    All input DMAs on the SP queue (the DMA bus serializes transfers at
    ~1.45us each; ACT-queue DMAs would block exp dispatch); out DMAs split
    SP/Pool. Evicts defer across pair/span boundaries.
Measured: 200778 ns (TimelineSim), rel err 1.20e-2 (gate 2e-2). Baseline
v3 was 240443 ns. ACT exp busy 146.9us (73%), PE 129us, DVE 118us.
Remaining idle: ~10us startup (DMA-bus chain + proj pipe), ~6us span-0/1
boundary (ss-pool bufs=2 serializes the exp chain across spans; psum is
full: psS 4 + psC 2 + psT 2 banks), small scattered bubbles.
"""

import os
import sys

try:
    import concourse.bass  # noqa: F401
except ImportError:  # pragma: no cover
    sys.path.insert(0, "/opt/trn_rl_repo")

import numpy as np

B, T, D = 4, 2048, 1024
H, HD = 16, 64
NCORES = 8
NPAIR = 4
NSPAN = 4
SPAN = 512
KC = 128
P = 128
LAG = 3

_CACHE = {}


def _build():
    import concourse.bacc as bacc
    import concourse.mybir as mybir
    import concourse.tile as tile

    f32 = mybir.dt.float32
    f16 = mybir.dt.float16
    bf16 = mybir.dt.bfloat16
    fp8 = mybir.dt.float8e4
    u8 = mybir.dt.uint8
    Exp = mybir.ActivationFunctionType.Exp

    DR = mybir.MatmulPerfMode.DoubleRow

    nc = bacc.Bacc("TRN2", target_bir_lowering=False, debug=False,
                   num_devices=NCORES)

    x8_h = nc.dram_tensor("x8", (D, T), u8, kind="ExternalInput")
    xh_h = nc.dram_tensor("xh", (D, T), f16, kind="ExternalInput")
    wq8_h = nc.dram_tensor("wq8", (D, 512), u8, kind="ExternalInput")
    wk8_h = nc.dram_tensor("wk8", (D, 512), u8, kind="ExternalInput")
    wvh_h = nc.dram_tensor("wvh", (D, 512), f16, kind="ExternalInput")
    woh_h = nc.dram_tensor("woh", (512, D), f16, kind="ExternalInput")
    out_h = nc.dram_tensor("out", (T, D), f16, kind="ExternalOutput")

    # x8: D = (dcp 4, k 2, p 128); per span slice on T
    x8_d = x8_h.ap().rearrange("(dcp k p) t -> p dcp k t", p=P, k=2)
    xh_d = xh_h.ap().rearrange("(dc p) t -> p dc t", p=P)
    wq8_d = wq8_h.ap().rearrange("(dcp k p) (t h m) -> p dcp k t h m",
                                 p=P, k=2, t=2, h=2)
    wk8_d = wk8_h.ap().rearrange("(dcp k p) (t h m) -> p dcp k t h m",
                                 p=P, k=2, t=2, h=2)
    wvh_d = wvh_h.ap().rearrange("(dc p) f -> p dc f", p=P)
    woh_d = woh_h.ap().rearrange("(pc p) f -> p pc f", p=P)

    with tile.TileContext(nc) as tc:
        with (
            tc.tile_pool(name="persist", bufs=1) as persist,
            tc.tile_pool(name="x8p", bufs=2) as x8pool,
            tc.tile_pool(name="xhp", bufs=2) as xhpool,
            tc.tile_pool(name="qp", bufs=4) as qpool,
            tc.tile_pool(name="ptp", bufs=6) as ptpool,
            tc.tile_pool(name="ptb", bufs=5) as ptbpool,
            tc.tile_pool(name="rzp", bufs=8) as rzpool,
            tc.tile_pool(name="cp", bufs=8) as cpool,
            tc.tile_pool(name="stg", bufs=6) as stgpool,
            tc.tile_pool(name="psS", bufs=2, space="PSUM") as psS,
            tc.tile_pool(name="psC", bufs=2, space="PSUM") as psC,
            tc.tile_pool(name="psT", bufs=2, space="PSUM") as psT,
        ):
            wq8 = persist.tile([P, 4, 2, 2, 2, P], u8, tag="wq8", name="wq8")
            wk8 = persist.tile([P, 4, 2, 2, 2, P], u8, tag="wk8", name="wk8")
            wvt = persist.tile([P, 8, 512], f16, tag="wvt", name="wvt")
            wot = persist.tile([P, 4, D], f16, tag="wot", name="wot")
            kT8 = [persist.tile([P, 2, T], fp8, tag=f"kT8{t}", name=f"kT8{t}")
                   for t in range(2)]
            # Vb: [p, kjp 8, kj 2, pr 4, u 2, 96]: 64 v-dims + ones col at
            # 64 + zero pad to 96 (dual-fp8 ldweights needs cols % 32 == 0,
            # >= 64; psum rows 65:96 are dead)
            VW = 96
            Vbh = persist.tile([P, 8, 2, NPAIR, 2, VW], fp8, tag="Vbh",
                               name="Vbh")
            # diag-chunk AV runs in bf16 (pt quantization error bites only
            # concentrated near-diagonal attention rows): [p, kc, pr, u, 65]
            Vbb = persist.tile([P, 16, NPAIR, 2, HD + 1], bf16, tag="Vbb",
                               name="Vbb")
            mask01 = persist.tile([P, KC], bf16, tag="mask01", name="mask01")
            one = nc.const_aps.tensor(1.0, (P, 1))

            # ---- initial DMAs ----
            x8ts = {0: x8pool.tile([P, 4, 2, SPAN], u8, tag="x8t", name="x8t0")}
            xhts = {0: xhpool.tile([P, 8, SPAN], f16, tag="xht", name="xht0")}
            nc.sync.dma_start(wq8[:], wq8_d)
            nc.sync.dma_start(x8ts[0][:], x8_d[:, :, :, 0:SPAN])
            nc.sync.dma_start(wk8[:], wk8_d)
            nc.sync.dma_start(xhts[0][:, 0:4], xh_d[:, 0:4, 0:SPAN])
            nc.sync.dma_start(wvt[:], wvh_d)
            nc.sync.dma_start(xhts[0][:, 4:8], xh_d[:, 4:8, 0:SPAN])
            nc.sync.dma_start(wot[:], woh_d)


            # mask01[p, f] = 1 if p <= f else 0 (post-exp diag pt mask)
            nc.gpsimd.memset(mask01[:], 1.0)
            nc.gpsimd.affine_select(
                out=mask01[:], in_=mask01[:],
                compare_op=mybir.AluOpType.is_ge, fill=0.0,
                base=0, channel_multiplier=-1, pattern=[[1, KC]],
            )
            # Vbh pad cols [65:96] must be zero (read by every off-diag
            # AV); data cols are always written before first read
            nslots = 8 * 2 * NPAIR * 2
            nc.gpsimd.memset(
                Vbh[:].rearrange("p a b c d e -> p (a b c d) e")
                [:, :, HD + 1:], 0.0)
            nc.vector.tensor_copy(
                Vbh[:].rearrange("p a b c d e -> p (a b c d) e")
                [:, :, HD:HD + 1],
                one.to_broadcast((P, nslots, 1)))
            nc.vector.tensor_copy(
                Vbb[:].rearrange("p a c d e -> p (a c d) e")
                [:, :, HD:HD + 1],
                one.to_broadcast((P, 16 * NPAIR * 2, 1)))

            qts = {}    # (sp, t) -> [P, 2, SPAN] fp8 tile
            ctxs = {}   # (sp, pr) -> [P, SPAN] f16 tile

            # ---------- emission helpers ----------
            def proj_qk(w8, pr_half, h, sp, x8t, scale, isq, scratch=False):
                t = pr_half

                def emit():
                    if scratch:
                        # startup only: borrow an idle psS slot so the four
                        # critical span-0 groups get independent psums
                        ps = psS.tile([P, 2, SPAN], f32, tag="psS",
                                      name="psqk0")[:, 0, :]
                    else:
                        ps = psT.tile([P, SPAN], f32, tag="tr", name="psqk")
                    for dcp in range(4):
                        nc.tensor.matmul(
                            ps[:], w8[:, dcp, :, t, h, :].bitcast(fp8),
                            x8t[:, dcp, :, :].bitcast(fp8),
                            start=(dcp == 0), stop=(dcp == 3), perf_mode=DR)
                    if isq:
                        if (sp, t) not in qts:
                            qts[(sp, t)] = qpool.tile(
                                [P, 2, SPAN], fp8, tag=f"qT{t}",
                                name=f"qT{t}_{sp}")
                        nc.vector.tensor_scalar_mul(
                            qts[(sp, t)][:, h, :], ps[:], scale)
                    else:
                        nc.vector.tensor_scalar_mul(
                            kT8[t][:, h, sp * SPAN:(sp + 1) * SPAN], ps[:],
                            scale)
                return emit

            def proj_v(sp, tb, xht):
                def emit():
                    ps = psT.tile([P, SPAN], f32, tag="tr", name="psv")
                    for dc in range(8):
                        nc.tensor.matmul(
                            ps[:], xht[:, dc, tb * P:(tb + 1) * P],
                            wvt[:, dc, :],
                            start=(dc == 0), stop=(dc == 7))
                    kc = sp * 4 + tb
                    psv = ps[:].rearrange("p (pr u v) -> p pr u v",
                                          pr=NPAIR, v=HD)
                    nc.vector.tensor_copy(
                        Vbh[:, kc // 2, kc % 2, :, :, 0:HD], psv)
                    nc.vector.tensor_copy(Vbb[:, kc, :, :, 0:HD], psv)
                return emit

            def q_groups(sp, x8t):
                return [proj_qk(wq8, t, h, sp, x8t, 1.0 / 32, True)
                        for t in range(2) for h in range(2)]

            def k_groups(sp, x8t):
                return [proj_qk(wk8, t, h, sp, x8t, 1.0 / 16, False)
                        for t in range(2) for h in range(2)]

            def v_groups(sp, xht):
                return [proj_v(sp, tb, xht) for tb in range(4)]

            def outproj_group(sp, tb, os_, dma_q, ps_ap=None, act_copy=False):
                def emit():
                    if ps_ap is None:
                        ps = psT.tile([P, SPAN], f32, tag="tr", name="pso")[:]
                    else:
                        ps = ps_ap
                    for pc in range(NPAIR):
                        nc.tensor.matmul(
                            ps,
                            ctxs[(sp, pc)][:, (tb - sp * 4) * P:
                                           (tb - sp * 4 + 1) * P],
                            wot[:, pc, os_ * SPAN:(os_ + 1) * SPAN],
                            start=(pc == 0), stop=(pc == NPAIR - 1))
                    stage = stgpool.tile([P, SPAN], f16, tag="st", name="stage")
                    if act_copy:
                        nc.scalar.activation(
                            stage[:], ps, mybir.ActivationFunctionType.Copy)
                    else:
                        nc.vector.tensor_copy(stage[:], ps)
                    dma_q.dma_start(
                        out_h.ap()[tb * P:(tb + 1) * P,
                                   os_ * SPAN:(os_ + 1) * SPAN], stage[:])
                return emit

            def outproj_groups(sp):
                return [outproj_group(sp, tb, os_,
                                      nc.sync if os_ == 0 else nc.gpsimd)
                        for tb in range(sp * 4, (sp + 1) * 4)
                        for os_ in range(2)]

            # ---------- attention for one span ----------
            deferred = []

            def attn_span(s, fillers, vgs=(), kgs=()):
                K = 4 * (s + 1)
                nslot = (K + 4) * NPAIR
                state = {"slot": 0, "fi": 0}

                def pace():
                    tgt = min(len(fillers),
                              len(fillers) * (state["slot"] + 1) // nslot)
                    while state["fi"] < tgt:
                        fillers[state["fi"]]()
                        state["fi"] += 1

                def tick():
                    state["slot"] += 1
                    pace()

                for pr in range(NPAIR):
                    t = pr // 2
                    ctxp = [psC.tile([96, SPAN], f32, tag="ctx",
                                     name=f"ctx{u}") for u in range(2)]
                    ct = cpool.tile([P, SPAN], f16, tag=f"cT{pr}",
                                    name=f"cT{pr}_{s}")
                    ctxs[(s, pr)] = ct
                    pts = {}
                    qt = qts[(s, t)]

                    avn = [0, 0]  # AV matmuls emitted per u (K total each)

                    def emit_qk(kj, pr=pr, t=t, pts=pts, qt=qt):
                        m = kj - 4 * s
                        ss = psS.tile([P, 2, SPAN], f32, tag="psS", name="ss")
                        c0 = 0 if m < 0 else m * KC
                        for u in range(2):
                            b32 = 32 * (2 * (pr % 2) + u)
                            r = slice(b32, b32 + 32)
                            nc.tensor.matmul(
                                ss[:, u, c0:],
                                kT8[t][r, :, kj * KC:(kj + 1) * KC],
                                qt[r, :, c0:],
                                start=True, stop=True, perf_mode=DR,
                                tile_position=(b32, 0))
                        if m < 0:
                            # off-diagonal: fp8 pt pair for DR AV
                            kjp, sl = kj // 2, kj % 2
                            if sl == 0:
                                pts[kjp] = ptpool.tile([P, 2, 2, SPAN], fp8,
                                                       tag="pt", name="pt")
                            nc.scalar.activation(pts[kjp][:, sl, :, :],
                                                 ss[:, :, :], Exp, scale=0.25)
                        else:
                            # diagonal: bf16 pt (exact-ish attention weights
                            # for concentrated rows) + post-exp triangle mask
                            ptb = ptbpool.tile([P, 2, SPAN], bf16,
                                               tag="ptb", name="ptb")
                            pts[("d", kj)] = ptb
                            nc.scalar.activation(ptb[:, :, c0:],
                                                 ss[:, :, c0:], Exp,
                                                 scale=0.25)
                            nc.vector.tensor_mul(
                                ptb[:, :, c0:c0 + KC], ptb[:, :, c0:c0 + KC],
                                mask01[:].rearrange("p (u f) -> p u f", u=1)
                                .to_broadcast((P, 2, KC)))

                    NU = 2 * s + 4  # AV matmuls per u-chain

                    def emit_av8(kjp, pr=pr, pts=pts, ctxp=ctxp, NU=NU):
                        pt = pts.pop(kjp)
                        for u in range(2):
                            nc.tensor.matmul(
                                ctxp[u][:],
                                Vbh[:, kjp, :, pr, u, :],
                                pt[:, :, u, :],
                                start=(avn[u] == 0),
                                stop=(avn[u] == NU - 1),
                                perf_mode=DR, skip_group_check=True)
                            avn[u] += 1

                    def emit_avd(kj, pr=pr, pts=pts, ctxp=ctxp, NU=NU):
                        ptb = pts.pop(("d", kj))
                        c0 = (kj - 4 * s) * KC
                        for u in range(2):
                            nc.tensor.matmul(
                                ctxp[u][0:HD + 1, c0:],
                                Vbb[:, kj, pr, u, :],
                                ptb[:, u, c0:],
                                start=(avn[u] == 0), stop=(avn[u] == NU - 1),
                                skip_group_check=True)
                            avn[u] += 1

                    def evict(pr=pr, ctxp=ctxp, ct=ct, s=s):
                        rz = rzpool.tile([P, 2, SPAN], bf16, tag="rz",
                                         name="rz")
                        rzbc = rzpool.tile([P, 2, SPAN], bf16, tag="rz",
                                           name="rzbc")

                        def fin(u):
                            def run():
                                # 1/Z row: psum row 64 -> sbuf row 0
                                # (cross-partition DVE write, hw-verified)
                                with nc.allow_low_precision(
                                        reason="1/Z bf16: 0.4% on ctx"):
                                    nc.vector.reciprocal(
                                        rz[0:1, u, :], ctxp[u][64:65, :])
                                nc.gpsimd.partition_broadcast(
                                    rzbc[:, u, :], rz[0:1, u, :])
                                nc.vector.tensor_mul(
                                    ct[u * HD:(u + 1) * HD, :],
                                    ctxp[u][0:HD, :], rzbc[0:HD, u, :])
                            return run
                        deferred.append(fin(0))
                        deferred.append(fin(1))

                    # AV work units: (ready_kj, emit_fn); off-diag kjp ready
                    # at its odd kj, diag kj ready at kj
                    units = []
                    for kj2 in range(K):
                        if kj2 < 4 * s:
                            if kj2 % 2 == 1:
                                units.append((kj2, kj2 // 2, emit_av8))
                        else:
                            units.append((kj2, kj2, emit_avd))
                    # JIT placement: V(s) spread across pr0's whole kj
                    # range (consumers: pr0 diag AV); K(s) t0-groups in pr0,
                    # t1-groups in pr1 (consumers: pr0/pr2 diag QK)
                    jit = {}
                    if pr == 0:
                        for i in range(len(vgs)):
                            jit.setdefault(max(i * K // 4, i), []).append(
                                vgs[i])
                        # K(s) t0 keys must precede the diag QK at kj=4s
                        # (jit runs after emit_qk in the same iteration)
                        for i in (0, 1) if kgs else ():
                            jit.setdefault(min(1 + i * (K // 3 + 1),
                                               4 * s - 1), []).append(kgs[i])
                    elif pr == 1:
                        for i in (2, 3) if kgs else ():
                            jit.setdefault(1 + (i - 2) * (K // 3 + 1),
                                           []).append(kgs[i])
                    ui = [0]
                    for kj in range(K):
                        emit_qk(kj)
                        for g in jit.get(kj, ()):
                            g()
                        if deferred:
                            deferred.pop(0)()
                        tick()
                        while (ui[0] < len(units)
                               and units[ui[0]][0] + LAG <= kj):
                            units[ui[0]][2](units[ui[0]][1])
                            ui[0] += 1
                    while ui[0] < len(units):
                        if deferred:
                            deferred.pop(0)()
                        tick()
                        units[ui[0]][2](units[ui[0]][1])
                        ui[0] += 1
                    evict()
                while state["fi"] < len(fillers):
                    fillers[state["fi"]]()
                    state["fi"] += 1

            # ---------- main schedule ----------
            # V(s) and K(s) run inside span s itself (their first consumers
            # are span s's own diag AV / diag QK) -- keeps earlier spans off
            # the PE; only Q(s+1) must finish during span s
            # pr0's attention needs (q,k) t0 groups first: interleave
            qg0, kg0 = q_groups(0, x8ts[0]), k_groups(0, x8ts[0])
            for i in range(4):
                qg0[i]()
                kg0[i]()
            for s in range(NSPAN):
                vgs = v_groups(s, xhts[s])
                kgs = k_groups(s, x8ts[s]) if s >= 1 else ()
                fillers = []
                if s + 1 < NSPAN:
                    x8t = x8pool.tile([P, 4, 2, SPAN], u8, tag="x8t",
                                      name=f"x8t{s + 1}")
                    xht = xhpool.tile([P, 8, SPAN], f16, tag="xht",
                                      name=f"xht{s + 1}")
                    x8ts[s + 1], xhts[s + 1] = x8t, xht
                    sl = slice((s + 1) * SPAN, (s + 2) * SPAN)
                    nc.sync.dma_start(x8t[:], x8_d[:, :, :, sl])
                    nc.sync.dma_start(xht[:, 0:4], xh_d[:, 0:4, sl])
                    nc.sync.dma_start(xht[:, 4:8], xh_d[:, 4:8, sl])
                    fillers += q_groups(s + 1, x8t)
                if s == 2:
                    fillers += outproj_groups(0)
                elif s == 3:
                    fillers += outproj_groups(1) + outproj_groups(2)
                attn_span(s, fillers, vgs, kgs)
            while deferred:
                deferred.pop(0)()
            # tail outproj(3): attention is done, so psS's 4 banks are free
            # scratch -- 6 parallel psums let all 32 matmuls run back-to-back
            # (no psT-rotation stalls, PE stays ramped); stage copies split
            # DVE/ACT (both idle at the tail)
            sc = []
            for i in range(2):
                t = psS.tile([P, 2, SPAN], f32, tag="psS", name=f"osc{i}")
                sc += [t[:, 0, :], t[:, 1, :]]
            for i in range(2):
                sc.append(psC.tile([P, SPAN], f32, tag="ctx",
                                   name=f"oscc{i}")[:])
            tails = [(tb, os_) for tb in range(12, 16) for os_ in range(2)]
            for gi, (tb, os_) in enumerate(tails):
                ps_ap = sc[gi - 2] if 2 <= gi < 8 else None
                outproj_group(3, tb, os_,
                              nc.sync if os_ == 0 else nc.gpsimd,
                              ps_ap=ps_ap, act_copy=(gi % 2 == 1))()

    nc.compile()
    return nc


def get_nc():
    if "nc" not in _CACHE:
        _CACHE["nc"] = _build()
    return _CACHE["nc"]


def _perm512():
    perm = np.empty(512, np.int64)
    i = 0
    for t in range(2):
        for h in range(2):
            for beta in range(4):
                pr = 2 * t + beta // 2
                u = beta % 2
                for dd in range(32):
                    perm[i] = pr * 128 + u * 64 + h * 32 + dd
                    i += 1
    return perm


def kernel(x, Wq, Wk, Wv, Wo, bo):
    import ml_dtypes
    from concourse import bass_utils

    e4 = ml_dtypes.float8_e4m3

    x = np.asarray(x, dtype=np.float32)
    Wq, Wk, Wv = (np.asarray(w, dtype=np.float32) for w in (Wq, Wk, Wv))
    Wo = np.asarray(Wo, dtype=np.float32)
    bo = np.asarray(bo, dtype=np.float32)
    perm = _perm512()

    in_maps = []
    for c in range(NCORES):
        b, g = c // 2, c % 2
        gsl = slice(g * 512, (g + 1) * 512)
        xT = np.ascontiguousarray(x[b].T)
        in_maps.append({
            "x8": xT.astype(e4).view(np.uint8),
            "xh": xT.astype(np.float16),
            # w quantized x16 to dodge e4m3 subnormals; unscaled on-chip
            "wq8": np.ascontiguousarray((16.0 * Wq[gsl].T)[:, perm])
            .astype(e4).view(np.uint8),
            "wk8": np.ascontiguousarray((16.0 * Wk[gsl].T)[:, perm])
            .astype(e4).view(np.uint8),
            "wvh": np.ascontiguousarray(Wv[gsl].T).astype(np.float16),
            "woh": np.ascontiguousarray(Wo[:, gsl].T).astype(np.float16),
        })

    nc = get_nc()
    res = bass_utils.run_bass_kernel_spmd(nc, in_maps,
                                          core_ids=list(range(NCORES)))
    parts = [res.results[c]["out"].astype(np.float32) for c in range(NCORES)]
    out = np.stack([parts[2 * b] + parts[2 * b + 1] + bo for b in range(B)])
    return out.astype(np.float32)


# revision 53
# speedup vs baseline: 1.0392x; 1.0051x over previous
"""Multi-head causal attention (B=4, T=2048, D=1024, H=16) on 8 Trainium2 cores.

Sharding: core c = (b, g) with b = c//2 (batch), g = c%2 (head-group of 8 heads).
Each core: Q/K/V projections for its 8 heads (column-parallel), causal attention,
row-parallel partial output projection. Host sums the g=0/g=1 partials + bias.

v4 design (fp8 DoubleRow + streaming AV; cost model: matmul = out-free-rows x
0.4167ns x cpr, fp8e4 DoubleRow cpr=0.5 contracting 2x128/instr; ACT exp at
0.8333ns/free-elem = ~147us busy is the floor):
  - Q/K proj: fp8 DR, x8 moving [128,2,512], w8 stationary [128,2,128] in 4
    col-groups (t=pair-half, h=dim-half) so psum partitions land as
    (beta=2*(pr%2)+u, dd) blocks of 32 -> qT8/kT8 [32-blocks, 2h, T] fp8 for
    2x32-contraction DR score matmuls. w quantized x16 (e4m3 subnormal
    dodge), unscaled in the DVE psum->fp8 copy; score scale 1/8 folded as
    qT8 = e4m3(q/2) + exp(scale=0.25). Sub-128 DR needs explicit
    tile_position (base-96 slices break base_partition inference).
  - scores: fp8 DR per (pr,u,kj): out ss [128 keys, 2u, 512-c0] psum.
  - exp: ACT psum -> sbuf, one instr per kj covering both heads. Off-diag
    chunks -> fp8 pt pairs [128, 2kj, 2u, 512]; diag chunks -> bf16 pt
    (pt-quant error only bites concentrated near-diagonal rows) with
    post-exp mask01 multiply on DVE (off ACT's critical path).
  - AV streaming into ctx psum [96, 512] per (pr,u), accumulated across the
    span: off-diag = single-fp8 DR (lhsT = Vbh [128, 2kj, 96]: 64 v-dims +
    ones col at 64 -> Z at psum row 64 + zero pad -- dual-fp8 ldweights
    needs cols%32==0, >=64); diag = bf16 non-DR (lhsT = Vbb [128, 65]).
    Diffuse off-diag rows average out single-fp8 V error; vonly ~1e-3.
  - evict per (pr,u): DVE reciprocal of Z row -> bf16 rz at partition 0
    (cross-partition psum read, hw-verified), gpsimd partition_broadcast ->
    rzbc sbuf, DVE mul ctx*rzbc -> ctx_sb f16 [128=(u,vd), 512] (u1 written
    cross-partition to rows 64:128). Evicts deferred into the next pair's /
    span's kj loop so the PE never stalls on them.
  - outproj: f16, 4x128-contraction per [128q, 512od] psum group; DVE f16
    stage -> DMA (psum DMA is forbidden); host sums g-partials + bias f32.
  - V proj: f16 (value-path precision), psum -> Vbh fp8 + Vbb bf16 copies.
  - schedule: only Q(s+1) is paced as filler through attention(s); K(s) and
    V(s) are JIT-emitted inside span s itself (first consumers are its own
    diag QK/AV), spread across pr0/pr1 kj iterations AFTER each emit_qk --
    K keys must land strictly before kj=4s or the diag QK reads stale kT
    (caused a NaN once). outproj(0)@s2, outproj(1,2)@s3, outproj(3) tail.
    All input DMAs on the SP queue (the DMA bus serializes transfers at
    ~1.45us each; ACT-queue DMAs would block exp dispatch); out DMAs split
    SP/Pool. Evicts defer across pair/span boundaries.
Measured: 196136 ns (TimelineSim), rel err 1.20e-2 (gate 2e-2). Baseline
v3 was 240443 ns. ACT exp busy 146.9us (73%), PE 129us, DVE 118us.
Remaining idle: ~10us startup (DMA-bus chain + proj pipe), ~6us span-0/1
boundary (ss-pool bufs=2 serializes the exp chain across spans; psum is
full: psS 4 + psC 2 + psT 2 banks), small scattered bubbles.
"""

import os
import sys

try:
    import concourse.bass  # noqa: F401
except ImportError:  # pragma: no cover
    sys.path.insert(0, "/opt/trn_rl_repo")

import numpy as np

B, T, D = 4, 2048, 1024
H, HD = 16, 64
NCORES = 8
NPAIR = 4
NSPAN = 4
SPAN = 512
KC = 128
P = 128
LAG = 3

_CACHE = {}


def _build():
    import concourse.bacc as bacc
    import concourse.mybir as mybir
    import concourse.tile as tile

    f32 = mybir.dt.float32
    f16 = mybir.dt.float16
    bf16 = mybir.dt.bfloat16
    fp8 = mybir.dt.float8e4
    u8 = mybir.dt.uint8
    Exp = mybir.ActivationFunctionType.Exp

    DR = mybir.MatmulPerfMode.DoubleRow

    nc = bacc.Bacc("TRN2", target_bir_lowering=False, debug=False,
                   num_devices=NCORES)

    x8_h = nc.dram_tensor("x8", (D, T), u8, kind="ExternalInput")
    x8l_h = nc.dram_tensor("x8l", (D, T), u8, kind="ExternalInput")
    wq8_h = nc.dram_tensor("wq8", (D, 512), u8, kind="ExternalInput")
    wk8_h = nc.dram_tensor("wk8", (D, 512), u8, kind="ExternalInput")
    wv8h_h = nc.dram_tensor("wv8h", (D, 512), u8, kind="ExternalInput")
    wv8l_h = nc.dram_tensor("wv8l", (D, 512), u8, kind="ExternalInput")
    woh_h = nc.dram_tensor("woh", (512, D), f16, kind="ExternalInput")
    out_h = nc.dram_tensor("out", (T, D), f16, kind="ExternalOutput")

    # x8: D = (dcp 4, k 2, p 128); per span slice on T
    x8_d = x8_h.ap().rearrange("(dcp k p) t -> p dcp k t", p=P, k=2)
    x8l_d = x8l_h.ap().rearrange("(dcp k p) t -> p dcp k t", p=P, k=2)
    wq8_d = wq8_h.ap().rearrange("(dcp k p) (t h m) -> p dcp k t h m",
                                 p=P, k=2, t=2, h=2)
    wk8_d = wk8_h.ap().rearrange("(dcp k p) (t h m) -> p dcp k t h m",
                                 p=P, k=2, t=2, h=2)
    wv8h_d = wv8h_h.ap().rearrange("(dcp k p) f -> p dcp k f", p=P, k=2)
    wv8l_d = wv8l_h.ap().rearrange("(dcp k p) f -> p dcp k f", p=P, k=2)
    woh_d = woh_h.ap().rearrange("(pc p) f -> p pc f", p=P)

    with tile.TileContext(nc) as tc:
        with (
            tc.tile_pool(name="persist", bufs=1) as persist,
            tc.tile_pool(name="x8p", bufs=2) as x8pool,
            tc.tile_pool(name="xlp", bufs=2) as xlpool,
            tc.tile_pool(name="qp", bufs=4) as qpool,
            tc.tile_pool(name="ptp", bufs=6) as ptpool,
            tc.tile_pool(name="ptb", bufs=5) as ptbpool,
            tc.tile_pool(name="rzp", bufs=8) as rzpool,
            tc.tile_pool(name="cp", bufs=8) as cpool,
            tc.tile_pool(name="stg", bufs=6) as stgpool,
            tc.tile_pool(name="psS", bufs=2, space="PSUM") as psS,
            tc.tile_pool(name="psC", bufs=2, space="PSUM") as psC,
            tc.tile_pool(name="psT", bufs=2, space="PSUM") as psT,
        ):
            wq8 = persist.tile([P, 4, 2, 2, 2, P], u8, tag="wq8", name="wq8")
            wk8 = persist.tile([P, 4, 2, 2, 2, P], u8, tag="wk8", name="wk8")
            wv8 = persist.tile([P, 4, 2, 2, 512], u8, tag="wv8", name="wv8")
            wot = persist.tile([P, 4, D], f16, tag="wot", name="wot")
            kT8 = [persist.tile([P, 2, T], fp8, tag=f"kT8{t}", name=f"kT8{t}")
                   for t in range(2)]
            # Vb: [p, kjp 8, kj 2, pr 4, u 2, 96]: 64 v-dims + ones col at
            # 64 + zero pad to 96 (dual-fp8 ldweights needs cols % 32 == 0,
            # >= 64; psum rows 65:96 are dead)
            VW = 96
            Vbh = persist.tile([P, 8, 2, NPAIR, 2, VW], fp8, tag="Vbh",
                               name="Vbh")
            # diag-chunk AV runs in bf16 (pt quantization error bites only
            # concentrated near-diagonal attention rows): [p, kc, pr, u, 65]
            Vbb = persist.tile([P, 16, NPAIR, 2, HD + 1], bf16, tag="Vbb",
                               name="Vbb")
            mask01 = persist.tile([P, KC], bf16, tag="mask01", name="mask01")
            one = nc.const_aps.tensor(1.0, (P, 1))

            # ---- initial DMAs ----
            x8ts = {0: x8pool.tile([P, 4, 2, SPAN], u8, tag="x8t", name="x8t0")}
            x8ls = {0: xlpool.tile([P, 4, 2, SPAN], u8, tag="x8l", name="x8l0")}
            nc.sync.dma_start(wq8[:], wq8_d)
            nc.sync.dma_start(x8ts[0][:], x8_d[:, :, :, 0:SPAN])
            nc.sync.dma_start(wk8[:], wk8_d)
            nc.sync.dma_start(wv8[:, :, :, 0, :], wv8h_d)
            nc.sync.dma_start(x8ls[0][:], x8l_d[:, :, :, 0:SPAN])
            nc.sync.dma_start(wv8[:, :, :, 1, :], wv8l_d)
            nc.sync.dma_start(wot[:], woh_d)


            # mask01[p, f] = 1 if p <= f else 0 (post-exp diag pt mask)
            nc.gpsimd.memset(mask01[:], 1.0)
            nc.gpsimd.affine_select(
                out=mask01[:], in_=mask01[:],
                compare_op=mybir.AluOpType.is_ge, fill=0.0,
                base=0, channel_multiplier=-1, pattern=[[1, KC]],
            )
            # Vbh pad cols [65:96] must be zero (read by every off-diag
            # AV); data cols are always written before first read
            nslots = 8 * 2 * NPAIR * 2
            nc.gpsimd.memset(
                Vbh[:].rearrange("p a b c d e -> p (a b c d) e")
                [:, :, HD + 1:], 0.0)
            nc.vector.tensor_copy(
                Vbh[:].rearrange("p a b c d e -> p (a b c d) e")
                [:, :, HD:HD + 1],
                one.to_broadcast((P, nslots, 1)))
            nc.vector.tensor_copy(
                Vbb[:].rearrange("p a c d e -> p (a c d) e")
                [:, :, HD:HD + 1],
                one.to_broadcast((P, 16 * NPAIR * 2, 1)))

            qts = {}    # (sp, t) -> [P, 2, SPAN] fp8 tile
            ctxs = {}   # (sp, pr) -> [P, SPAN] f16 tile

            # ---------- emission helpers ----------
            def proj_qk(w8, pr_half, h, sp, x8t, scale, isq, scratch=False):
                t = pr_half

                def emit():
                    if scratch:
                        # startup only: borrow an idle psS slot so the four
                        # critical span-0 groups get independent psums
                        ps = psS.tile([P, 2, SPAN], f32, tag="psS",
                                      name="psqk0")[:, 0, :]
                    else:
                        ps = psT.tile([P, SPAN], f32, tag="tr", name="psqk")
                    for dcp in range(4):
                        nc.tensor.matmul(
                            ps[:], w8[:, dcp, :, t, h, :].bitcast(fp8),
                            x8t[:, dcp, :, :].bitcast(fp8),
                            start=(dcp == 0), stop=(dcp == 3), perf_mode=DR)
                    if isq:
                        if (sp, t) not in qts:
                            qts[(sp, t)] = qpool.tile(
                                [P, 2, SPAN], fp8, tag=f"qT{t}",
                                name=f"qT{t}_{sp}")
                        nc.vector.tensor_scalar_mul(
                            qts[(sp, t)][:, h, :], ps[:], scale)
                    else:
                        nc.vector.tensor_scalar_mul(
                            kT8[t][:, h, sp * SPAN:(sp + 1) * SPAN], ps[:],
                            scale)
                return emit

            def proj_v(sp, tb, x8t, x8lt):
                # 3-pass fp8 DR: 16v = x_hi@w_hi + x_lo@w_hi + x_hi@w_lo
                # (w residual split keeps the value path at ~bf16 quality)
                def emit():
                    ps = psT.tile([P, SPAN], f32, tag="tr", name="psv")
                    ts = slice(tb * P, (tb + 1) * P)
                    passes = [(x8t, 0), (x8lt, 0), (x8t, 1)]
                    for pi, (xt, wl) in enumerate(passes):
                        for dcp in range(4):
                            nc.tensor.matmul(
                                ps[:], xt[:, dcp, :, ts].bitcast(fp8),
                                wv8[:, dcp, :, wl, :].bitcast(fp8),
                                start=(pi == 0 and dcp == 0),
                                stop=(pi == 2 and dcp == 3), perf_mode=DR)
                    kc = sp * 4 + tb
                    psv = ps[:].rearrange("p (pr u v) -> p pr u v",
                                          pr=NPAIR, v=HD)
                    nc.vector.tensor_scalar_mul(
                        Vbh[:, kc // 2, kc % 2, :, :, 0:HD], psv, 1.0 / 16)
                    nc.vector.tensor_scalar_mul(
                        Vbb[:, kc, :, :, 0:HD], psv, 1.0 / 16)
                return emit

            def q_groups(sp, x8t):
                return [proj_qk(wq8, t, h, sp, x8t, 1.0 / 32, True)
                        for t in range(2) for h in range(2)]

            def k_groups(sp, x8t):
                return [proj_qk(wk8, t, h, sp, x8t, 1.0 / 16, False)
                        for t in range(2) for h in range(2)]

            def v_groups(sp, x8t, x8lt):
                return [proj_v(sp, tb, x8t, x8lt) for tb in range(4)]

            def outproj_group(sp, tb, os_, dma_q, ps_ap=None, act_copy=False):
                def emit():
                    if ps_ap is None:
                        ps = psT.tile([P, SPAN], f32, tag="tr", name="pso")[:]
                    else:
                        ps = ps_ap
                    for pc in range(NPAIR):
                        nc.tensor.matmul(
                            ps,
                            ctxs[(sp, pc)][:, (tb - sp * 4) * P:
                                           (tb - sp * 4 + 1) * P],
                            wot[:, pc, os_ * SPAN:(os_ + 1) * SPAN],
                            start=(pc == 0), stop=(pc == NPAIR - 1))
                    stage = stgpool.tile([P, SPAN], f16, tag="st", name="stage")
                    if act_copy:
                        nc.scalar.activation(
                            stage[:], ps, mybir.ActivationFunctionType.Copy)
                    else:
                        nc.vector.tensor_copy(stage[:], ps)
                    dma_q.dma_start(
                        out_h.ap()[tb * P:(tb + 1) * P,
                                   os_ * SPAN:(os_ + 1) * SPAN], stage[:])
                return emit

            def outproj_groups(sp):
                return [outproj_group(sp, tb, os_,
                                      nc.sync if os_ == 0 else nc.gpsimd)
                        for tb in range(sp * 4, (sp + 1) * 4)
                        for os_ in range(2)]

            # ---------- attention for one span ----------
            deferred = []

            def attn_span(s, fillers, vgs=(), kgs=()):
                K = 4 * (s + 1)
                nslot = (K + 4) * NPAIR
                state = {"slot": 0, "fi": 0}

                def pace():
                    tgt = min(len(fillers),
                              len(fillers) * (state["slot"] + 1) // nslot)
                    while state["fi"] < tgt:
                        fillers[state["fi"]]()
                        state["fi"] += 1

                def tick():
                    state["slot"] += 1
                    pace()

                for pr in range(NPAIR):
                    t = pr // 2
                    ctxp = [psC.tile([96, SPAN], f32, tag="ctx",
                                     name=f"ctx{u}") for u in range(2)]
                    ct = cpool.tile([P, SPAN], f16, tag=f"cT{pr}",
                                    name=f"cT{pr}_{s}")
                    ctxs[(s, pr)] = ct
                    pts = {}
                    qt = qts[(s, t)]

                    avn = [0, 0]  # AV matmuls emitted per u (K total each)

                    def emit_qk(kj, pr=pr, t=t, pts=pts, qt=qt):
                        m = kj - 4 * s
                        ss = psS.tile([P, 2, SPAN], f32, tag="psS", name="ss")
                        c0 = 0 if m < 0 else m * KC
                        for u in range(2):
                            b32 = 32 * (2 * (pr % 2) + u)
                            r = slice(b32, b32 + 32)
                            nc.tensor.matmul(
                                ss[:, u, c0:],
                                kT8[t][r, :, kj * KC:(kj + 1) * KC],
                                qt[r, :, c0:],
                                start=True, stop=True, perf_mode=DR,
                                tile_position=(b32, 0))
                        if m < 0:
                            # off-diagonal: fp8 pt pair for DR AV
                            kjp, sl = kj // 2, kj % 2
                            if sl == 0:
                                pts[kjp] = ptpool.tile([P, 2, 2, SPAN], fp8,
                                                       tag="pt", name="pt")
                            nc.scalar.activation(pts[kjp][:, sl, :, :],
                                                 ss[:, :, :], Exp, scale=0.25)
                        else:
                            # diagonal: bf16 pt (exact-ish attention weights
                            # for concentrated rows) + post-exp triangle mask
                            ptb = ptbpool.tile([P, 2, SPAN], bf16,
                                               tag="ptb", name="ptb")
                            pts[("d", kj)] = ptb
                            nc.scalar.activation(ptb[:, :, c0:],
                                                 ss[:, :, c0:], Exp,
                                                 scale=0.25)
                            nc.vector.tensor_mul(
                                ptb[:, :, c0:c0 + KC], ptb[:, :, c0:c0 + KC],
                                mask01[:].rearrange("p (u f) -> p u f", u=1)
                                .to_broadcast((P, 2, KC)))

                    NU = 2 * s + 4  # AV matmuls per u-chain

                    def emit_av8(kjp, pr=pr, pts=pts, ctxp=ctxp, NU=NU):
                        pt = pts.pop(kjp)
                        for u in range(2):
                            nc.tensor.matmul(
                                ctxp[u][:],
                                Vbh[:, kjp, :, pr, u, :],
                                pt[:, :, u, :],
                                start=(avn[u] == 0),
                                stop=(avn[u] == NU - 1),
                                perf_mode=DR, skip_group_check=True)
                            avn[u] += 1

                    def emit_avd(kj, pr=pr, pts=pts, ctxp=ctxp, NU=NU):
                        ptb = pts.pop(("d", kj))
                        c0 = (kj - 4 * s) * KC
                        for u in range(2):
                            nc.tensor.matmul(
                                ctxp[u][0:HD + 1, c0:],
                                Vbb[:, kj, pr, u, :],
                                ptb[:, u, c0:],
                                start=(avn[u] == 0), stop=(avn[u] == NU - 1),
                                skip_group_check=True)
                            avn[u] += 1

                    def evict(pr=pr, ctxp=ctxp, ct=ct, s=s):
                        rz = rzpool.tile([P, 2, SPAN], bf16, tag="rz",
                                         name="rz")
                        rzbc = rzpool.tile([P, 2, SPAN], bf16, tag="rz",
                                           name="rzbc")

                        def fin(u):
                            def run():
                                # 1/Z row: psum row 64 -> sbuf row 0
                                # (cross-partition DVE write, hw-verified)
                                with nc.allow_low_precision(
                                        reason="1/Z bf16: 0.4% on ctx"):
                                    nc.vector.reciprocal(
                                        rz[0:1, u, :], ctxp[u][64:65, :])
                                nc.gpsimd.partition_broadcast(
                                    rzbc[:, u, :], rz[0:1, u, :])
                                nc.vector.tensor_mul(
                                    ct[u * HD:(u + 1) * HD, :],
                                    ctxp[u][0:HD, :], rzbc[0:HD, u, :])
                            return run
                        deferred.append(fin(0))
                        deferred.append(fin(1))

                    # AV work units: (ready_kj, emit_fn); off-diag kjp ready
                    # at its odd kj, diag kj ready at kj
                    units = []
                    for kj2 in range(K):
                        if kj2 < 4 * s:
                            if kj2 % 2 == 1:
                                units.append((kj2, kj2 // 2, emit_av8))
                        else:
                            units.append((kj2, kj2, emit_avd))
                    # JIT placement: V(s) spread across pr0's whole kj
                    # range (consumers: pr0 diag AV); K(s) t0-groups in pr0,
                    # t1-groups in pr1 (consumers: pr0/pr2 diag QK)
                    jit = {}
                    if pr == 0:
                        for i in range(len(vgs)):
                            jit.setdefault(max(i * K // 4, i), []).append(
                                vgs[i])
                        # K(s) t0 keys must precede the diag QK at kj=4s
                        # (jit runs after emit_qk in the same iteration)
                        for i in (0, 1) if kgs else ():
                            jit.setdefault(min(1 + i * (K // 3 + 1),
                                               4 * s - 1), []).append(kgs[i])
                    elif pr == 1:
                        for i in (2, 3) if kgs else ():
                            jit.setdefault(1 + (i - 2) * (K // 3 + 1),
                                           []).append(kgs[i])
                    ui = [0]
                    for kj in range(K):
                        emit_qk(kj)
                        for g in jit.get(kj, ()):
                            g()
                        if deferred:
                            deferred.pop(0)()
                        tick()
                        while (ui[0] < len(units)
                               and units[ui[0]][0] + LAG <= kj):
                            units[ui[0]][2](units[ui[0]][1])
                            ui[0] += 1
                    while ui[0] < len(units):
                        if deferred:
                            deferred.pop(0)()
                        tick()
                        units[ui[0]][2](units[ui[0]][1])
                        ui[0] += 1
                    evict()
                while state["fi"] < len(fillers):
                    fillers[state["fi"]]()
                    state["fi"] += 1

            # ---------- main schedule ----------
            # V(s) and K(s) run inside span s itself (their first consumers
            # are span s's own diag AV / diag QK) -- keeps earlier spans off
            # the PE; only Q(s+1) must finish during span s
            # pr0's attention needs (q,k) t0 groups first: interleave
            qg0, kg0 = q_groups(0, x8ts[0]), k_groups(0, x8ts[0])
            for i in range(4):
                qg0[i]()
                kg0[i]()
            for s in range(NSPAN):
                vgs = v_groups(s, x8ts[s], x8ls[s])
                kgs = k_groups(s, x8ts[s]) if s >= 1 else ()
                fillers = []
                if s + 1 < NSPAN:
                    x8t = x8pool.tile([P, 4, 2, SPAN], u8, tag="x8t",
                                      name=f"x8t{s + 1}")
                    x8lt = xlpool.tile([P, 4, 2, SPAN], u8, tag="x8l",
                                       name=f"x8l{s + 1}")
                    x8ts[s + 1], x8ls[s + 1] = x8t, x8lt
                    sl = slice((s + 1) * SPAN, (s + 2) * SPAN)
                    nc.sync.dma_start(x8t[:], x8_d[:, :, :, sl])
                    nc.sync.dma_start(x8lt[:], x8l_d[:, :, :, sl])
                    fillers += q_groups(s + 1, x8t)
                if s == 2:
                    fillers += outproj_groups(0)
                elif s == 3:
                    fillers += outproj_groups(1) + outproj_groups(2)
                attn_span(s, fillers, vgs, kgs)
            while deferred:
                deferred.pop(0)()
            # tail outproj(3): attention is done, so psS's 4 banks are free
            # scratch -- 6 parallel psums let all 32 matmuls run back-to-back
            # (no psT-rotation stalls, PE stays ramped); stage copies split
            # DVE/ACT (both idle at the tail)
            sc = []
            for i in range(2):
                t = psS.tile([P, 2, SPAN], f32, tag="psS", name=f"osc{i}")
                sc += [t[:, 0, :], t[:, 1, :]]
            for i in range(2):
                sc.append(psC.tile([P, SPAN], f32, tag="ctx",
                                   name=f"oscc{i}")[:])
            tails = [(tb, os_) for tb in range(12, 16) for os_ in range(2)]
            for gi, (tb, os_) in enumerate(tails):
                ps_ap = sc[gi - 2] if 2 <= gi < 8 else None
                outproj_group(3, tb, os_,
                              nc.sync if os_ == 0 else nc.gpsimd,
                              ps_ap=ps_ap, act_copy=(gi % 2 == 1))()

    nc.compile()
    return nc


def get_nc():
    if "nc" not in _CACHE:
        _CACHE["nc"] = _build()
    return _CACHE["nc"]


def _perm512():
    perm = np.empty(512, np.int64)
    i = 0
    for t in range(2):
        for h in range(2):
            for beta in range(4):
                pr = 2 * t + beta // 2
                u = beta % 2
                for dd in range(32):
                    perm[i] = pr * 128 + u * 64 + h * 32 + dd
                    i += 1
    return perm


def kernel(x, Wq, Wk, Wv, Wo, bo):
    import ml_dtypes
    from concourse import bass_utils

    e4 = ml_dtypes.float8_e4m3

    x = np.asarray(x, dtype=np.float32)
    Wq, Wk, Wv = (np.asarray(w, dtype=np.float32) for w in (Wq, Wk, Wv))
    Wo = np.asarray(Wo, dtype=np.float32)
    bo = np.asarray(bo, dtype=np.float32)
    perm = _perm512()

    in_maps = []
    for c in range(NCORES):
        b, g = c // 2, c % 2
        gsl = slice(g * 512, (g + 1) * 512)
        xT = np.ascontiguousarray(x[b].T)
        x8 = xT.astype(e4)
        wv16 = np.ascontiguousarray(16.0 * Wv[gsl].T)
        wv8h = wv16.astype(e4)
        in_maps.append({
            "x8": x8.view(np.uint8),
            # residual of the e4m3 x quantization (V-proj pass 2)
            "x8l": (xT - x8.astype(np.float32)).astype(e4).view(np.uint8),
            # w quantized x16 to dodge e4m3 subnormals; unscaled on-chip
            "wq8": np.ascontiguousarray((16.0 * Wq[gsl].T)[:, perm])
            .astype(e4).view(np.uint8),
            "wk8": np.ascontiguousarray((16.0 * Wk[gsl].T)[:, perm])
            .astype(e4).view(np.uint8),
            "wv8h": wv8h.view(np.uint8),
            "wv8l": (wv16 - wv8h.astype(np.float32)).astype(e4)
            .view(np.uint8),
            "woh": np.ascontiguousarray(Wo[:, gsl].T).astype(np.float16),
        })

    nc = get_nc()
    res = bass_utils.run_bass_kernel_spmd(nc, in_maps,
                                          core_ids=list(range(NCORES)))
    parts = [res.results[c]["out"].astype(np.float32) for c in range(NCORES)]
    out = np.stack([parts[2 * b] + parts[2 * b + 1] + bo for b in range(B)])
    return out.astype(np.float32)


# revision 55
# speedup vs baseline: 1.0437x; 1.0043x over previous
"""Multi-head causal attention (B=4, T=2048, D=1024, H=16) on 8 Trainium2 cores.

Sharding: core c = (b, g) with b = c//2 (batch), g = c%2 (head-group of 8 heads).
Each core: Q/K/V projections for its 8 heads (column-parallel), causal attention,
row-parallel partial output projection. Host sums the g=0/g=1 partials + bias.

v4 design (fp8 DoubleRow + streaming AV; cost model: matmul = out-free-rows x
0.4167ns x cpr, fp8e4 DoubleRow cpr=0.5 contracting 2x128/instr; ACT exp at
0.8333ns/free-elem = ~147us busy is the floor):
  - Q/K proj: fp8 DR, x8 moving [128,2,512], w8 stationary [128,2,128] in 4
    col-groups (t=pair-half, h=dim-half) so psum partitions land as
    (beta=2*(pr%2)+u, dd) blocks of 32 -> qT8/kT8 [32-blocks, 2h, T] fp8 for
    2x32-contraction DR score matmuls. w quantized x16 (e4m3 subnormal
    dodge), unscaled in the DVE psum->fp8 copy; score scale 1/8 folded as
    qT8 = e4m3(q/2) + exp(scale=0.25). Sub-128 DR needs explicit
    tile_position (base-96 slices break base_partition inference).
  - scores: fp8 DR per (pr,u,kj): out ss [128 keys, 2u, 512-c0] psum.
  - exp: ACT psum -> sbuf, one instr per kj covering both heads. Off-diag
    chunks -> fp8 pt pairs [128, 2kj, 2u, 512]; diag chunks -> bf16 pt
    (pt-quant error only bites concentrated near-diagonal rows) with
    post-exp mask01 multiply on DVE (off ACT's critical path).
  - AV streaming into ctx psum [96, 512] per (pr,u), accumulated across the
    span: off-diag = single-fp8 DR (lhsT = Vbh [128, 2kj, 96]: 64 v-dims +
    ones col at 64 -> Z at psum row 64 + zero pad -- dual-fp8 ldweights
    needs cols%32==0, >=64); diag = bf16 non-DR (lhsT = Vbb [128, 65]).
    Diffuse off-diag rows average out single-fp8 V error; vonly ~1e-3.
  - evict per (pr,u): DVE reciprocal of Z row -> bf16 rz at partition 0
    (cross-partition psum read, hw-verified), gpsimd partition_broadcast ->
    rzbc sbuf, DVE mul ctx*rzbc -> ctx_sb f16 [128=(u,vd), 512] (u1 written
    cross-partition to rows 64:128). Evicts deferred into the next pair's /
    span's kj loop so the PE never stalls on them.
  - outproj: f16, 4x128-contraction per [128q, 512od] psum group; DVE f16
    stage -> DMA (psum DMA is forbidden); host sums g-partials + bias f32.
  - V proj: 3-pass fp8 DR (16v = xh@wh + xl@wh + xh@wl; w x16-scaled
    hi/lo split keeps the value path ~bf16 quality; psum/16 on copy).
  - schedule: only Q(s+1) is paced as filler through attention(s); K(s) and
    V(s) are JIT-emitted inside span s itself (first consumers are its own
    diag QK/AV), spread across pr0/pr1 kj iterations AFTER each emit_qk --
    K keys must land strictly before kj=4s or the diag QK reads stale kT
    (caused a NaN once). outproj(0)@s2, outproj(1,2)@s3, outproj(3) tail.
    All input DMAs on the SP queue (the DMA bus serializes transfers at
    ~1.45us each; ACT-queue DMAs would block exp dispatch); out DMAs split
    SP/Pool. Evicts defer across pair/span boundaries.
Measured: 194317 ns (TimelineSim), rel err 1.21e-2 (gate 2e-2). Baseline
v3 was 240443 ns. ACT exp busy 146.9us (73%), PE 129us, DVE 118us.
Remaining idle: ~10us startup (DMA-bus chain + proj pipe), ~6us span-0/1
boundary (ss-pool bufs=2 serializes the exp chain across spans; psum is
full: psS 4 + psC 2 + psT 2 banks), small scattered bubbles.
"""

import os
import sys

try:
    import concourse.bass  # noqa: F401
except ImportError:  # pragma: no cover
    sys.path.insert(0, "/opt/trn_rl_repo")

import numpy as np

B, T, D = 4, 2048, 1024
H, HD = 16, 64
NCORES = 8
NPAIR = 4
NSPAN = 4
SPAN = 512
KC = 128
P = 128
LAG = 3

_CACHE = {}


def _build():
    import concourse.bacc as bacc
    import concourse.mybir as mybir
    import concourse.tile as tile

    f32 = mybir.dt.float32
    f16 = mybir.dt.float16
    bf16 = mybir.dt.bfloat16
    fp8 = mybir.dt.float8e4
    u8 = mybir.dt.uint8
    Exp = mybir.ActivationFunctionType.Exp

    DR = mybir.MatmulPerfMode.DoubleRow

    nc = bacc.Bacc("TRN2", target_bir_lowering=False, debug=False,
                   num_devices=NCORES)

    x8_h = nc.dram_tensor("x8", (D, T), u8, kind="ExternalInput")
    x8l_h = nc.dram_tensor("x8l", (D, T), u8, kind="ExternalInput")
    wq8_h = nc.dram_tensor("wq8", (D, 512), u8, kind="ExternalInput")
    wk8_h = nc.dram_tensor("wk8", (D, 512), u8, kind="ExternalInput")
    wv8h_h = nc.dram_tensor("wv8h", (D, 512), u8, kind="ExternalInput")
    wv8l_h = nc.dram_tensor("wv8l", (D, 512), u8, kind="ExternalInput")
    woh_h = nc.dram_tensor("woh", (512, D), f16, kind="ExternalInput")
    out_h = nc.dram_tensor("out", (T, D), f16, kind="ExternalOutput")

    # x8: D = (dcp 4, k 2, p 128); per span slice on T
    x8_d = x8_h.ap().rearrange("(dcp k p) t -> p dcp k t", p=P, k=2)
    x8l_d = x8l_h.ap().rearrange("(dcp k p) t -> p dcp k t", p=P, k=2)
    wq8_d = wq8_h.ap().rearrange("(dcp k p) (t h m) -> p dcp k t h m",
                                 p=P, k=2, t=2, h=2)
    wk8_d = wk8_h.ap().rearrange("(dcp k p) (t h m) -> p dcp k t h m",
                                 p=P, k=2, t=2, h=2)
    wv8h_d = wv8h_h.ap().rearrange("(dcp k p) f -> p dcp k f", p=P, k=2)
    wv8l_d = wv8l_h.ap().rearrange("(dcp k p) f -> p dcp k f", p=P, k=2)
    woh_d = woh_h.ap().rearrange("(pc p) f -> p pc f", p=P)

    with tile.TileContext(nc) as tc:
        with (
            tc.tile_pool(name="persist", bufs=1) as persist,
            tc.tile_pool(name="x8p", bufs=2) as x8pool,
            tc.tile_pool(name="xlp", bufs=2) as xlpool,
            tc.tile_pool(name="qp", bufs=4) as qpool,
            tc.tile_pool(name="ptp", bufs=6) as ptpool,
            tc.tile_pool(name="ptb", bufs=5) as ptbpool,
            tc.tile_pool(name="rzp", bufs=8) as rzpool,
            tc.tile_pool(name="cp", bufs=8) as cpool,
            tc.tile_pool(name="stg", bufs=6) as stgpool,
            tc.tile_pool(name="psS", bufs=2, space="PSUM") as psS,
            tc.tile_pool(name="psC", bufs=2, space="PSUM") as psC,
            tc.tile_pool(name="psT", bufs=2, space="PSUM") as psT,
        ):
            wq8 = persist.tile([P, 4, 2, 2, 2, P], u8, tag="wq8", name="wq8")
            wk8 = persist.tile([P, 4, 2, 2, 2, P], u8, tag="wk8", name="wk8")
            wv8 = persist.tile([P, 4, 2, 2, 512], u8, tag="wv8", name="wv8")
            wot = persist.tile([P, 4, D], f16, tag="wot", name="wot")
            kT8 = [persist.tile([P, 2, T], fp8, tag=f"kT8{t}", name=f"kT8{t}")
                   for t in range(2)]
            # Vb: [p, kjp 8, kj 2, pr 4, u 2, 96]: 64 v-dims + ones col at
            # 64 + zero pad to 96 (dual-fp8 ldweights needs cols % 32 == 0,
            # >= 64; psum rows 65:96 are dead)
            VW = 96
            Vbh = persist.tile([P, 8, 2, NPAIR, 2, VW], fp8, tag="Vbh",
                               name="Vbh")
            # diag-chunk AV runs in bf16 (pt quantization error bites only
            # concentrated near-diagonal attention rows): [p, kc, pr, u, 65]
            Vbb = persist.tile([P, 16, NPAIR, 2, HD + 1], bf16, tag="Vbb",
                               name="Vbb")
            mask01 = persist.tile([P, KC], bf16, tag="mask01", name="mask01")
            one = nc.const_aps.tensor(1.0, (P, 1))

            # ---- initial DMAs ----
            x8ts = {0: x8pool.tile([P, 4, 2, SPAN], u8, tag="x8t", name="x8t0")}
            x8ls = {0: xlpool.tile([P, 4, 2, SPAN], u8, tag="x8l", name="x8l0")}
            nc.sync.dma_start(wq8[:], wq8_d)
            nc.sync.dma_start(x8ts[0][:], x8_d[:, :, :, 0:SPAN])
            nc.sync.dma_start(wk8[:], wk8_d)
            nc.sync.dma_start(wv8[:, :, :, 0, :], wv8h_d)
            nc.sync.dma_start(x8ls[0][:], x8l_d[:, :, :, 0:SPAN])
            nc.sync.dma_start(wv8[:, :, :, 1, :], wv8l_d)
            nc.sync.dma_start(wot[:], woh_d)


            # mask01[p, f] = 1 if p <= f else 0 (post-exp diag pt mask)
            nc.gpsimd.memset(mask01[:], 1.0)
            nc.gpsimd.affine_select(
                out=mask01[:], in_=mask01[:],
                compare_op=mybir.AluOpType.is_ge, fill=0.0,
                base=0, channel_multiplier=-1, pattern=[[1, KC]],
            )
            # Vbh pad cols [65:96] must be zero (read by every off-diag
            # AV); data cols are always written before first read
            nslots = 8 * 2 * NPAIR * 2
            nc.gpsimd.memset(
                Vbh[:].rearrange("p a b c d e -> p (a b c d) e")
                [:, :, HD + 1:], 0.0)
            nc.vector.tensor_copy(
                Vbh[:].rearrange("p a b c d e -> p (a b c d) e")
                [:, :, HD:HD + 1],
                one.to_broadcast((P, nslots, 1)))
            nc.vector.tensor_copy(
                Vbb[:].rearrange("p a c d e -> p (a c d) e")
                [:, :, HD:HD + 1],
                one.to_broadcast((P, 16 * NPAIR * 2, 1)))

            qts = {}    # (sp, t) -> [P, 2, SPAN] fp8 tile
            ctxs = {}   # (sp, pr) -> [P, SPAN] f16 tile

            # ---------- emission helpers ----------
            def proj_qk(w8, pr_half, h, sp, x8t, scale, isq, scratch=False):
                t = pr_half

                def emit():
                    if scratch:
                        # startup only: borrow an idle psS slot so the four
                        # critical span-0 groups get independent psums
                        ps = psS.tile([P, 2, SPAN], f32, tag="psS",
                                      name="psqk0")[:, 0, :]
                    else:
                        ps = psT.tile([P, SPAN], f32, tag="tr", name="psqk")
                    for dcp in range(4):
                        nc.tensor.matmul(
                            ps[:], w8[:, dcp, :, t, h, :].bitcast(fp8),
                            x8t[:, dcp, :, :].bitcast(fp8),
                            start=(dcp == 0), stop=(dcp == 3), perf_mode=DR)
                    if isq:
                        if (sp, t) not in qts:
                            qts[(sp, t)] = qpool.tile(
                                [P, 2, SPAN], fp8, tag=f"qT{t}",
                                name=f"qT{t}_{sp}")
                        nc.vector.tensor_scalar_mul(
                            qts[(sp, t)][:, h, :], ps[:], scale)
                    else:
                        nc.vector.tensor_scalar_mul(
                            kT8[t][:, h, sp * SPAN:(sp + 1) * SPAN], ps[:],
                            scale)
                return emit

            def proj_v(sp, tb, x8t, x8lt):
                # 3-pass fp8 DR: 16v = x_hi@w_hi + x_lo@w_hi + x_hi@w_lo
                # (w residual split keeps the value path at ~bf16 quality)
                def emit():
                    ps = psT.tile([P, SPAN], f32, tag="tr", name="psv")
                    ts = slice(tb * P, (tb + 1) * P)
                    passes = [(x8t, 0), (x8lt, 0), (x8t, 1)]
                    for pi, (xt, wl) in enumerate(passes):
                        for dcp in range(4):
                            nc.tensor.matmul(
                                ps[:], xt[:, dcp, :, ts].bitcast(fp8),
                                wv8[:, dcp, :, wl, :].bitcast(fp8),
                                start=(pi == 0 and dcp == 0),
                                stop=(pi == 2 and dcp == 3), perf_mode=DR)
                    kc = sp * 4 + tb
                    psv = ps[:].rearrange("p (pr u v) -> p pr u v",
                                          pr=NPAIR, v=HD)
                    nc.vector.tensor_scalar_mul(
                        Vbh[:, kc // 2, kc % 2, :, :, 0:HD], psv, 1.0 / 16)
                    nc.vector.tensor_scalar_mul(
                        Vbb[:, kc, :, :, 0:HD], psv, 1.0 / 16)
                return emit

            def q_groups(sp, x8t):
                return [proj_qk(wq8, t, h, sp, x8t, 1.0 / 32, True)
                        for t in range(2) for h in range(2)]

            def k_groups(sp, x8t):
                return [proj_qk(wk8, t, h, sp, x8t, 1.0 / 16, False)
                        for t in range(2) for h in range(2)]

            def v_groups(sp, x8t, x8lt):
                return [proj_v(sp, tb, x8t, x8lt) for tb in range(4)]

            ostages = {}  # tb -> [P, 2, SPAN] f16 stage (one DMA per tb)

            def outproj_group(sp, tb, os_, dma_q, ps_ap=None, act_copy=False):
                def emit():
                    if ps_ap is None:
                        ps = psT.tile([P, SPAN], f32, tag="tr", name="pso")[:]
                    else:
                        ps = ps_ap
                    for pc in range(NPAIR):
                        nc.tensor.matmul(
                            ps,
                            ctxs[(sp, pc)][:, (tb - sp * 4) * P:
                                           (tb - sp * 4 + 1) * P],
                            wot[:, pc, os_ * SPAN:(os_ + 1) * SPAN],
                            start=(pc == 0), stop=(pc == NPAIR - 1))
                    if os_ == 0:
                        ostages[tb] = stgpool.tile([P, 2, SPAN], f16,
                                                   tag="st", name="stage")
                    stage = ostages[tb]
                    if act_copy:
                        nc.scalar.activation(
                            stage[:, os_, :], ps,
                            mybir.ActivationFunctionType.Copy)
                    else:
                        nc.vector.tensor_copy(stage[:, os_, :], ps)
                    if os_ == 1:
                        # one merged 1024-col DMA per tb: fewer bus slots
                        dma_q.dma_start(
                            out_h.ap()[tb * P:(tb + 1) * P, :],
                            ostages.pop(tb)[:].rearrange("p a f -> p (a f)"))
                return emit

            def outproj_groups(sp):
                return [outproj_group(sp, tb, os_,
                                      nc.sync if os_ == 0 else nc.gpsimd)
                        for tb in range(sp * 4, (sp + 1) * 4)
                        for os_ in range(2)]

            # ---------- attention for one span ----------
            deferred = []

            def attn_span(s, fillers, vgs=(), kgs=()):
                K = 4 * (s + 1)
                nslot = (K + 4) * NPAIR
                state = {"slot": 0, "fi": 0}

                def pace():
                    tgt = min(len(fillers),
                              len(fillers) * (state["slot"] + 1) // nslot)
                    while state["fi"] < tgt:
                        fillers[state["fi"]]()
                        state["fi"] += 1

                def tick():
                    state["slot"] += 1
                    pace()

                for pr in range(NPAIR):
                    t = pr // 2
                    ctxp = [psC.tile([96, SPAN], f32, tag="ctx",
                                     name=f"ctx{u}") for u in range(2)]
                    ct = cpool.tile([P, SPAN], f16, tag=f"cT{pr}",
                                    name=f"cT{pr}_{s}")
                    ctxs[(s, pr)] = ct
                    pts = {}
                    qt = qts[(s, t)]

                    avn = [0, 0]  # AV matmuls emitted per u (K total each)

                    def emit_qk(kj, pr=pr, t=t, pts=pts, qt=qt):
                        m = kj - 4 * s
                        ss = psS.tile([P, 2, SPAN], f32, tag="psS", name="ss")
                        c0 = 0 if m < 0 else m * KC
                        for u in range(2):
                            b32 = 32 * (2 * (pr % 2) + u)
                            r = slice(b32, b32 + 32)
                            nc.tensor.matmul(
                                ss[:, u, c0:],
                                kT8[t][r, :, kj * KC:(kj + 1) * KC],
                                qt[r, :, c0:],
                                start=True, stop=True, perf_mode=DR,
                                tile_position=(b32, 0))
                        if m < 0:
                            # off-diagonal: fp8 pt pair for DR AV
                            kjp, sl = kj // 2, kj % 2
                            if sl == 0:
                                pts[kjp] = ptpool.tile([P, 2, 2, SPAN], fp8,
                                                       tag="pt", name="pt")
                            nc.scalar.activation(pts[kjp][:, sl, :, :],
                                                 ss[:, :, :], Exp, scale=0.25)
                        else:
                            # diagonal: bf16 pt (exact-ish attention weights
                            # for concentrated rows) + post-exp triangle mask
                            ptb = ptbpool.tile([P, 2, SPAN], bf16,
                                               tag="ptb", name="ptb")
                            pts[("d", kj)] = ptb
                            nc.scalar.activation(ptb[:, :, c0:],
                                                 ss[:, :, c0:], Exp,
                                                 scale=0.25)
                            nc.vector.tensor_mul(
                                ptb[:, :, c0:c0 + KC], ptb[:, :, c0:c0 + KC],
                                mask01[:].rearrange("p (u f) -> p u f", u=1)
                                .to_broadcast((P, 2, KC)))

                    NU = 2 * s + 4  # AV matmuls per u-chain

                    def emit_av8(kjp, pr=pr, pts=pts, ctxp=ctxp, NU=NU):
                        pt = pts.pop(kjp)
                        for u in range(2):
                            nc.tensor.matmul(
                                ctxp[u][:],
                                Vbh[:, kjp, :, pr, u, :],
                                pt[:, :, u, :],
                                start=(avn[u] == 0),
                                stop=(avn[u] == NU - 1),
                                perf_mode=DR, skip_group_check=True)
                            avn[u] += 1

                    def emit_avd(kj, pr=pr, pts=pts, ctxp=ctxp, NU=NU):
                        ptb = pts.pop(("d", kj))
                        c0 = (kj - 4 * s) * KC
                        for u in range(2):
                            nc.tensor.matmul(
                                ctxp[u][0:HD + 1, c0:],
                                Vbb[:, kj, pr, u, :],
                                ptb[:, u, c0:],
                                start=(avn[u] == 0), stop=(avn[u] == NU - 1),
                                skip_group_check=True)
                            avn[u] += 1

                    def evict(pr=pr, ctxp=ctxp, ct=ct, s=s):
                        rz = rzpool.tile([P, 2, SPAN], bf16, tag="rz",
                                         name="rz")
                        rzbc = rzpool.tile([P, 2, SPAN], bf16, tag="rz",
                                           name="rzbc")

                        def fin(u):
                            def run():
                                # 1/Z row: psum row 64 -> sbuf row 0
                                # (cross-partition DVE write, hw-verified)
                                with nc.allow_low_precision(
                                        reason="1/Z bf16: 0.4% on ctx"):
                                    nc.vector.reciprocal(
                                        rz[0:1, u, :], ctxp[u][64:65, :])
                                nc.gpsimd.partition_broadcast(
                                    rzbc[:, u, :], rz[0:1, u, :])
                                nc.vector.tensor_mul(
                                    ct[u * HD:(u + 1) * HD, :],
                                    ctxp[u][0:HD, :], rzbc[0:HD, u, :])
                            return run
                        deferred.append(fin(0))
                        deferred.append(fin(1))

                    # AV work units: (ready_kj, emit_fn); off-diag kjp ready
                    # at its odd kj, diag kj ready at kj
                    units = []
                    for kj2 in range(K):
                        if kj2 < 4 * s:
                            if kj2 % 2 == 1:
                                units.append((kj2, kj2 // 2, emit_av8))
                        else:
                            units.append((kj2, kj2, emit_avd))
                    # JIT placement: V(s) spread across pr0's whole kj
                    # range (consumers: pr0 diag AV); K(s) t0-groups in pr0,
                    # t1-groups in pr1 (consumers: pr0/pr2 diag QK)
                    jit = {}
                    if pr == 0:
                        for i in range(len(vgs)):
                            jit.setdefault(max(i * K // 4, i), []).append(
                                vgs[i])
                        # K(s) t0 keys must precede the diag QK at kj=4s
                        # (jit runs after emit_qk in the same iteration)
                        for i in (0, 1) if kgs else ():
                            jit.setdefault(min(1 + i * (K // 3 + 1),
                                               4 * s - 1), []).append(kgs[i])
                    elif pr == 1:
                        for i in (2, 3) if kgs else ():
                            jit.setdefault(1 + (i - 2) * (K // 3 + 1),
                                           []).append(kgs[i])
                    ui = [0]
                    for kj in range(K):
                        emit_qk(kj)
                        for g in jit.get(kj, ()):
                            g()
                        if deferred:
                            deferred.pop(0)()
                        tick()
                        while (ui[0] < len(units)
                               and units[ui[0]][0] + LAG <= kj):
                            units[ui[0]][2](units[ui[0]][1])
                            ui[0] += 1
                    while ui[0] < len(units):
                        if deferred:
                            deferred.pop(0)()
                        tick()
                        units[ui[0]][2](units[ui[0]][1])
                        ui[0] += 1
                    evict()
                while state["fi"] < len(fillers):
                    fillers[state["fi"]]()
                    state["fi"] += 1

            # ---------- main schedule ----------
            # V(s) and K(s) run inside span s itself (their first consumers
            # are span s's own diag AV / diag QK) -- keeps earlier spans off
            # the PE; only Q(s+1) must finish during span s
            # pr0's attention needs (q,k) t0 groups first: interleave
            qg0, kg0 = q_groups(0, x8ts[0]), k_groups(0, x8ts[0])
            for i in range(4):
                qg0[i]()
                kg0[i]()
            for s in range(NSPAN):
                vgs = v_groups(s, x8ts[s], x8ls[s])
                kgs = k_groups(s, x8ts[s]) if s >= 1 else ()
                fillers = []
                if s + 1 < NSPAN:
                    x8t = x8pool.tile([P, 4, 2, SPAN], u8, tag="x8t",
                                      name=f"x8t{s + 1}")
                    x8lt = xlpool.tile([P, 4, 2, SPAN], u8, tag="x8l",
                                       name=f"x8l{s + 1}")
                    x8ts[s + 1], x8ls[s + 1] = x8t, x8lt
                    sl = slice((s + 1) * SPAN, (s + 2) * SPAN)
                    nc.sync.dma_start(x8t[:], x8_d[:, :, :, sl])
                    nc.sync.dma_start(x8lt[:], x8l_d[:, :, :, sl])
                    fillers += q_groups(s + 1, x8t)
                if s == 2:
                    fillers += outproj_groups(0)
                elif s == 3:
                    fillers += outproj_groups(1) + outproj_groups(2)
                attn_span(s, fillers, vgs, kgs)
            while deferred:
                deferred.pop(0)()
            # tail outproj(3): attention is done, so psS's 4 banks are free
            # scratch -- 6 parallel psums let all 32 matmuls run back-to-back
            # (no psT-rotation stalls, PE stays ramped); stage copies split
            # DVE/ACT (both idle at the tail)
            sc = []
            for i in range(2):
                t = psS.tile([P, 2, SPAN], f32, tag="psS", name=f"osc{i}")
                sc += [t[:, 0, :], t[:, 1, :]]
            for i in range(2):
                sc.append(psC.tile([P, SPAN], f32, tag="ctx",
                                   name=f"oscc{i}")[:])
            tails = [(tb, os_) for tb in range(12, 16) for os_ in range(2)]
            for gi, (tb, os_) in enumerate(tails):
                ps_ap = sc[gi - 2] if 2 <= gi < 8 else None
                outproj_group(3, tb, os_,
                              nc.sync if os_ == 0 else nc.gpsimd,
                              ps_ap=ps_ap, act_copy=(gi % 2 == 1))()

    nc.compile()
    return nc


def get_nc():
    if "nc" not in _CACHE:
        _CACHE["nc"] = _build()
    return _CACHE["nc"]


def _perm512():
    perm = np.empty(512, np.int64)
    i = 0
    for t in range(2):
        for h in range(2):
            for beta in range(4):
                pr = 2 * t + beta // 2
                u = beta % 2
                for dd in range(32):
                    perm[i] = pr * 128 + u * 64 + h * 32 + dd
                    i += 1
    return perm


def kernel(x, Wq, Wk, Wv, Wo, bo):
    import ml_dtypes
    from concourse import bass_utils

    e4 = ml_dtypes.float8_e4m3

    x = np.asarray(x, dtype=np.float32)
    Wq, Wk, Wv = (np.asarray(w, dtype=np.float32) for w in (Wq, Wk, Wv))
    Wo = np.asarray(Wo, dtype=np.float32)
    bo = np.asarray(bo, dtype=np.float32)
    perm = _perm512()

    in_maps = []
    for c in range(NCORES):
        b, g = c // 2, c % 2
        gsl = slice(g * 512, (g + 1) * 512)
        xT = np.ascontiguousarray(x[b].T)
        x8 = xT.astype(e4)
        wv16 = np.ascontiguousarray(16.0 * Wv[gsl].T)
        wv8h = wv16.astype(e4)
        in_maps.append({
            "x8": x8.view(np.uint8),
            # residual of the e4m3 x quantization (V-proj pass 2)
            "x8l": (xT - x8.astype(np.float32)).astype(e4).view(np.uint8),
            # w quantized x16 to dodge e4m3 subnormals; unscaled on-chip
            "wq8": np.ascontiguousarray((16.0 * Wq[gsl].T)[:, perm])
            .astype(e4).view(np.uint8),
            "wk8": np.ascontiguousarray((16.0 * Wk[gsl].T)[:, perm])
            .astype(e4).view(np.uint8),
            "wv8h": wv8h.view(np.uint8),
            "wv8l": (wv16 - wv8h.astype(np.float32)).astype(e4)
            .view(np.uint8),
            "woh": np.ascontiguousarray(Wo[:, gsl].T).astype(np.float16),
        })

    nc = get_nc()
    res = bass_utils.run_bass_kernel_spmd(nc, in_maps,
                                          core_ids=list(range(NCORES)))
    parts = [res.results[c]["out"].astype(np.float32) for c in range(NCORES)]
    out = np.stack([parts[2 * b] + parts[2 * b + 1] + bo for b in range(B)])
    return out.astype(np.float32)


# revision 60
# speedup vs baseline: 1.0538x; 1.0097x over previous
"""Multi-head causal attention (B=4, T=2048, D=1024, H=16) on 8 Trainium2 cores.

Sharding: core c = (b, g) with b = c//2 (batch), g = c%2 (head-group of 8 heads).
Each core: Q/K/V projections for its 8 heads (column-parallel), causal attention,
row-parallel partial output projection. Host sums the g=0/g=1 partials + bias.

v4 design (fp8 DoubleRow + streaming AV; cost model: matmul = out-free-rows x
0.4167ns x cpr, fp8e4 DoubleRow cpr=0.5 contracting 2x128/instr; ACT exp at
0.8333ns/free-elem = ~147us busy is the floor):
  - Q/K proj: fp8 DR, x8 moving [128,2,512], w8 stationary [128,2,128] in 4
    col-groups (t=pair-half, h=dim-half) so psum partitions land as
    (beta=2*(pr%2)+u, dd) blocks of 32 -> qT8/kT8 [32-blocks, 2h, T] fp8 for
    2x32-contraction DR score matmuls. w quantized x16 (e4m3 subnormal
    dodge), unscaled in the DVE psum->fp8 copy; score scale 1/8 folded as
    qT8 = e4m3(q/2) + exp(scale=0.25). Sub-128 DR needs explicit
    tile_position (base-96 slices break base_partition inference).
  - scores: fp8 DR per (pr,u,kj): out ss [128 keys, 2u, 512-c0] psum.
  - exp: ACT psum -> sbuf, one instr per kj covering both heads. Off-diag
    chunks -> fp8 pt pairs [128, 2kj, 2u, 512]; diag chunks -> bf16 pt
    (pt-quant error only bites concentrated near-diagonal rows) with
    post-exp mask01 multiply on DVE (off ACT's critical path).
  - AV streaming into ctx psum [96, 512] per (pr,u), accumulated across the
    span: off-diag = single-fp8 DR (lhsT = Vbh [128, 2kj, 96]: 64 v-dims +
    ones col at 64 -> Z at psum row 64 + zero pad -- dual-fp8 ldweights
    needs cols%32==0, >=64); diag = bf16 non-DR (lhsT = Vbb [128, 65]).
    Diffuse off-diag rows average out single-fp8 V error; vonly ~1e-3.
  - evict per (pr,u): DVE reciprocal of Z row -> bf16 rz at partition 0
    (cross-partition psum read, hw-verified), gpsimd partition_broadcast ->
    rzbc sbuf, DVE mul ctx*rzbc -> ctx_sb f16 [128=(u,vd), 512] (u1 written
    cross-partition to rows 64:128). Evicts deferred into the next pair's /
    span's kj loop so the PE never stalls on them.
  - outproj: f16, 4x128-contraction per [128q, 512od] psum group; DVE f16
    stage -> DMA (psum DMA is forbidden); host sums g-partials + bias f32.
  - V proj: 3-pass fp8 DR (16v = xh@wh + xl@wh + xh@wl; w x16-scaled
    hi/lo split keeps the value path ~bf16 quality; psum/16 on copy).
  - schedule: only Q(s+1) is paced as filler through attention(s); K(s) and
    V(s) are JIT-emitted inside span s itself (first consumers are its own
    diag QK/AV), spread across pr0/pr1 kj iterations AFTER each emit_qk --
    K keys must land strictly before kj=4s or the diag QK reads stale kT
    (caused a NaN once). outproj(0)@s2, outproj(1,2)@s3, outproj(3) tail.
    All input DMAs on the SP queue (the DMA bus serializes transfers at
    ~1.45us each; ACT-queue DMAs would block exp dispatch); out DMAs split
    SP/Pool. Evicts defer across pair/span boundaries.
Measured: 194317 ns (TimelineSim), rel err 1.21e-2 (gate 2e-2). Baseline
v3 was 240443 ns. ACT exp busy 146.9us (73%), PE 129us, DVE 118us.
Remaining idle: ~10us startup (DMA-bus chain + proj pipe), ~6us span-0/1
boundary (ss-pool bufs=2 serializes the exp chain across spans; psum is
full: psS 4 + psC 2 + psT 2 banks), small scattered bubbles.
"""

import os
import sys

try:
    import concourse.bass  # noqa: F401
except ImportError:  # pragma: no cover
    sys.path.insert(0, "/opt/trn_rl_repo")

import numpy as np

B, T, D = 4, 2048, 1024
H, HD = 16, 64
NCORES = 8
NPAIR = 4
NSPAN = 4
SPAN = 512
KC = 128
P = 128
LAG = 3

_CACHE = {}


def _build():
    import concourse.bacc as bacc
    import concourse.mybir as mybir
    import concourse.tile as tile

    f32 = mybir.dt.float32
    f16 = mybir.dt.float16
    bf16 = mybir.dt.bfloat16
    fp8 = mybir.dt.float8e4
    u8 = mybir.dt.uint8
    Exp = mybir.ActivationFunctionType.Exp

    DR = mybir.MatmulPerfMode.DoubleRow

    nc = bacc.Bacc("TRN2", target_bir_lowering=False, debug=False,
                   num_devices=NCORES)

    x8_h = nc.dram_tensor("x8", (D, T), u8, kind="ExternalInput")
    x8l_h = nc.dram_tensor("x8l", (D, T), u8, kind="ExternalInput")
    wq8_h = nc.dram_tensor("wq8", (D, 512), u8, kind="ExternalInput")
    wk8_h = nc.dram_tensor("wk8", (D, 512), u8, kind="ExternalInput")
    wv8h_h = nc.dram_tensor("wv8h", (D, 512), u8, kind="ExternalInput")
    wv8l_h = nc.dram_tensor("wv8l", (D, 512), u8, kind="ExternalInput")
    woh_h = nc.dram_tensor("woh", (512, D), f16, kind="ExternalInput")
    out_h = nc.dram_tensor("out", (T, D), f16, kind="ExternalOutput")

    # x8: D = (dcp 4, k 2, p 128); per span slice on T
    x8_d = x8_h.ap().rearrange("(dcp k p) t -> p dcp k t", p=P, k=2)
    x8l_d = x8l_h.ap().rearrange("(dcp k p) t -> p dcp k t", p=P, k=2)
    wq8_d = wq8_h.ap().rearrange("(dcp k p) (t h m) -> p dcp k t h m",
                                 p=P, k=2, t=2, h=2)
    wk8_d = wk8_h.ap().rearrange("(dcp k p) (t h m) -> p dcp k t h m",
                                 p=P, k=2, t=2, h=2)
    wv8h_d = wv8h_h.ap().rearrange("(dcp k p) f -> p dcp k f", p=P, k=2)
    wv8l_d = wv8l_h.ap().rearrange("(dcp k p) f -> p dcp k f", p=P, k=2)
    woh_d = woh_h.ap().rearrange("(pc p) f -> p pc f", p=P)

    with tile.TileContext(nc) as tc:
        with (
            tc.tile_pool(name="persist", bufs=1) as persist,
            tc.tile_pool(name="x8p", bufs=2) as x8pool,
            tc.tile_pool(name="xlp", bufs=2) as xlpool,
            tc.tile_pool(name="qp", bufs=4) as qpool,
            tc.tile_pool(name="ptp", bufs=6) as ptpool,
            tc.tile_pool(name="ptb", bufs=5) as ptbpool,
            tc.tile_pool(name="rzp", bufs=8) as rzpool,
            tc.tile_pool(name="cp", bufs=8) as cpool,
            tc.tile_pool(name="stg", bufs=6) as stgpool,
            tc.tile_pool(name="psS", bufs=2, space="PSUM") as psS,
            tc.tile_pool(name="psC", bufs=2, space="PSUM") as psC,
            tc.tile_pool(name="psT", bufs=2, space="PSUM") as psT,
        ):
            wq8 = persist.tile([P, 4, 2, 2, 2, P], u8, tag="wq8", name="wq8")
            wk8 = persist.tile([P, 4, 2, 2, 2, P], u8, tag="wk8", name="wk8")
            wv8 = persist.tile([P, 4, 2, 2, 512], u8, tag="wv8", name="wv8")
            wot = persist.tile([P, 4, D], f16, tag="wot", name="wot")
            kT8 = [persist.tile([P, 2, T], fp8, tag=f"kT8{t}", name=f"kT8{t}")
                   for t in range(2)]
            # Vb: [p, kjp 8, kj 2, pr 4, u 2, 96]: 64 v-dims + ones col at
            # 64 + zero pad to 96 (dual-fp8 ldweights needs cols % 32 == 0,
            # >= 64; psum rows 65:96 are dead)
            VW = 96
            Vbh = persist.tile([P, 8, 2, NPAIR, 2, VW], fp8, tag="Vbh",
                               name="Vbh")
            # diag-chunk AV runs in bf16 (pt quantization error bites only
            # concentrated near-diagonal attention rows): [p, kc, pr, u, 65]
            Vbb = persist.tile([P, 16, NPAIR, 2, HD + 1], bf16, tag="Vbb",
                               name="Vbb")
            mask01 = persist.tile([P, KC], bf16, tag="mask01", name="mask01")
            one = nc.const_aps.tensor(1.0, (P, 1))

            # ---- initial DMAs ----
            x8ts = {0: x8pool.tile([P, 4, 2, SPAN], u8, tag="x8t", name="x8t0")}
            x8ls = {0: xlpool.tile([P, 4, 2, SPAN], u8, tag="x8l", name="x8l0")}
            nc.sync.dma_start(wq8[:], wq8_d)
            nc.sync.dma_start(x8ts[0][:], x8_d[:, :, :, 0:SPAN])
            nc.sync.dma_start(wk8[:], wk8_d)
            nc.sync.dma_start(wv8[:, :, :, 0, :], wv8h_d)
            nc.sync.dma_start(x8ls[0][:], x8l_d[:, :, :, 0:SPAN])
            nc.sync.dma_start(wv8[:, :, :, 1, :], wv8l_d)
            nc.sync.dma_start(wot[:], woh_d)


            # mask01[p, f] = 1 if p <= f else 0 (post-exp diag pt mask)
            nc.gpsimd.memset(mask01[:], 1.0)
            nc.gpsimd.affine_select(
                out=mask01[:], in_=mask01[:],
                compare_op=mybir.AluOpType.is_ge, fill=0.0,
                base=0, channel_multiplier=-1, pattern=[[1, KC]],
            )
            # Vbh pad cols [65:96] must be zero (read by every off-diag
            # AV); data cols are always written before first read
            nslots = 8 * 2 * NPAIR * 2
            nc.gpsimd.memset(
                Vbh[:].rearrange("p a b c d e -> p (a b c d) e")
                [:, :, HD + 1:], 0.0)
            nc.vector.tensor_copy(
                Vbh[:].rearrange("p a b c d e -> p (a b c d) e")
                [:, :, HD:HD + 1],
                one.to_broadcast((P, nslots, 1)))
            nc.vector.tensor_copy(
                Vbb[:].rearrange("p a c d e -> p (a c d) e")
                [:, :, HD:HD + 1],
                one.to_broadcast((P, 16 * NPAIR * 2, 1)))

            qts = {}    # (sp, t) -> [P, 2, SPAN] fp8 tile
            ctxs = {}   # (sp, pr) -> [P, SPAN] f16 tile

            # ---------- emission helpers ----------
            def proj_qk(w8, pr_half, h, sp, x8t, scale, isq, on_act=False):
                t = pr_half

                def emit():
                    ps = psT.tile([P, SPAN], f32, tag="tr", name="psqk")
                    for dcp in range(4):
                        nc.tensor.matmul(
                            ps[:], w8[:, dcp, :, t, h, :].bitcast(fp8),
                            x8t[:, dcp, :, :].bitcast(fp8),
                            start=(dcp == 0), stop=(dcp == 3), perf_mode=DR)
                    if isq:
                        if (sp, t) not in qts:
                            qts[(sp, t)] = qpool.tile(
                                [P, 2, SPAN], fp8, tag=f"qT{t}",
                                name=f"qT{t}_{sp}")
                        dest = qts[(sp, t)][:, h, :]
                    else:
                        dest = kT8[t][:, h, sp * SPAN:(sp + 1) * SPAN]
                    if on_act:
                        # span-0 era: ACT is idle and DVE is the choke --
                        # the psum->fp8 scale-copy runs as activation Copy
                        nc.scalar.activation(
                            dest, ps[:], mybir.ActivationFunctionType.Copy,
                            scale=scale)
                    else:
                        nc.vector.tensor_scalar_mul(dest, ps[:], scale)
                return emit

            def proj_v(sp, tb, x8t, x8lt):
                # 3-pass fp8 DR: 16v = x_hi@w_hi + x_lo@w_hi + x_hi@w_lo
                # (w residual split keeps the value path at ~bf16 quality)
                def emit():
                    ps = psT.tile([P, SPAN], f32, tag="tr", name="psv")
                    ts = slice(tb * P, (tb + 1) * P)
                    passes = [(x8t, 0), (x8lt, 0), (x8t, 1)]
                    for pi, (xt, wl) in enumerate(passes):
                        for dcp in range(4):
                            nc.tensor.matmul(
                                ps[:], xt[:, dcp, :, ts].bitcast(fp8),
                                wv8[:, dcp, :, wl, :].bitcast(fp8),
                                start=(pi == 0 and dcp == 0),
                                stop=(pi == 2 and dcp == 3), perf_mode=DR)
                    kc = sp * 4 + tb
                    psv = ps[:].rearrange("p (pr u v) -> p pr u v",
                                          pr=NPAIR, v=HD)
                    nc.vector.tensor_scalar_mul(
                        Vbh[:, kc // 2, kc % 2, :, :, 0:HD], psv, 1.0 / 16)
                    nc.vector.tensor_scalar_mul(
                        Vbb[:, kc, :, :, 0:HD], psv, 1.0 / 16)
                return emit

            def q_groups(sp, x8t, on_act=False):
                return [proj_qk(wq8, t, h, sp, x8t, 1.0 / 32, True, on_act)
                        for t in range(2) for h in range(2)]

            def k_groups(sp, x8t, on_act=False):
                return [proj_qk(wk8, t, h, sp, x8t, 1.0 / 16, False, on_act)
                        for t in range(2) for h in range(2)]

            def v_groups(sp, x8t, x8lt):
                return [proj_v(sp, tb, x8t, x8lt) for tb in range(4)]

            ostages = {}  # tb -> [P, 2, SPAN] f16 stage (one DMA per tb)

            def outproj_group(sp, tb, os_, dma_q, ps_ap=None, act_copy=False):
                def emit():
                    if ps_ap is None:
                        ps = psT.tile([P, SPAN], f32, tag="tr", name="pso")[:]
                    else:
                        ps = ps_ap
                    for pc in range(NPAIR):
                        nc.tensor.matmul(
                            ps,
                            ctxs[(sp, pc)][:, (tb - sp * 4) * P:
                                           (tb - sp * 4 + 1) * P],
                            wot[:, pc, os_ * SPAN:(os_ + 1) * SPAN],
                            start=(pc == 0), stop=(pc == NPAIR - 1))
                    if os_ == 0:
                        ostages[tb] = stgpool.tile([P, 2, SPAN], f16,
                                                   tag="st", name="stage")
                    stage = ostages[tb]
                    if act_copy:
                        nc.scalar.activation(
                            stage[:, os_, :], ps,
                            mybir.ActivationFunctionType.Copy)
                    else:
                        nc.vector.tensor_copy(stage[:, os_, :], ps)
                    if os_ == 1:
                        # one merged 1024-col DMA per tb: fewer bus slots
                        dma_q.dma_start(
                            out_h.ap()[tb * P:(tb + 1) * P, :],
                            ostages.pop(tb)[:].rearrange("p a f -> p (a f)"))
                return emit

            def outproj_groups(sp):
                # all out-DMAs on SP: a Pool-queue DMACopy hold would block
                # partition_broadcast (evict chain)
                return [outproj_group(sp, tb, os_, nc.sync)
                        for tb in range(sp * 4, (sp + 1) * 4)
                        for os_ in range(2)]

            # ---------- attention for one span ----------
            deferred = []

            def attn_span(s, fillers, vgs=(), kgs=()):
                K = 4 * (s + 1)
                nslot = (K + 4) * NPAIR
                state = {"slot": 0, "fi": 0}

                def pace():
                    tgt = min(len(fillers),
                              len(fillers) * (state["slot"] + 1) // nslot)
                    while state["fi"] < tgt:
                        fillers[state["fi"]]()
                        state["fi"] += 1

                def tick():
                    state["slot"] += 1
                    pace()

                for pr in range(NPAIR):
                    t = pr // 2
                    ctxp = [psC.tile([96, SPAN], f32, tag="ctx",
                                     name=f"ctx{u}") for u in range(2)]
                    ct = cpool.tile([P, SPAN], f16, tag=f"cT{pr}",
                                    name=f"cT{pr}_{s}")
                    ctxs[(s, pr)] = ct
                    pts = {}
                    qt = qts[(s, t)]

                    avn = [0, 0]  # AV matmuls emitted per u (K total each)

                    def emit_qk(kj, pr=pr, t=t, pts=pts, qt=qt):
                        m = kj - 4 * s
                        ss = psS.tile([P, 2, SPAN], f32, tag="psS", name="ss")
                        c0 = 0 if m < 0 else m * KC
                        for u in range(2):
                            b32 = 32 * (2 * (pr % 2) + u)
                            r = slice(b32, b32 + 32)
                            nc.tensor.matmul(
                                ss[:, u, c0:],
                                kT8[t][r, :, kj * KC:(kj + 1) * KC],
                                qt[r, :, c0:],
                                start=True, stop=True, perf_mode=DR,
                                tile_position=(b32, 0))
                        if m < 0:
                            # off-diagonal: fp8 pt pair for DR AV
                            kjp, sl = kj // 2, kj % 2
                            if sl == 0:
                                pts[kjp] = ptpool.tile([P, 2, 2, SPAN], fp8,
                                                       tag="pt", name="pt")
                            nc.scalar.activation(pts[kjp][:, sl, :, :],
                                                 ss[:, :, :], Exp, scale=0.25)
                        else:
                            # diagonal: bf16 pt (exact-ish attention weights
                            # for concentrated rows) + post-exp triangle mask
                            ptb = ptbpool.tile([P, 2, SPAN], bf16,
                                               tag="ptb", name="ptb")
                            pts[("d", kj)] = ptb
                            nc.scalar.activation(ptb[:, :, c0:],
                                                 ss[:, :, c0:], Exp,
                                                 scale=0.25)
                            nc.vector.tensor_mul(
                                ptb[:, :, c0:c0 + KC], ptb[:, :, c0:c0 + KC],
                                mask01[:].rearrange("p (u f) -> p u f", u=1)
                                .to_broadcast((P, 2, KC)))

                    NU = 2 * s + 4  # AV matmuls per u-chain

                    def emit_av8(kjp, pr=pr, pts=pts, ctxp=ctxp, NU=NU):
                        pt = pts.pop(kjp)
                        for u in range(2):
                            nc.tensor.matmul(
                                ctxp[u][:],
                                Vbh[:, kjp, :, pr, u, :],
                                pt[:, :, u, :],
                                start=(avn[u] == 0),
                                stop=(avn[u] == NU - 1),
                                perf_mode=DR, skip_group_check=True)
                            avn[u] += 1

                    def emit_avd(kj, pr=pr, pts=pts, ctxp=ctxp, NU=NU):
                        ptb = pts.pop(("d", kj))
                        c0 = (kj - 4 * s) * KC
                        for u in range(2):
                            nc.tensor.matmul(
                                ctxp[u][0:HD + 1, c0:],
                                Vbb[:, kj, pr, u, :],
                                ptb[:, u, c0:],
                                start=(avn[u] == 0), stop=(avn[u] == NU - 1),
                                skip_group_check=True)
                            avn[u] += 1

                    def evict(pr=pr, ctxp=ctxp, ct=ct, s=s):
                        rz = rzpool.tile([P, 2, SPAN], bf16, tag="rz",
                                         name="rz")
                        rzbc = rzpool.tile([P, 2, SPAN], bf16, tag="rz",
                                           name="rzbc")

                        def fin(u):
                            def run():
                                # 1/Z row: psum row 64 -> sbuf row 0
                                # (cross-partition DVE write, hw-verified)
                                with nc.allow_low_precision(
                                        reason="1/Z bf16: 0.4% on ctx"):
                                    nc.vector.reciprocal(
                                        rz[0:1, u, :], ctxp[u][64:65, :])
                                nc.gpsimd.partition_broadcast(
                                    rzbc[:, u, :], rz[0:1, u, :])
                                nc.vector.tensor_mul(
                                    ct[u * HD:(u + 1) * HD, :],
                                    ctxp[u][0:HD, :], rzbc[0:HD, u, :])
                            return run
                        deferred.append(fin(0))
                        deferred.append(fin(1))

                    # AV work units: (ready_kj, emit_fn); off-diag kjp ready
                    # at its odd kj, diag kj ready at kj
                    units = []
                    for kj2 in range(K):
                        if kj2 < 4 * s:
                            if kj2 % 2 == 1:
                                units.append((kj2, kj2 // 2, emit_av8))
                        else:
                            units.append((kj2, kj2, emit_avd))
                    # JIT placement: V(s) spread across pr0's whole kj
                    # range (consumers: pr0 diag AV); K(s) t0-groups in pr0,
                    # t1-groups in pr1 (consumers: pr0/pr2 diag QK)
                    jit = {}
                    if pr == 0:
                        for i in range(len(vgs)):
                            jit.setdefault(max(i * K // 4, i), []).append(
                                vgs[i])
                        # K(s) t0 keys must precede the diag QK at kj=4s
                        # (jit runs after emit_qk in the same iteration)
                        for i in (0, 1) if kgs else ():
                            jit.setdefault(min(1 + i * (K // 3 + 1),
                                               4 * s - 1), []).append(kgs[i])
                    elif pr == 1:
                        for i in (2, 3) if kgs else ():
                            jit.setdefault(1 + (i - 2) * (K // 3 + 1),
                                           []).append(kgs[i])
                    ui = [0]
                    for kj in range(K):
                        emit_qk(kj)
                        for g in jit.get(kj, ()):
                            g()
                        if deferred:
                            deferred.pop(0)()
                        tick()
                        while (ui[0] < len(units)
                               and units[ui[0]][0] + LAG <= kj):
                            units[ui[0]][2](units[ui[0]][1])
                            ui[0] += 1
                    while ui[0] < len(units):
                        if deferred:
                            deferred.pop(0)()
                        tick()
                        units[ui[0]][2](units[ui[0]][1])
                        ui[0] += 1
                    evict()
                while state["fi"] < len(fillers):
                    fillers[state["fi"]]()
                    state["fi"] += 1

            # ---------- main schedule ----------
            # V(s) and K(s) run inside span s itself (their first consumers
            # are span s's own diag AV / diag QK) -- keeps earlier spans off
            # the PE; only Q(s+1) must finish during span s
            # pr0's attention needs (q,k) t0 groups first: interleave
            qg0 = q_groups(0, x8ts[0], on_act=True)
            kg0 = k_groups(0, x8ts[0], on_act=True)
            for i in range(4):
                qg0[i]()
                kg0[i]()
            for s in range(NSPAN):
                vgs = v_groups(s, x8ts[s], x8ls[s])
                kgs = k_groups(s, x8ts[s]) if s >= 1 else ()
                fillers = []
                if s + 1 < NSPAN:
                    x8t = x8pool.tile([P, 4, 2, SPAN], u8, tag="x8t",
                                      name=f"x8t{s + 1}")
                    x8lt = xlpool.tile([P, 4, 2, SPAN], u8, tag="x8l",
                                       name=f"x8l{s + 1}")
                    x8ts[s + 1], x8ls[s + 1] = x8t, x8lt
                    sl = slice((s + 1) * SPAN, (s + 2) * SPAN)
                    nc.sync.dma_start(x8t[:], x8_d[:, :, :, sl])
                    nc.sync.dma_start(x8lt[:], x8l_d[:, :, :, sl])
                    fillers += q_groups(s + 1, x8t, on_act=(s == 0))
                if s == 2:
                    fillers += outproj_groups(0)
                elif s == 3:
                    fillers += outproj_groups(1) + outproj_groups(2)
                attn_span(s, fillers, vgs, kgs)
            while deferred:
                deferred.pop(0)()
            # tail outproj(3): attention is done, so psS's 4 banks are free
            # scratch -- 6 parallel psums let all 32 matmuls run back-to-back
            # (no psT-rotation stalls, PE stays ramped); stage copies split
            # DVE/ACT (both idle at the tail)
            sc = [psT.tile([P, SPAN], f32, tag="tr", name=f"ost{i}")[:]
                  for i in range(2)]
            for i in range(2):
                t = psS.tile([P, 2, SPAN], f32, tag="psS", name=f"osc{i}")
                sc += [t[:, 0, :], t[:, 1, :]]
            for i in range(2):
                sc.append(psC.tile([P, SPAN], f32, tag="ctx",
                                   name=f"oscc{i}")[:])
            tails = [(tb, os_) for tb in range(12, 16) for os_ in range(2)]
            # pc-interleaved: pc0..2 matmuls only need the earlier pairs'
            # ctx and run under the last evict chain; only pc3 gates on it
            for pc in range(NPAIR):
                for gi, (tb, os_) in enumerate(tails):
                    nc.tensor.matmul(
                        sc[gi],
                        ctxs[(3, pc)][:, (tb - 12) * P:(tb - 11) * P],
                        wot[:, pc, os_ * SPAN:(os_ + 1) * SPAN],
                        start=(pc == 0), stop=(pc == NPAIR - 1),
                        skip_group_check=True)
            tstg = {}
            for gi, (tb, os_) in enumerate(tails):
                if os_ == 0:
                    tstg[tb] = stgpool.tile([P, 2, SPAN], f16, tag="st",
                                            name="tstage")
                if gi % 2 == 1:
                    nc.scalar.activation(
                        tstg[tb][:, os_, :], sc[gi],
                        mybir.ActivationFunctionType.Copy)
                else:
                    nc.vector.tensor_copy(tstg[tb][:, os_, :], sc[gi])
                if os_ == 1:
                    nc.sync.dma_start(
                        out_h.ap()[tb * P:(tb + 1) * P, :],
                        tstg.pop(tb)[:].rearrange("p a f -> p (a f)"))

    nc.compile()
    return nc


def get_nc():
    if "nc" not in _CACHE:
        _CACHE["nc"] = _build()
    return _CACHE["nc"]


def _perm512():
    perm = np.empty(512, np.int64)
    i = 0
    for t in range(2):
        for h in range(2):
            for beta in range(4):
                pr = 2 * t + beta // 2
                u = beta % 2
                for dd in range(32):
                    perm[i] = pr * 128 + u * 64 + h * 32 + dd
                    i += 1
    return perm


def kernel(x, Wq, Wk, Wv, Wo, bo):
    import ml_dtypes
    from concourse import bass_utils

    e4 = ml_dtypes.float8_e4m3

    x = np.asarray(x, dtype=np.float32)
    Wq, Wk, Wv = (np.asarray(w, dtype=np.float32) for w in (Wq, Wk, Wv))
    Wo = np.asarray(Wo, dtype=np.float32)
    bo = np.asarray(bo, dtype=np.float32)
    perm = _perm512()

    in_maps = []
    for c in range(NCORES):
        b, g = c // 2, c % 2
        gsl = slice(g * 512, (g + 1) * 512)
        xT = np.ascontiguousarray(x[b].T)
        x8 = xT.astype(e4)
        wv16 = np.ascontiguousarray(16.0 * Wv[gsl].T)
        wv8h = wv16.astype(e4)
        in_maps.append({
            "x8": x8.view(np.uint8),
            # residual of the e4m3 x quantization (V-proj pass 2)
            "x8l": (xT - x8.astype(np.float32)).astype(e4).view(np.uint8),
            # w quantized x16 to dodge e4m3 subnormals; unscaled on-chip
            "wq8": np.ascontiguousarray((16.0 * Wq[gsl].T)[:, perm])
            .astype(e4).view(np.uint8),
            "wk8": np.ascontiguousarray((16.0 * Wk[gsl].T)[:, perm])
            .astype(e4).view(np.uint8),
            "wv8h": wv8h.view(np.uint8),
            "wv8l": (wv16 - wv8h.astype(np.float32)).astype(e4)
            .view(np.uint8),
            "woh": np.ascontiguousarray(Wo[:, gsl].T).astype(np.float16),
        })

    nc = get_nc()
    res = bass_utils.run_bass_kernel_spmd(nc, in_maps,
                                          core_ids=list(range(NCORES)))
    parts = [res.results[c]["out"].astype(np.float32) for c in range(NCORES)]
    out = np.stack([parts[2 * b] + parts[2 * b + 1] + bo for b in range(B)])
    return out.astype(np.float32)


# revision 64
# speedup vs baseline: 1.0711x; 1.0164x over previous
"""Multi-head causal attention (B=4, T=2048, D=1024, H=16) on 8 Trainium2 cores.

Sharding: core c = (b, g) with b = c//2 (batch), g = c%2 (head-group of 8 heads).
Each core: Q/K/V projections for its 8 heads (column-parallel), causal attention,
row-parallel partial output projection. Host sums the g=0/g=1 partials + bias.

v4 design (fp8 DoubleRow + streaming AV; cost model: matmul = out-free-rows x
0.4167ns x cpr, fp8e4 DoubleRow cpr=0.5 contracting 2x128/instr; ACT exp at
0.8333ns/free-elem = ~147us busy is the floor):
  - Q/K proj: fp8 DR, x8 moving [128,2,512], w8 stationary [128,2,128] in 4
    col-groups (t=pair-half, h=dim-half) so psum partitions land as
    (beta=2*(pr%2)+u, dd) blocks of 32 -> qT8/kT8 [32-blocks, 2h, T] fp8 for
    2x32-contraction DR score matmuls. w quantized x16 (e4m3 subnormal
    dodge), unscaled in the DVE psum->fp8 copy; score scale 1/8 folded as
    qT8 = e4m3(q/2) + exp(scale=0.25). Sub-128 DR needs explicit
    tile_position (base-96 slices break base_partition inference).
  - scores: fp8 DR per (pr,u,kj): out ss [128 keys, 2u, 512-c0] psum.
  - exp: ACT psum -> sbuf, one instr per kj covering both heads. Off-diag
    chunks -> fp8 pt pairs [128, 2kj, 2u, 512]; diag chunks -> bf16 pt
    (pt-quant error only bites concentrated near-diagonal rows) with
    post-exp mask01 multiply on DVE (off ACT's critical path).
  - AV streaming into ctx psum [96, 512] per (pr,u), accumulated across the
    span: off-diag = single-fp8 DR (lhsT = Vbh [128, 2kj, 96]: 64 v-dims +
    ones col at 64 -> Z at psum row 64 + zero pad -- dual-fp8 ldweights
    needs cols%32==0, >=64); diag = bf16 non-DR (lhsT = Vbb [128, 65]).
    Diffuse off-diag rows average out single-fp8 V error; vonly ~1e-3.
  - evict per (pr,u): DVE reciprocal of Z row -> bf16 rz at partition 0
    (cross-partition psum read, hw-verified), gpsimd partition_broadcast ->
    rzbc sbuf, DVE mul ctx*rzbc -> ctx_sb f16 [128=(u,vd), 512] (u1 written
    cross-partition to rows 64:128). Evicts deferred into the next pair's /
    span's kj loop so the PE never stalls on them.
  - outproj: f16, 4x128-contraction per [128q, 512od] psum group; DVE f16
    stage -> DMA (psum DMA is forbidden); host sums g-partials + bias f32.
  - V proj: 3-pass fp8 DR (16v = xh@wh + xl@wh + xh@wl; w x16-scaled
    hi/lo split keeps the value path ~bf16 quality; psum/16 on copy).
  - schedule: only Q(s+1) is paced as filler through attention(s); K(s) and
    V(s) are JIT-emitted inside span s itself (first consumers are its own
    diag QK/AV), spread across pr0/pr1 kj iterations AFTER each emit_qk --
    K keys must land strictly before kj=4s or the diag QK reads stale kT
    (caused a NaN once). outproj(0)@s2, outproj(1,2)@s3, outproj(3) tail.
    ALL DMAs on the SP queue (the DMA bus serializes at ~1.45us/512KB; a
    DMACopy SEQ-hold on ACT blocks exp dispatch, on Pool it blocks
    partition_broadcast). Early proj psum->fp8 copies (span-0 era) run as
    ACT activation-Copy w/ scale (ACT idle there, DVE is the choke) -- but
    ONLY ops whose inputs are ready early (ACT is in-order; late-input
    copies head-of-line block exps). Tail outproj pc-interleaved over 8
    scratch psums; per-tb merged 1024-col out DMAs.
Measured: 192448 ns (TimelineSim), rel err 1.21e-2 (gate 2e-2). Baseline
v3 was 240443 ns. ACT exp busy 146.9us (73%), PE 129us, DVE 118us.
Remaining idle: ~10us startup (DMA-bus chain + proj pipe), ~6us span-0/1
boundary (ss-pool bufs=2 serializes the exp chain across spans; psum is
full: psS 4 + psC 2 + psT 2 banks), small scattered bubbles.
"""

import os
import sys

try:
    import concourse.bass  # noqa: F401
except ImportError:  # pragma: no cover
    sys.path.insert(0, "/opt/trn_rl_repo")

import numpy as np

B, T, D = 4, 2048, 1024
H, HD = 16, 64
NCORES = 8
NPAIR = 4
NSPAN = 4
SPAN = 512
KC = 128
P = 128
LAG = 3

_CACHE = {}


def _build():
    import concourse.bacc as bacc
    import concourse.mybir as mybir
    import concourse.tile as tile

    f32 = mybir.dt.float32
    f16 = mybir.dt.float16
    bf16 = mybir.dt.bfloat16
    fp8 = mybir.dt.float8e4
    u8 = mybir.dt.uint8
    Exp = mybir.ActivationFunctionType.Exp

    DR = mybir.MatmulPerfMode.DoubleRow

    nc = bacc.Bacc("TRN2", target_bir_lowering=False, debug=False,
                   num_devices=NCORES)

    x8_h = nc.dram_tensor("x8", (D, T), u8, kind="ExternalInput")
    x8l_h = nc.dram_tensor("x8l", (D, T), u8, kind="ExternalInput")
    wq8_h = nc.dram_tensor("wq8", (D, 512), u8, kind="ExternalInput")
    wk8_h = nc.dram_tensor("wk8", (D, 512), u8, kind="ExternalInput")
    wv8h_h = nc.dram_tensor("wv8h", (D, 512), u8, kind="ExternalInput")
    wv8l_h = nc.dram_tensor("wv8l", (D, 512), u8, kind="ExternalInput")
    woh_h = nc.dram_tensor("woh", (512, D), f16, kind="ExternalInput")
    out_h = nc.dram_tensor("out", (T, D), f16, kind="ExternalOutput")

    # x8: D = (dcp 4, k 2, p 128); per span slice on T
    x8_d = x8_h.ap().rearrange("(dcp k p) t -> p dcp k t", p=P, k=2)
    x8l_d = x8l_h.ap().rearrange("(dcp k p) t -> p dcp k t", p=P, k=2)
    wq8_d = wq8_h.ap().rearrange("(dcp k p) (t h m) -> p dcp k t h m",
                                 p=P, k=2, t=2, h=2)
    wk8_d = wk8_h.ap().rearrange("(dcp k p) (t h m) -> p dcp k t h m",
                                 p=P, k=2, t=2, h=2)
    wv8h_d = wv8h_h.ap().rearrange("(dcp k p) f -> p dcp k f", p=P, k=2)
    wv8l_d = wv8l_h.ap().rearrange("(dcp k p) f -> p dcp k f", p=P, k=2)
    woh_d = woh_h.ap().rearrange("(pc p) f -> p pc f", p=P)

    with tile.TileContext(nc) as tc:
        with (
            tc.tile_pool(name="persist", bufs=1) as persist,
            tc.tile_pool(name="x8p", bufs=2) as x8pool,
            tc.tile_pool(name="xlp", bufs=2) as xlpool,
            tc.tile_pool(name="qp", bufs=4) as qpool,
            tc.tile_pool(name="ptp", bufs=6) as ptpool,
            tc.tile_pool(name="ptb", bufs=5) as ptbpool,
            tc.tile_pool(name="rzp", bufs=8) as rzpool,
            tc.tile_pool(name="cp", bufs=8) as cpool,
            tc.tile_pool(name="stg", bufs=6) as stgpool,
            tc.tile_pool(name="psS", bufs=2, space="PSUM") as psS,
            tc.tile_pool(name="psC", bufs=2, space="PSUM") as psC,
            tc.tile_pool(name="psT", bufs=2, space="PSUM") as psT,
        ):
            wq8 = persist.tile([P, 4, 2, 2, 2, P], u8, tag="wq8", name="wq8")
            wk8 = persist.tile([P, 4, 2, 2, 2, P], u8, tag="wk8", name="wk8")
            wv8 = persist.tile([P, 4, 2, 2, 512], u8, tag="wv8", name="wv8")
            wot = persist.tile([P, 4, D], f16, tag="wot", name="wot")
            kT8 = [persist.tile([P, 2, T], fp8, tag=f"kT8{t}", name=f"kT8{t}")
                   for t in range(2)]
            # Vb: [p, kjp 8, kj 2, pr 4, u 2, 96]: 64 v-dims + ones col at
            # 64 + zero pad to 96 (dual-fp8 ldweights needs cols % 32 == 0,
            # >= 64; psum rows 65:96 are dead)
            VW = 96
            Vbh = persist.tile([P, 8, 2, NPAIR, 2, VW], fp8, tag="Vbh",
                               name="Vbh")
            # diag-chunk AV runs in bf16 (pt quantization error bites only
            # concentrated near-diagonal attention rows): [p, kc, pr, u, 65]
            Vbb = persist.tile([P, 16, NPAIR, 2, HD + 1], bf16, tag="Vbb",
                               name="Vbb")
            mask01 = persist.tile([P, KC], bf16, tag="mask01", name="mask01")
            one = nc.const_aps.tensor(1.0, (P, 1))

            # ---- initial DMAs ----
            x8ts = {0: x8pool.tile([P, 4, 2, SPAN], u8, tag="x8t", name="x8t0")}
            x8ls = {0: xlpool.tile([P, 4, 2, SPAN], u8, tag="x8l", name="x8l0")}
            nc.sync.dma_start(wq8[:], wq8_d)
            nc.sync.dma_start(x8ts[0][:], x8_d[:, :, :, 0:SPAN])
            nc.sync.dma_start(wk8[:], wk8_d)
            nc.sync.dma_start(wv8[:, :, :, 0, :], wv8h_d)
            nc.sync.dma_start(x8ls[0][:], x8l_d[:, :, :, 0:SPAN])
            nc.sync.dma_start(wv8[:, :, :, 1, :], wv8l_d)
            nc.sync.dma_start(wot[:], woh_d)


            # mask01[p, f] = 1 if p <= f else 0 (post-exp diag pt mask)
            nc.gpsimd.memset(mask01[:], 1.0)
            nc.gpsimd.affine_select(
                out=mask01[:], in_=mask01[:],
                compare_op=mybir.AluOpType.is_ge, fill=0.0,
                base=0, channel_multiplier=-1, pattern=[[1, KC]],
            )
            # Vbh pad cols [65:96] must be zero (read by every off-diag
            # AV); data cols are always written before first read
            nslots = 8 * 2 * NPAIR * 2
            nc.gpsimd.memset(
                Vbh[:].rearrange("p a b c d e -> p (a b c d) e")
                [:, :, HD + 1:], 0.0)
            nc.vector.tensor_copy(
                Vbh[:].rearrange("p a b c d e -> p (a b c d) e")
                [:, :, HD:HD + 1],
                one.to_broadcast((P, nslots, 1)))
            nc.vector.tensor_copy(
                Vbb[:].rearrange("p a c d e -> p (a c d) e")
                [:, :, HD:HD + 1],
                one.to_broadcast((P, 16 * NPAIR * 2, 1)))

            qts = {}    # (sp, t) -> [P, 2, SPAN] fp8 tile
            ctxs = {}   # (sp, pr) -> [P, SPAN] f16 tile

            # ---------- emission helpers ----------
            def proj_qk(w8, pr_half, h, sp, x8t, scale, isq, on_act=False):
                t = pr_half

                def emit():
                    ps = psT.tile([P, SPAN], f32, tag="tr", name="psqk")
                    for dcp in range(4):
                        nc.tensor.matmul(
                            ps[:], w8[:, dcp, :, t, h, :].bitcast(fp8),
                            x8t[:, dcp, :, :].bitcast(fp8),
                            start=(dcp == 0), stop=(dcp == 3), perf_mode=DR)
                    if isq:
                        if (sp, t) not in qts:
                            qts[(sp, t)] = qpool.tile(
                                [P, 2, SPAN], fp8, tag=f"qT{t}",
                                name=f"qT{t}_{sp}")
                        dest = qts[(sp, t)][:, h, :]
                    else:
                        dest = kT8[t][:, h, sp * SPAN:(sp + 1) * SPAN]
                    if on_act:
                        # span-0 era: ACT is idle and DVE is the choke --
                        # the psum->fp8 scale-copy runs as activation Copy
                        nc.scalar.activation(
                            dest, ps[:], mybir.ActivationFunctionType.Copy,
                            scale=scale)
                    else:
                        nc.vector.tensor_scalar_mul(dest, ps[:], scale)
                return emit

            def proj_v(sp, tb, x8t, x8lt):
                # 3-pass fp8 DR: 16v = x_hi@w_hi + x_lo@w_hi + x_hi@w_lo
                # (w residual split keeps the value path at ~bf16 quality)
                def emit():
                    ps = psT.tile([P, SPAN], f32, tag="tr", name="psv")
                    ts = slice(tb * P, (tb + 1) * P)
                    passes = [(x8t, 0), (x8lt, 0), (x8t, 1)]
                    for pi, (xt, wl) in enumerate(passes):
                        for dcp in range(4):
                            nc.tensor.matmul(
                                ps[:], xt[:, dcp, :, ts].bitcast(fp8),
                                wv8[:, dcp, :, wl, :].bitcast(fp8),
                                start=(pi == 0 and dcp == 0),
                                stop=(pi == 2 and dcp == 3), perf_mode=DR)
                    kc = sp * 4 + tb
                    psv = ps[:].rearrange("p (pr u v) -> p pr u v",
                                          pr=NPAIR, v=HD)
                    nc.vector.tensor_scalar_mul(
                        Vbh[:, kc // 2, kc % 2, :, :, 0:HD], psv, 1.0 / 16)
                    nc.vector.tensor_scalar_mul(
                        Vbb[:, kc, :, :, 0:HD], psv, 1.0 / 16)
                return emit

            def q_groups(sp, x8t, on_act=False):
                return [proj_qk(wq8, t, h, sp, x8t, 1.0 / 32, True, on_act)
                        for t in range(2) for h in range(2)]

            def k_groups(sp, x8t, on_act=False):
                return [proj_qk(wk8, t, h, sp, x8t, 1.0 / 16, False, on_act)
                        for t in range(2) for h in range(2)]

            def v_groups(sp, x8t, x8lt):
                return [proj_v(sp, tb, x8t, x8lt) for tb in range(4)]

            ostages = {}  # tb -> [P, 2, SPAN] f16 stage (one DMA per tb)

            def outproj_group(sp, tb, os_, dma_q, ps_ap=None, act_copy=False):
                def emit():
                    if ps_ap is None:
                        ps = psT.tile([P, SPAN], f32, tag="tr", name="pso")[:]
                    else:
                        ps = ps_ap
                    for pc in range(NPAIR):
                        nc.tensor.matmul(
                            ps,
                            ctxs[(sp, pc)][:, (tb - sp * 4) * P:
                                           (tb - sp * 4 + 1) * P],
                            wot[:, pc, os_ * SPAN:(os_ + 1) * SPAN],
                            start=(pc == 0), stop=(pc == NPAIR - 1))
                    if os_ == 0:
                        ostages[tb] = stgpool.tile([P, 2, SPAN], f16,
                                                   tag="st", name="stage")
                    stage = ostages[tb]
                    if act_copy:
                        nc.scalar.activation(
                            stage[:, os_, :], ps,
                            mybir.ActivationFunctionType.Copy)
                    else:
                        nc.vector.tensor_copy(stage[:, os_, :], ps)
                    if os_ == 1:
                        # one merged 1024-col DMA per tb: fewer bus slots
                        dma_q.dma_start(
                            out_h.ap()[tb * P:(tb + 1) * P, :],
                            ostages.pop(tb)[:].rearrange("p a f -> p (a f)"))
                return emit

            def outproj_groups(sp):
                # all out-DMAs on SP: a Pool-queue DMACopy hold would block
                # partition_broadcast (evict chain)
                return [outproj_group(sp, tb, os_, nc.sync)
                        for tb in range(sp * 4, (sp + 1) * 4)
                        for os_ in range(2)]

            # ---------- attention for one span ----------
            deferred = []

            def attn_span(s, fillers, vgs=(), kgs=()):
                K = 4 * (s + 1)
                nslot = (K + 4) * NPAIR
                state = {"slot": 0, "fi": 0}

                def pace():
                    tgt = min(len(fillers),
                              len(fillers) * (state["slot"] + 1) // nslot)
                    while state["fi"] < tgt:
                        fillers[state["fi"]]()
                        state["fi"] += 1

                def tick():
                    state["slot"] += 1
                    pace()

                for pr in range(NPAIR):
                    t = pr // 2
                    ctxp = [psC.tile([96, SPAN], f32, tag="ctx",
                                     name=f"ctx{u}") for u in range(2)]
                    ct = cpool.tile([P, SPAN], f16, tag=f"cT{pr}",
                                    name=f"cT{pr}_{s}")
                    ctxs[(s, pr)] = ct
                    pts = {}
                    qt = qts[(s, t)]

                    avn = [0, 0]  # AV matmuls emitted per u (K total each)

                    def emit_qk(kj, pr=pr, t=t, pts=pts, qt=qt):
                        m = kj - 4 * s
                        c0 = 0 if m < 0 else m * KC
                        if m == 2:
                            # m2+m3 share one ss tile and one exp: m2 scores
                            # at cols [256:512], m3's 128 live cols parked in
                            # the unused [128:256]; exp'd together at m3
                            ss = psS.tile([P, 2, SPAN], f32, tag="psS",
                                          name="ss23")
                            pts[("ss23",)] = ss
                        elif m == 3:
                            ss = pts[("ss23",)]
                        else:
                            ss = psS.tile([P, 2, SPAN], f32, tag="psS",
                                          name="ss")
                        dst = slice(KC, 2 * KC) if m == 3 else slice(c0, SPAN)
                        for u in range(2):
                            b32 = 32 * (2 * (pr % 2) + u)
                            r = slice(b32, b32 + 32)
                            nc.tensor.matmul(
                                ss[:, u, dst],
                                kT8[t][r, :, kj * KC:(kj + 1) * KC],
                                qt[r, :, c0:],
                                start=True, stop=True, perf_mode=DR,
                                tile_position=(b32, 0))
                        if m < 0:
                            # off-diagonal: fp8 pt pair for DR AV
                            kjp, sl = kj // 2, kj % 2
                            if sl == 0:
                                pts[kjp] = ptpool.tile([P, 2, 2, SPAN], fp8,
                                                       tag="pt", name="pt")
                            nc.scalar.activation(pts[kjp][:, sl, :, :],
                                                 ss[:, :, :], Exp, scale=0.25)
                        elif m == 2:
                            # defer: exp'd together with m3
                            ptb = ptbpool.tile([P, 2, SPAN], bf16,
                                               tag="ptb", name="ptb")
                            pts[("d", kj)] = ptb
                            pts[("d", kj + 1)] = ptb
                        elif m == 3:
                            ptb = pts[("d", kj)]
                            del pts[("ss23",)]
                            nc.scalar.activation(ptb[:, :, KC:],
                                                 ss[:, :, KC:], Exp,
                                                 scale=0.25)
                            # both triangles ([128:256] = m3, [256:384] = m2)
                            # use the same 128x128 mask: one DVE mul
                            nc.vector.tensor_mul(
                                ptb[:, :, KC:3 * KC].rearrange(
                                    "p u (b f) -> p u b f", b=2),
                                ptb[:, :, KC:3 * KC].rearrange(
                                    "p u (b f) -> p u b f", b=2),
                                mask01[:].rearrange("p (u b f) -> p u b f",
                                                    u=1, b=1)
                                .to_broadcast((P, 2, 2, KC)))
                        else:
                            # diagonal m0/m1: own tile + exp + triangle mask
                            ptb = ptbpool.tile([P, 2, SPAN], bf16,
                                               tag="ptb", name="ptb")
                            pts[("d", kj)] = ptb
                            nc.scalar.activation(ptb[:, :, c0:],
                                                 ss[:, :, c0:], Exp,
                                                 scale=0.25)
                            nc.vector.tensor_mul(
                                ptb[:, :, c0:c0 + KC], ptb[:, :, c0:c0 + KC],
                                mask01[:].rearrange("p (u f) -> p u f", u=1)
                                .to_broadcast((P, 2, KC)))

                    NU = 2 * s + 4  # AV matmuls per u-chain

                    def emit_av8(kjp, pr=pr, pts=pts, ctxp=ctxp, NU=NU):
                        pt = pts.pop(kjp)
                        for u in range(2):
                            nc.tensor.matmul(
                                ctxp[u][:],
                                Vbh[:, kjp, :, pr, u, :],
                                pt[:, :, u, :],
                                start=(avn[u] == 0),
                                stop=(avn[u] == NU - 1),
                                perf_mode=DR, skip_group_check=True)
                            avn[u] += 1

                    def emit_avd(kj, pr=pr, pts=pts, ctxp=ctxp, NU=NU):
                        ptb = pts.pop(("d", kj))
                        m = kj - 4 * s
                        c0 = m * KC
                        # m3's pt lives at stored cols [128:256] (positional
                        # matmul mapping: out offset is independent of rhs)
                        rsl = slice(KC, 2 * KC) if m == 3 else slice(c0, SPAN)
                        for u in range(2):
                            nc.tensor.matmul(
                                ctxp[u][0:HD + 1, c0:],
                                Vbb[:, kj, pr, u, :],
                                ptb[:, u, rsl],
                                start=(avn[u] == 0), stop=(avn[u] == NU - 1),
                                skip_group_check=True)
                            avn[u] += 1

                    def evict(pr=pr, ctxp=ctxp, ct=ct, s=s):
                        rz = rzpool.tile([P, 2, SPAN], bf16, tag="rz",
                                         name="rz")
                        rzbc = rzpool.tile([P, 2, SPAN], bf16, tag="rz",
                                           name="rzbc")

                        def fin(u):
                            def run():
                                # 1/Z row: psum row 64 -> sbuf row 0
                                # (cross-partition DVE write, hw-verified)
                                with nc.allow_low_precision(
                                        reason="1/Z bf16: 0.4% on ctx"):
                                    nc.vector.reciprocal(
                                        rz[0:1, u, :], ctxp[u][64:65, :])
                                nc.gpsimd.partition_broadcast(
                                    rzbc[:, u, :], rz[0:1, u, :])
                                nc.vector.tensor_mul(
                                    ct[u * HD:(u + 1) * HD, :],
                                    ctxp[u][0:HD, :], rzbc[0:HD, u, :])
                            return run
                        deferred.append(fin(0))
                        deferred.append(fin(1))

                    # AV work units: (ready_kj, emit_fn); off-diag kjp ready
                    # at its odd kj, diag kj ready at kj
                    units = []
                    for kj2 in range(K):
                        if kj2 < 4 * s:
                            if kj2 % 2 == 1:
                                units.append((kj2, kj2 // 2, emit_av8))
                        else:
                            units.append((kj2, kj2, emit_avd))
                    # JIT placement: V(s) spread across pr0's whole kj
                    # range (consumers: pr0 diag AV); K(s) t0-groups in pr0,
                    # t1-groups in pr1 (consumers: pr0/pr2 diag QK)
                    jit = {}
                    if pr == 0:
                        for i in range(len(vgs)):
                            jit.setdefault(max(i * K // 4, i), []).append(
                                vgs[i])
                        # K(s) t0 keys must precede the diag QK at kj=4s
                        # (jit runs after emit_qk in the same iteration)
                        for i in (0, 1) if kgs else ():
                            jit.setdefault(min(1 + i * (K // 3 + 1),
                                               4 * s - 1), []).append(kgs[i])
                    elif pr == 1:
                        for i in (2, 3) if kgs else ():
                            jit.setdefault(1 + (i - 2) * (K // 3 + 1),
                                           []).append(kgs[i])
                    ui = [0]
                    for kj in range(K):
                        emit_qk(kj)
                        for g in jit.get(kj, ()):
                            g()
                        if deferred:
                            deferred.pop(0)()
                        tick()
                        while (ui[0] < len(units)
                               and units[ui[0]][0] + LAG <= kj):
                            units[ui[0]][2](units[ui[0]][1])
                            ui[0] += 1
                    while ui[0] < len(units):
                        if deferred:
                            deferred.pop(0)()
                        tick()
                        units[ui[0]][2](units[ui[0]][1])
                        ui[0] += 1
                    evict()
                while state["fi"] < len(fillers):
                    fillers[state["fi"]]()
                    state["fi"] += 1

            # ---------- main schedule ----------
            # V(s) and K(s) run inside span s itself (their first consumers
            # are span s's own diag AV / diag QK) -- keeps earlier spans off
            # the PE; only Q(s+1) must finish during span s
            # pr0's attention needs (q,k) t0 groups first: interleave
            qg0 = q_groups(0, x8ts[0], on_act=True)
            kg0 = k_groups(0, x8ts[0], on_act=True)
            for i in range(4):
                qg0[i]()
                kg0[i]()
            for s in range(NSPAN):
                vgs = v_groups(s, x8ts[s], x8ls[s])
                kgs = k_groups(s, x8ts[s]) if s >= 1 else ()
                fillers = []
                if s + 1 < NSPAN:
                    x8t = x8pool.tile([P, 4, 2, SPAN], u8, tag="x8t",
                                      name=f"x8t{s + 1}")
                    x8lt = xlpool.tile([P, 4, 2, SPAN], u8, tag="x8l",
                                       name=f"x8l{s + 1}")
                    x8ts[s + 1], x8ls[s + 1] = x8t, x8lt
                    sl = slice((s + 1) * SPAN, (s + 2) * SPAN)
                    nc.sync.dma_start(x8t[:], x8_d[:, :, :, sl])
                    nc.sync.dma_start(x8lt[:], x8l_d[:, :, :, sl])
                    fillers += q_groups(s + 1, x8t, on_act=(s == 0))
                if s == 2:
                    fillers += outproj_groups(0)
                elif s == 3:
                    fillers += outproj_groups(1) + outproj_groups(2)
                attn_span(s, fillers, vgs, kgs)
            while deferred:
                deferred.pop(0)()
            # tail outproj(3): attention is done, so psS's 4 banks are free
            # scratch -- 6 parallel psums let all 32 matmuls run back-to-back
            # (no psT-rotation stalls, PE stays ramped); stage copies split
            # DVE/ACT (both idle at the tail)
            sc = [psT.tile([P, SPAN], f32, tag="tr", name=f"ost{i}")[:]
                  for i in range(2)]
            for i in range(2):
                t = psS.tile([P, 2, SPAN], f32, tag="psS", name=f"osc{i}")
                sc += [t[:, 0, :], t[:, 1, :]]
            for i in range(2):
                sc.append(psC.tile([P, SPAN], f32, tag="ctx",
                                   name=f"oscc{i}")[:])
            tails = [(tb, os_) for tb in range(12, 16) for os_ in range(2)]
            # pc-interleaved: pc0..2 matmuls only need the earlier pairs'
            # ctx and run under the last evict chain; only pc3 gates on it
            for pc in range(NPAIR):
                for gi, (tb, os_) in enumerate(tails):
                    nc.tensor.matmul(
                        sc[gi],
                        ctxs[(3, pc)][:, (tb - 12) * P:(tb - 11) * P],
                        wot[:, pc, os_ * SPAN:(os_ + 1) * SPAN],
                        start=(pc == 0), stop=(pc == NPAIR - 1),
                        skip_group_check=True)
            tstg = {}
            for gi, (tb, os_) in enumerate(tails):
                if os_ == 0:
                    tstg[tb] = stgpool.tile([P, 2, SPAN], f16, tag="st",
                                            name="tstage")
                if gi % 2 == 1:
                    nc.scalar.activation(
                        tstg[tb][:, os_, :], sc[gi],
                        mybir.ActivationFunctionType.Copy)
                else:
                    nc.vector.tensor_copy(tstg[tb][:, os_, :], sc[gi])
                if os_ == 1:
                    nc.sync.dma_start(
                        out_h.ap()[tb * P:(tb + 1) * P, :],
                        tstg.pop(tb)[:].rearrange("p a f -> p (a f)"))

    nc.compile()
    return nc


def get_nc():
    if "nc" not in _CACHE:
        _CACHE["nc"] = _build()
    return _CACHE["nc"]


def _perm512():
    perm = np.empty(512, np.int64)
    i = 0
    for t in range(2):
        for h in range(2):
            for beta in range(4):
                pr = 2 * t + beta // 2
                u = beta % 2
                for dd in range(32):
                    perm[i] = pr * 128 + u * 64 + h * 32 + dd
                    i += 1
    return perm


def kernel(x, Wq, Wk, Wv, Wo, bo):
    import ml_dtypes
    from concourse import bass_utils

    e4 = ml_dtypes.float8_e4m3

    x = np.asarray(x, dtype=np.float32)
    Wq, Wk, Wv = (np.asarray(w, dtype=np.float32) for w in (Wq, Wk, Wv))
    Wo = np.asarray(Wo, dtype=np.float32)
    bo = np.asarray(bo, dtype=np.float32)
    perm = _perm512()

    in_maps = []
    for c in range(NCORES):
        b, g = c // 2, c % 2
        gsl = slice(g * 512, (g + 1) * 512)
        xT = np.ascontiguousarray(x[b].T)
        x8 = xT.astype(e4)
        wv16 = np.ascontiguousarray(16.0 * Wv[gsl].T)
        wv8h = wv16.astype(e4)
        in_maps.append({
            "x8": x8.view(np.uint8),
            # residual of the e4m3 x quantization (V-proj pass 2)
            "x8l": (xT - x8.astype(np.float32)).astype(e4).view(np.uint8),
            # w quantized x16 to dodge e4m3 subnormals; unscaled on-chip
            "wq8": np.ascontiguousarray((16.0 * Wq[gsl].T)[:, perm])
            .astype(e4).view(np.uint8),
            "wk8": np.ascontiguousarray((16.0 * Wk[gsl].T)[:, perm])
            .astype(e4).view(np.uint8),
            "wv8h": wv8h.view(np.uint8),
            "wv8l": (wv16 - wv8h.astype(np.float32)).astype(e4)
            .view(np.uint8),
            "woh": np.ascontiguousarray(Wo[:, gsl].T).astype(np.float16),
        })

    nc = get_nc()
    res = bass_utils.run_bass_kernel_spmd(nc, in_maps,
                                          core_ids=list(range(NCORES)))
    parts = [res.results[c]["out"].astype(np.float32) for c in range(NCORES)]
    out = np.stack([parts[2 * b] + parts[2 * b + 1] + bo for b in range(B)])
    return out.astype(np.float32)


# revision 66
# speedup vs baseline: 1.0767x; 1.0053x over previous
"""Multi-head causal attention (B=4, T=2048, D=1024, H=16) on 8 Trainium2 cores.

Sharding: core c = (b, g) with b = c//2 (batch), g = c%2 (head-group of 8 heads).
Each core: Q/K/V projections for its 8 heads (column-parallel), causal attention,
row-parallel partial output projection. Host sums the g=0/g=1 partials + bias.

v4 design (fp8 DoubleRow + streaming AV; cost model: matmul = out-free-rows x
0.4167ns x cpr, fp8e4 DoubleRow cpr=0.5 contracting 2x128/instr; ACT exp at
0.8333ns/free-elem = ~147us busy is the floor):
  - Q/K proj: fp8 DR, x8 moving [128,2,512], w8 stationary [128,2,128] in 4
    col-groups (t=pair-half, h=dim-half) so psum partitions land as
    (beta=2*(pr%2)+u, dd) blocks of 32 -> qT8/kT8 [32-blocks, 2h, T] fp8 for
    2x32-contraction DR score matmuls. w quantized x16 (e4m3 subnormal
    dodge), unscaled in the DVE psum->fp8 copy; score scale 1/8 folded as
    qT8 = e4m3(q/2) + exp(scale=0.25). Sub-128 DR needs explicit
    tile_position (base-96 slices break base_partition inference).
  - scores: fp8 DR per (pr,u,kj): out ss [128 keys, 2u, 512-c0] psum.
  - exp: ACT psum -> sbuf, one instr per kj covering both heads. Off-diag
    chunks -> fp8 pt pairs [128, 2kj, 2u, 512]; diag chunks -> bf16 pt
    (pt-quant error only bites concentrated near-diagonal rows) with
    post-exp mask01 multiply on DVE (off ACT's critical path). Diag m2+m3
    share one ss tile (m3's 128 live cols parked at [128:256]) and one
    exp + one mask-mul; the AV remaps positionally (out col offset is
    independent of rhs offset) -- same elements, 16 fewer ACT instrs.
  - AV streaming into ctx psum [96, 512] per (pr,u), accumulated across the
    span: off-diag = single-fp8 DR (lhsT = Vbh [128, 2kj, 96]: 64 v-dims +
    ones col at 64 -> Z at psum row 64 + zero pad -- dual-fp8 ldweights
    needs cols%32==0, >=64); diag = bf16 non-DR (lhsT = Vbb [128, 65]).
    Diffuse off-diag rows average out single-fp8 V error; vonly ~1e-3.
  - evict per (pr,u): DVE reciprocal of Z row -> bf16 rz at partition 0
    (cross-partition psum read, hw-verified), gpsimd partition_broadcast ->
    rzbc sbuf, DVE mul ctx*rzbc -> ctx_sb f16 [128=(u,vd), 512] (u1 written
    cross-partition to rows 64:128). Evicts deferred into the next pair's /
    span's kj loop so the PE never stalls on them.
  - outproj: f16, 4x128-contraction per [128q, 512od] psum group; DVE f16
    stage -> DMA (psum DMA is forbidden); host sums g-partials + bias f32.
  - V proj: 3-pass fp8 DR (16v = xh@wh + xl@wh + xh@wl; w x16-scaled
    hi/lo split keeps the value path ~bf16 quality; psum/16 on copy).
  - schedule: only Q(s+1) is paced as filler through attention(s); K(s) and
    V(s) are JIT-emitted inside span s itself (first consumers are its own
    diag QK/AV), spread across pr0/pr1 kj iterations AFTER each emit_qk --
    K keys must land strictly before kj=4s or the diag QK reads stale kT
    (caused a NaN once). outproj(0)@s2, outproj(1,2)@s3, outproj(3) tail.
    ALL DMAs on the SP queue (the DMA bus serializes at ~1.45us/512KB; a
    DMACopy SEQ-hold on ACT blocks exp dispatch, on Pool it blocks
    partition_broadcast). Early proj psum->fp8 copies (span-0 era) run as
    ACT activation-Copy w/ scale (ACT idle there, DVE is the choke) -- but
    ONLY ops whose inputs are ready early (ACT is in-order; late-input
    copies head-of-line block exps). Tail outproj pc-interleaved over 8
    scratch psums; per-tb merged 1024-col out DMAs.
Measured: 189347 ns (TimelineSim), rel err 1.21e-2 (gate 2e-2). Baseline
v3 was 240443 ns. ACT exp busy 146.9us (73%), PE 129us, DVE 118us.
Remaining idle: ~10us startup (DMA-bus chain + proj pipe), ~6us span-0/1
boundary (ss-pool bufs=2 serializes the exp chain across spans; psum is
full: psS 4 + psC 2 + psT 2 banks), small scattered bubbles.
"""

import os
import sys

try:
    import concourse.bass  # noqa: F401
except ImportError:  # pragma: no cover
    sys.path.insert(0, "/opt/trn_rl_repo")

import numpy as np

B, T, D = 4, 2048, 1024
H, HD = 16, 64
NCORES = 8
NPAIR = 4
NSPAN = 4
SPAN = 512
KC = 128
P = 128
LAG = 3

_CACHE = {}


def _build():
    import concourse.bacc as bacc
    import concourse.mybir as mybir
    import concourse.tile as tile

    f32 = mybir.dt.float32
    f16 = mybir.dt.float16
    bf16 = mybir.dt.bfloat16
    fp8 = mybir.dt.float8e4
    u8 = mybir.dt.uint8
    Exp = mybir.ActivationFunctionType.Exp

    DR = mybir.MatmulPerfMode.DoubleRow

    nc = bacc.Bacc("TRN2", target_bir_lowering=False, debug=False,
                   num_devices=NCORES)

    x8_h = nc.dram_tensor("x8", (D, T), u8, kind="ExternalInput")
    x8l_h = nc.dram_tensor("x8l", (D, T), u8, kind="ExternalInput")
    wq8_h = nc.dram_tensor("wq8", (D, 512), u8, kind="ExternalInput")
    wk8_h = nc.dram_tensor("wk8", (D, 512), u8, kind="ExternalInput")
    wv8h_h = nc.dram_tensor("wv8h", (D, 512), u8, kind="ExternalInput")
    wv8l_h = nc.dram_tensor("wv8l", (D, 512), u8, kind="ExternalInput")
    woh_h = nc.dram_tensor("woh", (512, D), f16, kind="ExternalInput")
    out_h = nc.dram_tensor("out", (T, D), f16, kind="ExternalOutput")

    # x8: D = (dcp 4, k 2, p 128); per span slice on T
    x8_d = x8_h.ap().rearrange("(dcp k p) t -> p dcp k t", p=P, k=2)
    x8l_d = x8l_h.ap().rearrange("(dcp k p) t -> p dcp k t", p=P, k=2)
    wq8_d = wq8_h.ap().rearrange("(dcp k p) (t h m) -> p dcp k t h m",
                                 p=P, k=2, t=2, h=2)
    wk8_d = wk8_h.ap().rearrange("(dcp k p) (t h m) -> p dcp k t h m",
                                 p=P, k=2, t=2, h=2)
    wv8h_d = wv8h_h.ap().rearrange("(dcp k p) f -> p dcp k f", p=P, k=2)
    wv8l_d = wv8l_h.ap().rearrange("(dcp k p) f -> p dcp k f", p=P, k=2)
    woh_d = woh_h.ap().rearrange("(pc p) f -> p pc f", p=P)

    with tile.TileContext(nc) as tc:
        with (
            tc.tile_pool(name="persist", bufs=1) as persist,
            tc.tile_pool(name="x8p", bufs=2) as x8pool,
            tc.tile_pool(name="xlp", bufs=2) as xlpool,
            tc.tile_pool(name="qp", bufs=4) as qpool,
            tc.tile_pool(name="ptp", bufs=6) as ptpool,
            tc.tile_pool(name="ptb", bufs=5) as ptbpool,
            tc.tile_pool(name="rzp", bufs=8) as rzpool,
            tc.tile_pool(name="cp", bufs=8) as cpool,
            tc.tile_pool(name="stg", bufs=6) as stgpool,
            tc.tile_pool(name="psS", bufs=2, space="PSUM") as psS,
            tc.tile_pool(name="psC", bufs=2, space="PSUM") as psC,
            tc.tile_pool(name="psT", bufs=2, space="PSUM") as psT,
        ):
            wq8 = persist.tile([P, 4, 2, 2, 2, P], u8, tag="wq8", name="wq8")
            wk8 = persist.tile([P, 4, 2, 2, 2, P], u8, tag="wk8", name="wk8")
            wv8 = persist.tile([P, 4, 2, 2, 512], u8, tag="wv8", name="wv8")
            wot = persist.tile([P, 4, D], f16, tag="wot", name="wot")
            kT8 = [persist.tile([P, 2, T], fp8, tag=f"kT8{t}", name=f"kT8{t}")
                   for t in range(2)]
            # Vb: [p, kjp 8, kj 2, pr 4, u 2, 96]: 64 v-dims + ones col at
            # 64 + zero pad to 96 (dual-fp8 ldweights needs cols % 32 == 0,
            # >= 64; psum rows 65:96 are dead)
            VW = 96
            Vbh = persist.tile([P, 8, 2, NPAIR, 2, VW], fp8, tag="Vbh",
                               name="Vbh")
            # diag-chunk AV runs in bf16 (pt quantization error bites only
            # concentrated near-diagonal attention rows): [p, kc, pr, u, 65]
            Vbb = persist.tile([P, 16, NPAIR, 2, HD + 1], bf16, tag="Vbb",
                               name="Vbb")
            mask01 = persist.tile([P, KC], bf16, tag="mask01", name="mask01")
            one = nc.const_aps.tensor(1.0, (P, 1))

            # ---- initial DMAs ----
            x8ts = {0: x8pool.tile([P, 4, 2, SPAN], u8, tag="x8t", name="x8t0")}
            x8ls = {0: xlpool.tile([P, 4, 2, SPAN], u8, tag="x8l", name="x8l0")}
            nc.sync.dma_start(wq8[:], wq8_d)
            nc.sync.dma_start(x8ts[0][:], x8_d[:, :, :, 0:SPAN])
            nc.sync.dma_start(wk8[:], wk8_d)
            nc.sync.dma_start(wv8[:, :, :, 0, :], wv8h_d)
            nc.sync.dma_start(x8ls[0][:], x8l_d[:, :, :, 0:SPAN])
            nc.sync.dma_start(wv8[:, :, :, 1, :], wv8l_d)
            nc.sync.dma_start(wot[:], woh_d)


            # mask01[p, f] = 1 if p <= f else 0 (post-exp diag pt mask)
            nc.gpsimd.memset(mask01[:], 1.0)
            nc.gpsimd.affine_select(
                out=mask01[:], in_=mask01[:],
                compare_op=mybir.AluOpType.is_ge, fill=0.0,
                base=0, channel_multiplier=-1, pattern=[[1, KC]],
            )
            # Vbh pad cols [65:96] must be zero (read by every off-diag
            # AV); data cols are always written before first read
            nslots = 8 * 2 * NPAIR * 2
            nc.gpsimd.memset(
                Vbh[:].rearrange("p a b c d e -> p (a b c d) e")
                [:, :, HD + 1:], 0.0)
            nc.vector.tensor_copy(
                Vbh[:].rearrange("p a b c d e -> p (a b c d) e")
                [:, :, HD:HD + 1],
                one.to_broadcast((P, nslots, 1)))
            nc.vector.tensor_copy(
                Vbb[:].rearrange("p a c d e -> p (a c d) e")
                [:, :, HD:HD + 1],
                one.to_broadcast((P, 16 * NPAIR * 2, 1)))

            qts = {}    # (sp, t) -> [P, 2, SPAN] fp8 tile
            ctxs = {}   # (sp, pr) -> [P, SPAN] f16 tile

            # ---------- emission helpers ----------
            def proj_qk(w8, pr_half, h, sp, x8t, scale, isq, on_act=False):
                t = pr_half

                def emit():
                    ps = psT.tile([P, SPAN], f32, tag="tr", name="psqk")
                    for dcp in range(4):
                        nc.tensor.matmul(
                            ps[:], w8[:, dcp, :, t, h, :].bitcast(fp8),
                            x8t[:, dcp, :, :].bitcast(fp8),
                            start=(dcp == 0), stop=(dcp == 3), perf_mode=DR)
                    if isq:
                        if (sp, t) not in qts:
                            qts[(sp, t)] = qpool.tile(
                                [P, 2, SPAN], fp8, tag=f"qT{t}",
                                name=f"qT{t}_{sp}")
                        dest = qts[(sp, t)][:, h, :]
                    else:
                        dest = kT8[t][:, h, sp * SPAN:(sp + 1) * SPAN]
                    if on_act:
                        # span-0 era: ACT is idle and DVE is the choke --
                        # the psum->fp8 scale-copy runs as activation Copy
                        nc.scalar.activation(
                            dest, ps[:], mybir.ActivationFunctionType.Copy,
                            scale=scale)
                    else:
                        nc.vector.tensor_scalar_mul(dest, ps[:], scale)
                return emit

            def proj_v(sp, tb, x8t, x8lt):
                # 3-pass fp8 DR: 16v = x_hi@w_hi + x_lo@w_hi + x_hi@w_lo
                # (w residual split keeps the value path at ~bf16 quality)
                def emit():
                    ps = psT.tile([P, SPAN], f32, tag="tr", name="psv")
                    ts = slice(tb * P, (tb + 1) * P)
                    passes = [(x8t, 0), (x8lt, 0), (x8t, 1)]
                    for pi, (xt, wl) in enumerate(passes):
                        for dcp in range(4):
                            nc.tensor.matmul(
                                ps[:], xt[:, dcp, :, ts].bitcast(fp8),
                                wv8[:, dcp, :, wl, :].bitcast(fp8),
                                start=(pi == 0 and dcp == 0),
                                stop=(pi == 2 and dcp == 3), perf_mode=DR)
                    kc = sp * 4 + tb
                    psv = ps[:].rearrange("p (pr u v) -> p pr u v",
                                          pr=NPAIR, v=HD)
                    nc.vector.tensor_scalar_mul(
                        Vbh[:, kc // 2, kc % 2, :, :, 0:HD], psv, 1.0 / 16)
                    nc.vector.tensor_scalar_mul(
                        Vbb[:, kc, :, :, 0:HD], psv, 1.0 / 16)
                return emit

            def q_groups(sp, x8t, on_act=False):
                return [proj_qk(wq8, t, h, sp, x8t, 1.0 / 32, True, on_act)
                        for t in range(2) for h in range(2)]

            def k_groups(sp, x8t, on_act=False):
                return [proj_qk(wk8, t, h, sp, x8t, 1.0 / 16, False, on_act)
                        for t in range(2) for h in range(2)]

            def v_groups(sp, x8t, x8lt):
                return [proj_v(sp, tb, x8t, x8lt) for tb in range(4)]

            ostages = {}  # tb -> [P, 2, SPAN] f16 stage (one DMA per tb)

            def outproj_group(sp, tb, os_, dma_q, ps_ap=None, act_copy=False):
                def emit():
                    if ps_ap is None:
                        ps = psT.tile([P, SPAN], f32, tag="tr", name="pso")[:]
                    else:
                        ps = ps_ap
                    for pc in range(NPAIR):
                        nc.tensor.matmul(
                            ps,
                            ctxs[(sp, pc)][:, (tb - sp * 4) * P:
                                           (tb - sp * 4 + 1) * P],
                            wot[:, pc, os_ * SPAN:(os_ + 1) * SPAN],
                            start=(pc == 0), stop=(pc == NPAIR - 1))
                    if os_ == 0:
                        ostages[tb] = stgpool.tile([P, 2, SPAN], f16,
                                                   tag="st", name="stage")
                    stage = ostages[tb]
                    if act_copy:
                        nc.scalar.activation(
                            stage[:, os_, :], ps,
                            mybir.ActivationFunctionType.Copy)
                    else:
                        nc.vector.tensor_copy(stage[:, os_, :], ps)
                    if os_ == 1:
                        # one merged 1024-col DMA per tb: fewer bus slots
                        dma_q.dma_start(
                            out_h.ap()[tb * P:(tb + 1) * P, :],
                            ostages.pop(tb)[:].rearrange("p a f -> p (a f)"))
                return emit

            def outproj_groups(sp):
                # all out-DMAs on SP: a Pool-queue DMACopy hold would block
                # partition_broadcast (evict chain)
                return [outproj_group(sp, tb, os_, nc.sync)
                        for tb in range(sp * 4, (sp + 1) * 4)
                        for os_ in range(2)]

            # ---------- attention for one span ----------
            deferred = []

            def attn_span(s, fillers, vgs=(), kgs=()):
                K = 4 * (s + 1)
                nslot = (K + 4) * NPAIR
                state = {"slot": 0, "fi": 0}

                def pace():
                    tgt = min(len(fillers),
                              len(fillers) * (state["slot"] + 1) // nslot)
                    while state["fi"] < tgt:
                        fillers[state["fi"]]()
                        state["fi"] += 1

                def tick():
                    state["slot"] += 1
                    pace()

                for pr in range(NPAIR):
                    t = pr // 2
                    ctxp = [psC.tile([96, SPAN], f32, tag="ctx",
                                     name=f"ctx{u}") for u in range(2)]
                    ct = cpool.tile([P, SPAN], f16, tag=f"cT{pr}",
                                    name=f"cT{pr}_{s}")
                    ctxs[(s, pr)] = ct
                    pts = {}
                    qt = qts[(s, t)]

                    avn = [0, 0]  # AV matmuls emitted per u (K total each)

                    def emit_qk(kj, pr=pr, t=t, pts=pts, qt=qt):
                        m = kj - 4 * s
                        c0 = 0 if m < 0 else m * KC
                        if m == 2:
                            # m2+m3 share one ss tile and one exp: m2 scores
                            # at cols [256:512], m3's 128 live cols parked in
                            # the unused [128:256]; exp'd together at m3
                            ss = psS.tile([P, 2, SPAN], f32, tag="psS",
                                          name="ss23")
                            pts[("ss23",)] = ss
                        elif m == 3:
                            ss = pts[("ss23",)]
                        else:
                            ss = psS.tile([P, 2, SPAN], f32, tag="psS",
                                          name="ss")
                        dst = slice(KC, 2 * KC) if m == 3 else slice(c0, SPAN)
                        for u in range(2):
                            b32 = 32 * (2 * (pr % 2) + u)
                            r = slice(b32, b32 + 32)
                            nc.tensor.matmul(
                                ss[:, u, dst],
                                kT8[t][r, :, kj * KC:(kj + 1) * KC],
                                qt[r, :, c0:],
                                start=True, stop=True, perf_mode=DR,
                                tile_position=(b32, 0))
                        if m < 0:
                            # off-diagonal: fp8 pt pair for DR AV
                            kjp, sl = kj // 2, kj % 2
                            if sl == 0:
                                pts[kjp] = ptpool.tile([P, 2, 2, SPAN], fp8,
                                                       tag="pt", name="pt")
                            nc.scalar.activation(pts[kjp][:, sl, :, :],
                                                 ss[:, :, :], Exp, scale=0.25)
                        elif m == 2:
                            # defer: exp'd together with m3
                            ptb = ptbpool.tile([P, 2, SPAN], bf16,
                                               tag="ptb", name="ptb")
                            pts[("d", kj)] = ptb
                            pts[("d", kj + 1)] = ptb
                        elif m == 3:
                            ptb = pts[("d", kj)]
                            del pts[("ss23",)]
                            nc.scalar.activation(ptb[:, :, KC:],
                                                 ss[:, :, KC:], Exp,
                                                 scale=0.25)
                            # both triangles ([128:256] = m3, [256:384] = m2)
                            # use the same 128x128 mask: one DVE mul
                            nc.vector.tensor_mul(
                                ptb[:, :, KC:3 * KC].rearrange(
                                    "p u (b f) -> p u b f", b=2),
                                ptb[:, :, KC:3 * KC].rearrange(
                                    "p u (b f) -> p u b f", b=2),
                                mask01[:].rearrange("p (u b f) -> p u b f",
                                                    u=1, b=1)
                                .to_broadcast((P, 2, 2, KC)))
                        else:
                            # diagonal m0/m1: own tile + exp + triangle mask
                            ptb = ptbpool.tile([P, 2, SPAN], bf16,
                                               tag="ptb", name="ptb")
                            pts[("d", kj)] = ptb
                            nc.scalar.activation(ptb[:, :, c0:],
                                                 ss[:, :, c0:], Exp,
                                                 scale=0.25)
                            nc.vector.tensor_mul(
                                ptb[:, :, c0:c0 + KC], ptb[:, :, c0:c0 + KC],
                                mask01[:].rearrange("p (u f) -> p u f", u=1)
                                .to_broadcast((P, 2, KC)))

                    NU = 2 * s + 4  # AV matmuls per u-chain

                    def emit_av8(kjp, pr=pr, pts=pts, ctxp=ctxp, NU=NU):
                        pt = pts.pop(kjp)
                        for u in range(2):
                            nc.tensor.matmul(
                                ctxp[u][:],
                                Vbh[:, kjp, :, pr, u, :],
                                pt[:, :, u, :],
                                start=(avn[u] == 0),
                                stop=(avn[u] == NU - 1),
                                perf_mode=DR, skip_group_check=True)
                            avn[u] += 1

                    def emit_avd(kj, pr=pr, pts=pts, ctxp=ctxp, NU=NU):
                        ptb = pts.pop(("d", kj))
                        m = kj - 4 * s
                        c0 = m * KC
                        # m3's pt lives at stored cols [128:256] (positional
                        # matmul mapping: out offset is independent of rhs)
                        rsl = slice(KC, 2 * KC) if m == 3 else slice(c0, SPAN)
                        for u in range(2):
                            nc.tensor.matmul(
                                ctxp[u][0:HD + 1, c0:],
                                Vbb[:, kj, pr, u, :],
                                ptb[:, u, rsl],
                                start=(avn[u] == 0), stop=(avn[u] == NU - 1),
                                skip_group_check=True)
                            avn[u] += 1

                    rzt = [rzpool.tile([P, 2, SPAN], bf16, tag="rz",
                                       name="rz"),
                           rzpool.tile([P, 2, SPAN], bf16, tag="rz",
                                       name="rzbc")]

                    def fin(u, cs=slice(0, SPAN), ctxp=ctxp, ct=ct,
                            rzt=rzt):
                        rz, rzbc = rzt

                        def run():
                            # 1/Z row: psum row 64 -> sbuf row 0
                            # (cross-partition DVE write, hw-verified)
                            with nc.allow_low_precision(
                                    reason="1/Z bf16: 0.4% on ctx"):
                                nc.vector.reciprocal(
                                    rz[0:1, u, cs], ctxp[u][64:65, cs])
                            nc.gpsimd.partition_broadcast(
                                rzbc[:, u, cs], rz[0:1, u, cs])
                            nc.vector.tensor_mul(
                                ct[u * HD:(u + 1) * HD, cs],
                                ctxp[u][0:HD, cs], rzbc[0:HD, u, cs])
                        return run

                    def evict(pr=pr, s=s):
                        if s == 3 and pr == 3:
                            # halves [0:256] were evicted early (see units);
                            # only [256:512] rides the tail critical chain
                            deferred.append(fin(0, slice(256, SPAN)))
                            deferred.append(fin(1, slice(256, SPAN)))
                        else:
                            deferred.append(fin(0))
                            deferred.append(fin(1))

                    # AV work units: (ready_kj, emit_fn); off-diag kjp ready
                    # at its odd kj, diag kj ready at kj
                    units = []
                    for kj2 in range(K):
                        if kj2 < 4 * s:
                            if kj2 % 2 == 1:
                                units.append((kj2, kj2 // 2, emit_av8))
                        else:
                            units.append((kj2, kj2, emit_avd))
                            if s == 3 and pr == 3 and kj2 == 4 * s + 1:
                                # ctx cols [0:256] are final after avd(m1):
                                # evict them under the remaining diag AVs
                                units.append(
                                    (kj2, None,
                                     lambda _, f0=fin(0, slice(0, 256)),
                                     f1=fin(1, slice(0, 256)):
                                     (f0(), f1())))
                    # JIT placement: V(s) spread across pr0's whole kj
                    # range (consumers: pr0 diag AV); K(s) t0-groups in pr0,
                    # t1-groups in pr1 (consumers: pr0/pr2 diag QK)
                    jit = {}
                    if pr == 0:
                        for i in range(len(vgs)):
                            jit.setdefault(max(i * K // 4, i), []).append(
                                vgs[i])
                        # K(s) t0 keys must precede the diag QK at kj=4s
                        # (jit runs after emit_qk in the same iteration)
                        for i in (0, 1) if kgs else ():
                            jit.setdefault(min(1 + i * (K // 3 + 1),
                                               4 * s - 1), []).append(kgs[i])
                    elif pr == 1:
                        for i in (2, 3) if kgs else ():
                            jit.setdefault(1 + (i - 2) * (K // 3 + 1),
                                           []).append(kgs[i])
                    ui = [0]
                    for kj in range(K):
                        emit_qk(kj)
                        for g in jit.get(kj, ()):
                            g()
                        if deferred:
                            deferred.pop(0)()
                        tick()
                        while (ui[0] < len(units)
                               and units[ui[0]][0] + LAG <= kj):
                            units[ui[0]][2](units[ui[0]][1])
                            ui[0] += 1
                    while ui[0] < len(units):
                        if deferred:
                            deferred.pop(0)()
                        tick()
                        units[ui[0]][2](units[ui[0]][1])
                        ui[0] += 1
                    evict()
                while state["fi"] < len(fillers):
                    fillers[state["fi"]]()
                    state["fi"] += 1

            # ---------- main schedule ----------
            # V(s) and K(s) run inside span s itself (their first consumers
            # are span s's own diag AV / diag QK) -- keeps earlier spans off
            # the PE; only Q(s+1) must finish during span s
            # pr0's attention needs (q,k) t0 groups first: interleave
            qg0 = q_groups(0, x8ts[0], on_act=True)
            kg0 = k_groups(0, x8ts[0], on_act=True)
            for i in range(4):
                qg0[i]()
                kg0[i]()
            for s in range(NSPAN):
                vgs = v_groups(s, x8ts[s], x8ls[s])
                kgs = k_groups(s, x8ts[s]) if s >= 1 else ()
                fillers = []
                if s + 1 < NSPAN:
                    x8t = x8pool.tile([P, 4, 2, SPAN], u8, tag="x8t",
                                      name=f"x8t{s + 1}")
                    x8lt = xlpool.tile([P, 4, 2, SPAN], u8, tag="x8l",
                                       name=f"x8l{s + 1}")
                    x8ts[s + 1], x8ls[s + 1] = x8t, x8lt
                    sl = slice((s + 1) * SPAN, (s + 2) * SPAN)
                    nc.sync.dma_start(x8t[:], x8_d[:, :, :, sl])
                    nc.sync.dma_start(x8lt[:], x8l_d[:, :, :, sl])
                    fillers += q_groups(s + 1, x8t, on_act=(s == 0))
                if s == 2:
                    fillers += outproj_groups(0)
                elif s == 3:
                    fillers += outproj_groups(1) + outproj_groups(2)
                attn_span(s, fillers, vgs, kgs)
            while deferred:
                deferred.pop(0)()
            # tail outproj(3): attention is done, so psS's 4 banks are free
            # scratch -- 6 parallel psums let all 32 matmuls run back-to-back
            # (no psT-rotation stalls, PE stays ramped); stage copies split
            # DVE/ACT (both idle at the tail)
            sc = [psT.tile([P, SPAN], f32, tag="tr", name=f"ost{i}")[:]
                  for i in range(2)]
            for i in range(2):
                t = psS.tile([P, 2, SPAN], f32, tag="psS", name=f"osc{i}")
                sc += [t[:, 0, :], t[:, 1, :]]
            for i in range(2):
                sc.append(psC.tile([P, SPAN], f32, tag="ctx",
                                   name=f"oscc{i}")[:])
            tails = [(tb, os_) for tb in range(12, 16) for os_ in range(2)]
            # pc-interleaved: pc0..2 matmuls only need the earlier pairs'
            # ctx and run under the last evict chain; only pc3 gates on it
            for pc in range(NPAIR):
                for gi, (tb, os_) in enumerate(tails):
                    nc.tensor.matmul(
                        sc[gi],
                        ctxs[(3, pc)][:, (tb - 12) * P:(tb - 11) * P],
                        wot[:, pc, os_ * SPAN:(os_ + 1) * SPAN],
                        start=(pc == 0), stop=(pc == NPAIR - 1),
                        skip_group_check=True)
            tstg = {}
            for gi, (tb, os_) in enumerate(tails):
                if os_ == 0:
                    tstg[tb] = stgpool.tile([P, 2, SPAN], f16, tag="st",
                                            name="tstage")
                if gi % 2 == 1:
                    nc.scalar.activation(
                        tstg[tb][:, os_, :], sc[gi],
                        mybir.ActivationFunctionType.Copy)
                else:
                    nc.vector.tensor_copy(tstg[tb][:, os_, :], sc[gi])
                if os_ == 1:
                    nc.sync.dma_start(
                        out_h.ap()[tb * P:(tb + 1) * P, :],
                        tstg.pop(tb)[:].rearrange("p a f -> p (a f)"))

    nc.compile()
    return nc


def get_nc():
    if "nc" not in _CACHE:
        _CACHE["nc"] = _build()
    return _CACHE["nc"]


def _perm512():
    perm = np.empty(512, np.int64)
    i = 0
    for t in range(2):
        for h in range(2):
            for beta in range(4):
                pr = 2 * t + beta // 2
                u = beta % 2
                for dd in range(32):
                    perm[i] = pr * 128 + u * 64 + h * 32 + dd
                    i += 1
    return perm


def kernel(x, Wq, Wk, Wv, Wo, bo):
    import ml_dtypes
    from concourse import bass_utils

    e4 = ml_dtypes.float8_e4m3

    x = np.asarray(x, dtype=np.float32)
    Wq, Wk, Wv = (np.asarray(w, dtype=np.float32) for w in (Wq, Wk, Wv))
    Wo = np.asarray(Wo, dtype=np.float32)
    bo = np.asarray(bo, dtype=np.float32)
    perm = _perm512()

    in_maps = []
    for c in range(NCORES):
        b, g = c // 2, c % 2
        gsl = slice(g * 512, (g + 1) * 512)
        xT = np.ascontiguousarray(x[b].T)
        x8 = xT.astype(e4)
        wv16 = np.ascontiguousarray(16.0 * Wv[gsl].T)
        wv8h = wv16.astype(e4)
        in_maps.append({
            "x8": x8.view(np.uint8),
            # residual of the e4m3 x quantization (V-proj pass 2)
            "x8l": (xT - x8.astype(np.float32)).astype(e4).view(np.uint8),
            # w quantized x16 to dodge e4m3 subnormals; unscaled on-chip
            "wq8": np.ascontiguousarray((16.0 * Wq[gsl].T)[:, perm])
            .astype(e4).view(np.uint8),
            "wk8": np.ascontiguousarray((16.0 * Wk[gsl].T)[:, perm])
            .astype(e4).view(np.uint8),
            "wv8h": wv8h.view(np.uint8),
            "wv8l": (wv16 - wv8h.astype(np.float32)).astype(e4)
            .view(np.uint8),
            "woh": np.ascontiguousarray(Wo[:, gsl].T).astype(np.float16),
        })

    nc = get_nc()
    res = bass_utils.run_bass_kernel_spmd(nc, in_maps,
                                          core_ids=list(range(NCORES)))
    parts = [res.results[c]["out"].astype(np.float32) for c in range(NCORES)]
    out = np.stack([parts[2 * b] + parts[2 * b + 1] + bo for b in range(B)])
    return out.astype(np.float32)


# revision 70
# speedup vs baseline: 1.0992x; 1.0209x over previous
"""Multi-head causal attention (B=4, T=2048, D=1024, H=16) on 8 Trainium2 cores.

Sharding: core c = (b, g) with b = c//2 (batch), g = c%2 (head-group of 8 heads).
Each core: Q/K/V projections for its 8 heads (column-parallel), causal attention,
row-parallel partial output projection. Host sums the g=0/g=1 partials + bias.

v4 design (fp8 DoubleRow + streaming AV; cost model: matmul = out-free-rows x
0.4167ns x cpr, fp8e4 DoubleRow cpr=0.5 contracting 2x128/instr; ACT exp at
0.8333ns/free-elem = ~147us busy is the floor):
  - Q/K proj: fp8 DR, x8 moving [128,2,512], w8 stationary [128,2,128] in 4
    col-groups (t=pair-half, h=dim-half) so psum partitions land as
    (beta=2*(pr%2)+u, dd) blocks of 32 -> qT8/kT8 [32-blocks, 2h, T] fp8 for
    2x32-contraction DR score matmuls. w quantized x16 (e4m3 subnormal
    dodge), unscaled in the DVE psum->fp8 copy; score scale 1/8 folded as
    qT8 = e4m3(q/2) + exp(scale=0.25). Sub-128 DR needs explicit
    tile_position (base-96 slices break base_partition inference).
  - scores: fp8 DR per (pr,u,kj): out ss [128 keys, 2u, 512-c0] psum.
  - exp: ACT psum -> sbuf, one instr per kj covering both heads. Off-diag
    chunks -> fp8 pt pairs [128, 2kj, 2u, 512]; diag chunks -> bf16 pt
    (pt-quant error only bites concentrated near-diagonal rows) with
    post-exp mask01 multiply on DVE (off ACT's critical path). Diag m2+m3
    share one ss tile (m3's 128 live cols parked at [128:256]) and one
    exp + one mask-mul; the AV remaps positionally (out col offset is
    independent of rhs offset) -- same elements, 16 fewer ACT instrs.
  - AV streaming into ctx psum [96, 512] per (pr,u), accumulated across the
    span: off-diag = single-fp8 DR (lhsT = Vbh [128, 2kj, 96]: 64 v-dims +
    ones col at 64 -> Z at psum row 64 + zero pad -- dual-fp8 ldweights
    needs cols%32==0, >=64); diag = bf16 non-DR (lhsT = Vbb [128, 65]).
    Diffuse off-diag rows average out single-fp8 V error; vonly ~1e-3.
  - evict per (pr,u): DVE reciprocal of Z row -> bf16 rz at partition 0
    (cross-partition psum read, hw-verified), gpsimd partition_broadcast ->
    rzbc sbuf, DVE mul ctx*rzbc -> ctx_sb f16 [128=(u,vd), 512] (u1 written
    cross-partition to rows 64:128). Evicts deferred into the next pair's /
    span's kj loop so the PE never stalls on them.
  - outproj: f16, 4x128-contraction per [128q, 512od] psum group; DVE f16
    stage -> DMA (psum DMA is forbidden); host sums g-partials + bias f32.
  - V proj: 3-pass fp8 DR (16v = xh@wh + xl@wh + xh@wl; w x16-scaled
    hi/lo split keeps the value path ~bf16 quality; psum/16 on copy).
  - schedule: only Q(s+1) is paced as filler through attention(s); K(s) and
    V(s) are JIT-emitted inside span s itself (first consumers are its own
    diag QK/AV), spread across pr0/pr1 kj iterations AFTER each emit_qk --
    K keys must land strictly before kj=4s or the diag QK reads stale kT
    (caused a NaN once). outproj(0)@s2, outproj(1,2)@s3, outproj(3) tail.
    ALL DMAs on the SP queue (the DMA bus serializes at ~1.45us/512KB; a
    DMACopy SEQ-hold on ACT blocks exp dispatch, on Pool it blocks
    partition_broadcast). Early proj psum->fp8 copies (span-0 era) run as
    ACT activation-Copy w/ scale (ACT idle there, DVE is the choke) -- but
    ONLY ops whose inputs are ready early (ACT is in-order; late-input
    copies head-of-line block exps). Tail outproj pc-interleaved over 8
    scratch psums; per-tb merged 1024-col out DMAs.
Measured: 188348 ns (TimelineSim), rel err 1.21e-2 (gate 2e-2). Baseline
v3 was 240443 ns. ACT exp busy 146.9us (73%), PE 129us, DVE 118us.
The last pair's evict is split: cols [0:256] normalize under the
remaining diag AVs (final after avd(m1); later AVs only touch higher
cols), so only [256:512] rides the tail chain. Remaining idle: ~5us
startup (DMA-bus chain), ~6.5us span-0/1 boundary (ss-pool bufs=2
serializes the exp restart; psum full at 8 banks), tail drain ~8us.
"""

import os
import sys

try:
    import concourse.bass  # noqa: F401
except ImportError:  # pragma: no cover
    sys.path.insert(0, "/opt/trn_rl_repo")

import numpy as np

B, T, D = 4, 2048, 1024
H, HD = 16, 64
NCORES = 8
NPAIR = 4
NSPAN = 4
SPAN = 512
KC = 128
P = 128
LAG = 3

_CACHE = {}


def _build():
    import concourse.bacc as bacc
    import concourse.mybir as mybir
    import concourse.tile as tile

    f32 = mybir.dt.float32
    f16 = mybir.dt.float16
    bf16 = mybir.dt.bfloat16
    fp8 = mybir.dt.float8e4
    u8 = mybir.dt.uint8
    Exp = mybir.ActivationFunctionType.Exp

    DR = mybir.MatmulPerfMode.DoubleRow

    nc = bacc.Bacc("TRN2", target_bir_lowering=False, debug=False,
                   num_devices=NCORES)

    x8_h = nc.dram_tensor("x8", (D, T), u8, kind="ExternalInput")
    x8l_h = nc.dram_tensor("x8l", (D, T), u8, kind="ExternalInput")
    wq8_h = nc.dram_tensor("wq8", (D, 512), u8, kind="ExternalInput")
    wk8_h = nc.dram_tensor("wk8", (D, 512), u8, kind="ExternalInput")
    wv8h_h = nc.dram_tensor("wv8h", (D, 512), u8, kind="ExternalInput")
    wv8l_h = nc.dram_tensor("wv8l", (D, 512), u8, kind="ExternalInput")
    woh_h = nc.dram_tensor("woh", (512, D), f16, kind="ExternalInput")
    out_h = nc.dram_tensor("out", (T, D), f16, kind="ExternalOutput")

    # x8: D = (dcp 4, k 2, p 128); per span slice on T
    x8_d = x8_h.ap().rearrange("(dcp k p) t -> p dcp k t", p=P, k=2)
    x8l_d = x8l_h.ap().rearrange("(dcp k p) t -> p dcp k t", p=P, k=2)
    wq8_d = wq8_h.ap().rearrange("(dcp k p) (t h m) -> p dcp k t h m",
                                 p=P, k=2, t=2, h=2)
    wk8_d = wk8_h.ap().rearrange("(dcp k p) (t h m) -> p dcp k t h m",
                                 p=P, k=2, t=2, h=2)
    wv8h_d = wv8h_h.ap().rearrange("(dcp k p) f -> p dcp k f", p=P, k=2)
    wv8l_d = wv8l_h.ap().rearrange("(dcp k p) f -> p dcp k f", p=P, k=2)
    woh_d = woh_h.ap().rearrange("(pc p) f -> p pc f", p=P)

    with tile.TileContext(nc) as tc:
        with (
            tc.tile_pool(name="persist", bufs=1) as persist,
            tc.tile_pool(name="x8p", bufs=2) as x8pool,
            tc.tile_pool(name="xlp", bufs=2) as xlpool,
            tc.tile_pool(name="qp", bufs=4) as qpool,
            tc.tile_pool(name="ptp", bufs=6) as ptpool,
            tc.tile_pool(name="ptb", bufs=5) as ptbpool,
            tc.tile_pool(name="rzp", bufs=8) as rzpool,
            tc.tile_pool(name="cp", bufs=8) as cpool,
            tc.tile_pool(name="stg", bufs=6) as stgpool,
            tc.tile_pool(name="psS", bufs=2, space="PSUM") as psS,
            tc.tile_pool(name="psC", bufs=2, space="PSUM") as psC,
            tc.tile_pool(name="psT", bufs=2, space="PSUM") as psT,
        ):
            wq8 = persist.tile([P, 4, 2, 2, 2, P], u8, tag="wq8", name="wq8")
            wk8 = persist.tile([P, 4, 2, 2, 2, P], u8, tag="wk8", name="wk8")
            wv8 = persist.tile([P, 4, 2, 2, 512], u8, tag="wv8", name="wv8")
            wot = persist.tile([P, 4, D], f16, tag="wot", name="wot")
            kT8 = [persist.tile([P, 2, T], fp8, tag=f"kT8{t}", name=f"kT8{t}")
                   for t in range(2)]
            # Vb: [p, kjp 8, kj 2, pr 4, u 2, 96]: 64 v-dims + ones col at
            # 64 + zero pad to 96 (dual-fp8 ldweights needs cols % 32 == 0,
            # >= 64; psum rows 65:96 are dead)
            VW = 96
            Vbh = persist.tile([P, 8, 2, NPAIR, 2, VW], fp8, tag="Vbh",
                               name="Vbh")
            # diag-chunk AV runs in bf16 (pt quantization error bites only
            # concentrated near-diagonal attention rows): [p, kc, pr, u, 65]
            Vbb = persist.tile([P, 16, NPAIR, 2, HD + 1], bf16, tag="Vbb",
                               name="Vbb")
            mask01 = persist.tile([P, KC], bf16, tag="mask01", name="mask01")
            one = nc.const_aps.tensor(1.0, (P, 1))

            # ---- initial DMAs ----
            x8ts = {0: x8pool.tile([P, 4, 2, SPAN], u8, tag="x8t", name="x8t0")}
            x8ls = {0: xlpool.tile([P, 4, 2, SPAN], u8, tag="x8l", name="x8l0")}
            nc.sync.dma_start(wq8[:], wq8_d)
            nc.sync.dma_start(x8ts[0][:], x8_d[:, :, :, 0:SPAN])
            nc.sync.dma_start(wk8[:], wk8_d)
            nc.sync.dma_start(wv8[:, :, :, 0, :], wv8h_d)
            nc.sync.dma_start(x8ls[0][:], x8l_d[:, :, :, 0:SPAN])
            nc.sync.dma_start(wv8[:, :, :, 1, :], wv8l_d)
            nc.sync.dma_start(wot[:], woh_d)


            # mask01[p, f] = 1 if p <= f else 0 (post-exp diag pt mask)
            nc.gpsimd.memset(mask01[:], 1.0)
            nc.gpsimd.affine_select(
                out=mask01[:], in_=mask01[:],
                compare_op=mybir.AluOpType.is_ge, fill=0.0,
                base=0, channel_multiplier=-1, pattern=[[1, KC]],
            )
            # Vbh pad cols [65:96] must be zero (read by every off-diag
            # AV); data cols are always written before first read
            nslots = 8 * 2 * NPAIR * 2
            nc.gpsimd.memset(
                Vbh[:].rearrange("p a b c d e -> p (a b c d) e")
                [:, :, HD + 1:], 0.0)
            nc.vector.tensor_copy(
                Vbh[:].rearrange("p a b c d e -> p (a b c d) e")
                [:, :, HD:HD + 1],
                one.to_broadcast((P, nslots, 1)))
            nc.vector.tensor_copy(
                Vbb[:].rearrange("p a c d e -> p (a c d) e")
                [:, :, HD:HD + 1],
                one.to_broadcast((P, 16 * NPAIR * 2, 1)))

            qts = {}    # (sp, t) -> [P, 2, SPAN] fp8 tile
            ctxs = {}   # (sp, pr) -> [P, SPAN] f16 tile

            # ---------- emission helpers ----------
            def proj_qk(w8, pr_half, h, sp, x8t, scale, isq, on_act=False):
                t = pr_half

                def emit():
                    ps = psT.tile([P, SPAN], f32, tag="tr", name="psqk")
                    for dcp in range(4):
                        nc.tensor.matmul(
                            ps[:], w8[:, dcp, :, t, h, :].bitcast(fp8),
                            x8t[:, dcp, :, :].bitcast(fp8),
                            start=(dcp == 0), stop=(dcp == 3), perf_mode=DR)
                    if isq:
                        if (sp, t) not in qts:
                            qts[(sp, t)] = qpool.tile(
                                [P, 2, SPAN], fp8, tag=f"qT{t}",
                                name=f"qT{t}_{sp}")
                        dest = qts[(sp, t)][:, h, :]
                    else:
                        dest = kT8[t][:, h, sp * SPAN:(sp + 1) * SPAN]
                    if on_act:
                        # span-0 era: ACT is idle and DVE is the choke --
                        # the psum->fp8 scale-copy runs as activation Copy
                        nc.scalar.activation(
                            dest, ps[:], mybir.ActivationFunctionType.Copy,
                            scale=scale)
                    else:
                        nc.vector.tensor_scalar_mul(dest, ps[:], scale)
                return emit

            def proj_v(sp, tb, x8t, x8lt):
                # 3-pass fp8 DR: 16v = x_hi@w_hi + x_lo@w_hi + x_hi@w_lo
                # (w residual split keeps the value path at ~bf16 quality)
                def emit():
                    ps = psT.tile([P, SPAN], f32, tag="tr", name="psv")
                    ts = slice(tb * P, (tb + 1) * P)
                    passes = [(x8t, 0), (x8lt, 0), (x8t, 1)]
                    for pi, (xt, wl) in enumerate(passes):
                        for dcp in range(4):
                            nc.tensor.matmul(
                                ps[:], xt[:, dcp, :, ts].bitcast(fp8),
                                wv8[:, dcp, :, wl, :].bitcast(fp8),
                                start=(pi == 0 and dcp == 0),
                                stop=(pi == 2 and dcp == 3), perf_mode=DR)
                    kc = sp * 4 + tb
                    psv = ps[:].rearrange("p (pr u v) -> p pr u v",
                                          pr=NPAIR, v=HD)
                    nc.vector.tensor_scalar_mul(
                        Vbh[:, kc // 2, kc % 2, :, :, 0:HD], psv, 1.0 / 16)
                    nc.vector.tensor_scalar_mul(
                        Vbb[:, kc, :, :, 0:HD], psv, 1.0 / 16)
                return emit

            def q_groups(sp, x8t, on_act=False):
                return [proj_qk(wq8, t, h, sp, x8t, 1.0 / 32, True, on_act)
                        for t in range(2) for h in range(2)]

            def k_groups(sp, x8t, on_act=False):
                return [proj_qk(wk8, t, h, sp, x8t, 1.0 / 16, False, on_act)
                        for t in range(2) for h in range(2)]

            def v_groups(sp, x8t, x8lt):
                return [proj_v(sp, tb, x8t, x8lt) for tb in range(4)]

            ostages = {}  # tb -> [P, 2, SPAN] f16 stage (one DMA per tb)

            def outproj_group(sp, tb, os_, dma_q, ps_ap=None, act_copy=False):
                def emit():
                    if ps_ap is None:
                        ps = psT.tile([P, SPAN], f32, tag="tr", name="pso")[:]
                    else:
                        ps = ps_ap
                    for pc in range(NPAIR):
                        nc.tensor.matmul(
                            ps,
                            ctxs[(sp, pc)][:, (tb - sp * 4) * P:
                                           (tb - sp * 4 + 1) * P],
                            wot[:, pc, os_ * SPAN:(os_ + 1) * SPAN],
                            start=(pc == 0), stop=(pc == NPAIR - 1))
                    if os_ == 0:
                        ostages[tb] = stgpool.tile([P, 2, SPAN], f16,
                                                   tag="st", name="stage")
                    stage = ostages[tb]
                    if act_copy:
                        nc.scalar.activation(
                            stage[:, os_, :], ps,
                            mybir.ActivationFunctionType.Copy)
                    else:
                        nc.vector.tensor_copy(stage[:, os_, :], ps)
                    if os_ == 1:
                        # one merged 1024-col DMA per tb: fewer bus slots
                        dma_q.dma_start(
                            out_h.ap()[tb * P:(tb + 1) * P, :],
                            ostages.pop(tb)[:].rearrange("p a f -> p (a f)"))
                return emit

            def outproj_groups(sp):
                # all out-DMAs on SP: a Pool-queue DMACopy hold would block
                # partition_broadcast (evict chain)
                return [outproj_group(sp, tb, os_, nc.sync)
                        for tb in range(sp * 4, (sp + 1) * 4)
                        for os_ in range(2)]

            # ---------- attention for one span ----------
            deferred = []

            def attn_span(s, fillers, vgs=(), kgs=()):
                K = 4 * (s + 1)
                nslot = (K + 4) * NPAIR
                state = {"slot": 0, "fi": 0}

                def pace():
                    tgt = min(len(fillers),
                              len(fillers) * (state["slot"] + 1) // nslot)
                    while state["fi"] < tgt:
                        fillers[state["fi"]]()
                        state["fi"] += 1

                def tick():
                    state["slot"] += 1
                    pace()

                for pr in range(NPAIR):
                    t = pr // 2
                    ctxp = [psC.tile([96, SPAN], f32, tag="ctx",
                                     name=f"ctx{u}") for u in range(2)]
                    ct = cpool.tile([P, SPAN], f16, tag=f"cT{pr}",
                                    name=f"cT{pr}_{s}")
                    ctxs[(s, pr)] = ct
                    pts = {}
                    qt = qts[(s, t)]

                    avn = [0, 0]  # AV matmuls emitted per u (K total each)

                    def emit_qk(kj, pr=pr, t=t, pts=pts, qt=qt):
                        m = kj - 4 * s
                        c0 = 0 if m < 0 else m * KC
                        if m == 2:
                            # m2+m3 share one ss tile and one exp: m2 scores
                            # at cols [256:512], m3's 128 live cols parked in
                            # the unused [128:256]; exp'd together at m3
                            ss = psS.tile([P, 2, SPAN], f32, tag="psS",
                                          name="ss23")
                            pts[("ss23",)] = ss
                        elif m == 3:
                            ss = pts[("ss23",)]
                        else:
                            ss = psS.tile([P, 2, SPAN], f32, tag="psS",
                                          name="ss")
                        dst = slice(KC, 2 * KC) if m == 3 else slice(c0, SPAN)
                        for u in range(2):
                            b32 = 32 * (2 * (pr % 2) + u)
                            r = slice(b32, b32 + 32)
                            nc.tensor.matmul(
                                ss[:, u, dst],
                                kT8[t][r, :, kj * KC:(kj + 1) * KC],
                                qt[r, :, c0:],
                                start=True, stop=True, perf_mode=DR,
                                tile_position=(b32, 0))
                        if m < 0:
                            # off-diagonal: fp8 pt pair for DR AV
                            kjp, sl = kj // 2, kj % 2
                            if sl == 0:
                                pts[kjp] = ptpool.tile([P, 2, 2, SPAN], fp8,
                                                       tag="pt", name="pt")
                            nc.scalar.activation(pts[kjp][:, sl, :, :],
                                                 ss[:, :, :], Exp, scale=0.25)
                        elif m == 2:
                            # defer: exp'd together with m3
                            ptb = ptbpool.tile([P, 2, SPAN], bf16,
                                               tag="ptb", name="ptb")
                            pts[("d", kj)] = ptb
                            pts[("d", kj + 1)] = ptb
                        elif m == 3:
                            ptb = pts[("d", kj)]
                            del pts[("ss23",)]
                            nc.scalar.activation(ptb[:, :, KC:],
                                                 ss[:, :, KC:], Exp,
                                                 scale=0.25)
                            # both triangles ([128:256] = m3, [256:384] = m2)
                            # use the same 128x128 mask: one DVE mul
                            nc.vector.tensor_mul(
                                ptb[:, :, KC:3 * KC].rearrange(
                                    "p u (b f) -> p u b f", b=2),
                                ptb[:, :, KC:3 * KC].rearrange(
                                    "p u (b f) -> p u b f", b=2),
                                mask01[:].rearrange("p (u b f) -> p u b f",
                                                    u=1, b=1)
                                .to_broadcast((P, 2, 2, KC)))
                        else:
                            # diagonal m0/m1: own tile + exp + triangle mask
                            ptb = ptbpool.tile([P, 2, SPAN], bf16,
                                               tag="ptb", name="ptb")
                            pts[("d", kj)] = ptb
                            nc.scalar.activation(ptb[:, :, c0:],
                                                 ss[:, :, c0:], Exp,
                                                 scale=0.25)
                            nc.vector.tensor_mul(
                                ptb[:, :, c0:c0 + KC], ptb[:, :, c0:c0 + KC],
                                mask01[:].rearrange("p (u f) -> p u f", u=1)
                                .to_broadcast((P, 2, KC)))

                    NU = 2 * s + 4  # AV matmuls per u-chain

                    def emit_av8(kjp, pr=pr, pts=pts, ctxp=ctxp, NU=NU):
                        pt = pts.pop(kjp)
                        for u in range(2):
                            nc.tensor.matmul(
                                ctxp[u][:],
                                Vbh[:, kjp, :, pr, u, :],
                                pt[:, :, u, :],
                                start=(avn[u] == 0),
                                stop=(avn[u] == NU - 1),
                                perf_mode=DR, skip_group_check=True)
                            avn[u] += 1

                    def emit_avd(kj, pr=pr, pts=pts, ctxp=ctxp, NU=NU):
                        ptb = pts.pop(("d", kj))
                        m = kj - 4 * s
                        c0 = m * KC
                        # m3's pt lives at stored cols [128:256] (positional
                        # matmul mapping: out offset is independent of rhs)
                        rsl = slice(KC, 2 * KC) if m == 3 else slice(c0, SPAN)
                        for u in range(2):
                            nc.tensor.matmul(
                                ctxp[u][0:HD + 1, c0:],
                                Vbb[:, kj, pr, u, :],
                                ptb[:, u, rsl],
                                start=(avn[u] == 0), stop=(avn[u] == NU - 1),
                                skip_group_check=True)
                            avn[u] += 1

                    rzt = [rzpool.tile([P, 2, SPAN], bf16, tag="rz",
                                       name="rz"),
                           rzpool.tile([P, 2, SPAN], bf16, tag="rz",
                                       name="rzbc")]

                    def fin(u, cs=slice(0, SPAN), ctxp=ctxp, ct=ct,
                            rzt=rzt):
                        rz, rzbc = rzt

                        def run():
                            # 1/Z row: psum row 64 -> sbuf row 0
                            # (cross-partition DVE write, hw-verified)
                            with nc.allow_low_precision(
                                    reason="1/Z bf16: 0.4% on ctx"):
                                nc.vector.reciprocal(
                                    rz[0:1, u, cs], ctxp[u][64:65, cs])
                            nc.gpsimd.partition_broadcast(
                                rzbc[:, u, cs], rz[0:1, u, cs])
                            nc.vector.tensor_mul(
                                ct[u * HD:(u + 1) * HD, cs],
                                ctxp[u][0:HD, cs], rzbc[0:HD, u, cs])
                        return run

                    def evict(pr=pr, s=s):
                        # [0:256] was evicted early (see units)
                        deferred.append(fin(0, slice(256, SPAN)))
                        deferred.append(fin(1, slice(256, SPAN)))

                    # AV work units: (ready_kj, emit_fn); off-diag kjp ready
                    # at its odd kj, diag kj ready at kj
                    units = []
                    for kj2 in range(K):
                        if kj2 < 4 * s:
                            if kj2 % 2 == 1:
                                units.append((kj2, kj2 // 2, emit_av8))
                        else:
                            units.append((kj2, kj2, emit_avd))
                            if kj2 == 4 * s + 1:
                                # ctx cols [0:256] are final after avd(m1):
                                # evict them under the remaining diag AVs
                                # (frees the psC tile ~1us earlier for the
                                # next pair's AV chain)
                                units.append(
                                    (kj2, None,
                                     lambda _, f0=fin(0, slice(0, 256)),
                                     f1=fin(1, slice(0, 256)):
                                     (f0(), f1())))
                    # JIT placement: V(s) spread across pr0's whole kj
                    # range (consumers: pr0 diag AV); K(s) t0-groups in pr0,
                    # t1-groups in pr1 (consumers: pr0/pr2 diag QK)
                    jit = {}
                    if pr == 0:
                        for i in range(len(vgs)):
                            jit.setdefault(max(i * K // 4, i), []).append(
                                vgs[i])
                        # K(s) t0 keys must precede the diag QK at kj=4s
                        # (jit runs after emit_qk in the same iteration)
                        for i in (0, 1) if kgs else ():
                            jit.setdefault(min(1 + i * (K // 3 + 1),
                                               4 * s - 1), []).append(kgs[i])
                    elif pr == 1:
                        for i in (2, 3) if kgs else ():
                            jit.setdefault(1 + (i - 2) * (K // 3 + 1),
                                           []).append(kgs[i])
                    ui = [0]
                    for kj in range(K):
                        emit_qk(kj)
                        for g in jit.get(kj, ()):
                            g()
                        if deferred:
                            deferred.pop(0)()
                        tick()
                        while (ui[0] < len(units)
                               and units[ui[0]][0] + LAG <= kj):
                            units[ui[0]][2](units[ui[0]][1])
                            ui[0] += 1
                    while ui[0] < len(units):
                        if deferred:
                            deferred.pop(0)()
                        tick()
                        units[ui[0]][2](units[ui[0]][1])
                        ui[0] += 1
                    evict()
                while state["fi"] < len(fillers):
                    fillers[state["fi"]]()
                    state["fi"] += 1

            # ---------- main schedule ----------
            # V(s) and K(s) run inside span s itself (their first consumers
            # are span s's own diag AV / diag QK) -- keeps earlier spans off
            # the PE; only Q(s+1) must finish during span s
            # pr0's attention needs (q,k) t0 groups first: interleave
            qg0 = q_groups(0, x8ts[0], on_act=True)
            kg0 = k_groups(0, x8ts[0], on_act=True)
            for i in range(4):
                qg0[i]()
                kg0[i]()
            for s in range(NSPAN):
                vgs = v_groups(s, x8ts[s], x8ls[s])
                kgs = k_groups(s, x8ts[s]) if s >= 1 else ()
                fillers = []
                if s + 1 < NSPAN:
                    x8t = x8pool.tile([P, 4, 2, SPAN], u8, tag="x8t",
                                      name=f"x8t{s + 1}")
                    x8lt = xlpool.tile([P, 4, 2, SPAN], u8, tag="x8l",
                                       name=f"x8l{s + 1}")
                    x8ts[s + 1], x8ls[s + 1] = x8t, x8lt
                    sl = slice((s + 1) * SPAN, (s + 2) * SPAN)
                    nc.sync.dma_start(x8t[:], x8_d[:, :, :, sl])
                    nc.sync.dma_start(x8lt[:], x8l_d[:, :, :, sl])
                    fillers += q_groups(s + 1, x8t, on_act=(s == 0))
                if s == 2:
                    fillers += outproj_groups(0)
                elif s == 3:
                    fillers += outproj_groups(1) + outproj_groups(2)
                attn_span(s, fillers, vgs, kgs)
            while deferred:
                deferred.pop(0)()
            # tail outproj(3): attention is done, so psS's 4 banks are free
            # scratch -- 6 parallel psums let all 32 matmuls run back-to-back
            # (no psT-rotation stalls, PE stays ramped); stage copies split
            # DVE/ACT (both idle at the tail)
            sc = [psT.tile([P, SPAN], f32, tag="tr", name=f"ost{i}")[:]
                  for i in range(2)]
            for i in range(2):
                t = psS.tile([P, 2, SPAN], f32, tag="psS", name=f"osc{i}")
                sc += [t[:, 0, :], t[:, 1, :]]
            for i in range(2):
                sc.append(psC.tile([P, SPAN], f32, tag="ctx",
                                   name=f"oscc{i}")[:])
            tails = [(tb, os_) for tb in range(12, 16) for os_ in range(2)]
            # pc-interleaved: pc0..2 matmuls only need the earlier pairs'
            # ctx and run under the last evict chain; only pc3 gates on it
            for pc in range(NPAIR):
                for gi, (tb, os_) in enumerate(tails):
                    nc.tensor.matmul(
                        sc[gi],
                        ctxs[(3, pc)][:, (tb - 12) * P:(tb - 11) * P],
                        wot[:, pc, os_ * SPAN:(os_ + 1) * SPAN],
                        start=(pc == 0), stop=(pc == NPAIR - 1),
                        skip_group_check=True)
            tstg = {}
            for gi, (tb, os_) in enumerate(tails):
                if os_ == 0:
                    tstg[tb] = stgpool.tile([P, 2, SPAN], f16, tag="st",
                                            name="tstage")
                if gi % 2 == 1:
                    nc.scalar.activation(
                        tstg[tb][:, os_, :], sc[gi],
                        mybir.ActivationFunctionType.Copy)
                else:
                    nc.vector.tensor_copy(tstg[tb][:, os_, :], sc[gi])
                if os_ == 1:
                    nc.sync.dma_start(
                        out_h.ap()[tb * P:(tb + 1) * P, :],
                        tstg.pop(tb)[:].rearrange("p a f -> p (a f)"))

    nc.compile()
    return nc


def get_nc():
    if "nc" not in _CACHE:
        _CACHE["nc"] = _build()
    return _CACHE["nc"]


def _perm512():
    perm = np.empty(512, np.int64)
    i = 0
    for t in range(2):
        for h in range(2):
            for beta in range(4):
                pr = 2 * t + beta // 2
                u = beta % 2
                for dd in range(32):
                    perm[i] = pr * 128 + u * 64 + h * 32 + dd
                    i += 1
    return perm


def kernel(x, Wq, Wk, Wv, Wo, bo):
    import ml_dtypes
    from concourse import bass_utils

    e4 = ml_dtypes.float8_e4m3

    x = np.asarray(x, dtype=np.float32)
    Wq, Wk, Wv = (np.asarray(w, dtype=np.float32) for w in (Wq, Wk, Wv))
    Wo = np.asarray(Wo, dtype=np.float32)
    bo = np.asarray(bo, dtype=np.float32)
    perm = _perm512()

    in_maps = []
    for c in range(NCORES):
        b, g = c // 2, c % 2
        gsl = slice(g * 512, (g + 1) * 512)
        xT = np.ascontiguousarray(x[b].T)
        x8 = xT.astype(e4)
        wv16 = np.ascontiguousarray(16.0 * Wv[gsl].T)
        wv8h = wv16.astype(e4)
        in_maps.append({
            "x8": x8.view(np.uint8),
            # residual of the e4m3 x quantization (V-proj pass 2)
            "x8l": (xT - x8.astype(np.float32)).astype(e4).view(np.uint8),
            # w quantized x16 to dodge e4m3 subnormals; unscaled on-chip
            "wq8": np.ascontiguousarray((16.0 * Wq[gsl].T)[:, perm])
            .astype(e4).view(np.uint8),
            "wk8": np.ascontiguousarray((16.0 * Wk[gsl].T)[:, perm])
            .astype(e4).view(np.uint8),
            "wv8h": wv8h.view(np.uint8),
            "wv8l": (wv16 - wv8h.astype(np.float32)).astype(e4)
            .view(np.uint8),
            "woh": np.ascontiguousarray(Wo[:, gsl].T).astype(np.float16),
        })

    nc = get_nc()
    res = bass_utils.run_bass_kernel_spmd(nc, in_maps,
                                          core_ids=list(range(NCORES)))
    parts = [res.results[c]["out"].astype(np.float32) for c in range(NCORES)]
    out = np.stack([parts[2 * b] + parts[2 * b + 1] + bo for b in range(B)])
    return out.astype(np.float32)


# revision 76
# speedup vs baseline: 1.0997x; 1.0004x over previous
"""Multi-head causal attention (B=4, T=2048, D=1024, H=16) on 8 Trainium2 cores.

Sharding: core c = (b, g) with b = c//2 (batch), g = c%2 (head-group of 8 heads).
Each core: Q/K/V projections for its 8 heads (column-parallel), causal attention,
row-parallel partial output projection. Host sums the g=0/g=1 partials + bias.

v4 design (fp8 DoubleRow + streaming AV; cost model: matmul = out-free-rows x
0.4167ns x cpr, fp8e4 DoubleRow cpr=0.5 contracting 2x128/instr; ACT exp at
0.8333ns/free-elem = ~147us busy is the floor):
  - Q/K proj: fp8 DR, x8 moving [128,2,512], w8 stationary [128,2,128] in 4
    col-groups (t=pair-half, h=dim-half) so psum partitions land as
    (beta=2*(pr%2)+u, dd) blocks of 32 -> qT8/kT8 [32-blocks, 2h, T] fp8 for
    2x32-contraction DR score matmuls. w quantized x16 (e4m3 subnormal
    dodge), unscaled in the DVE psum->fp8 copy; score scale 1/8 folded as
    qT8 = e4m3(q/2) + exp(scale=0.25). Sub-128 DR needs explicit
    tile_position (base-96 slices break base_partition inference).
  - scores: fp8 DR per (pr,u,kj): out ss [128 keys, 2u, 512-c0] psum.
  - exp: ACT psum -> sbuf, one instr per kj covering both heads. Off-diag
    chunks -> fp8 pt pairs [128, 2kj, 2u, 512]; diag chunks -> bf16 pt
    (pt-quant error only bites concentrated near-diagonal rows) with
    post-exp mask01 multiply on DVE (off ACT's critical path). Diag m2+m3
    share one ss tile (m3's 128 live cols parked at [128:256]) and one
    exp + one mask-mul; the AV remaps positionally (out col offset is
    independent of rhs offset) -- same elements, 16 fewer ACT instrs.
  - AV streaming into ctx psum [96, 512] per (pr,u), accumulated across the
    span: off-diag = single-fp8 DR (lhsT = Vbh [128, 2kj, 96]: 64 v-dims +
    ones col at 64 -> Z at psum row 64 + zero pad -- dual-fp8 ldweights
    needs cols%32==0, >=64); diag = bf16 non-DR (lhsT = Vbb [128, 65]).
    Diffuse off-diag rows average out single-fp8 V error; vonly ~1e-3.
  - evict per (pr,u): DVE reciprocal of Z row -> bf16 rz at partition 0
    (cross-partition psum read, hw-verified), gpsimd partition_broadcast ->
    rzbc sbuf, DVE mul ctx*rzbc -> ctx_sb f16 [128=(u,vd), 512] (u1 written
    cross-partition to rows 64:128). Evicts deferred into the next pair's /
    span's kj loop so the PE never stalls on them.
  - outproj: f16, 4x128-contraction per [128q, 512od] psum group; DVE f16
    stage -> DMA (psum DMA is forbidden); host sums g-partials + bias f32.
  - V proj: 3-pass fp8 DR (16v = xh@wh + xl@wh + xh@wl; w x16-scaled
    hi/lo split keeps the value path ~bf16 quality; psum/16 on copy).
  - schedule: only Q(s+1) is paced as filler through attention(s); K(s) and
    V(s) are JIT-emitted inside span s itself (first consumers are its own
    diag QK/AV), spread across pr0/pr1 kj iterations AFTER each emit_qk --
    K keys must land strictly before kj=4s or the diag QK reads stale kT
    (caused a NaN once). outproj(0)@s2, outproj(1,2)@s3, outproj(3) tail.
    ALL DMAs on the SP queue (the DMA bus serializes at ~1.45us/512KB; a
    DMACopy SEQ-hold on ACT blocks exp dispatch, on Pool it blocks
    partition_broadcast). Early proj psum->fp8 copies (span-0 era) run as
    ACT activation-Copy w/ scale (ACT idle there, DVE is the choke) -- but
    ONLY ops whose inputs are ready early (ACT is in-order; late-input
    copies head-of-line block exps). Tail outproj pc-interleaved over 8
    scratch psums; per-tb merged 1024-col out DMAs.
Measured: 184497 ns (TimelineSim), rel err 1.21e-2 (gate 2e-2). Baseline
v3 was 240443 ns. ACT exp busy 146.9us (73%), PE 129us, DVE 118us.
Every pair's evict is split: cols [0:256] normalize under the remaining
diag AVs (final after avd(m1); m2/m3 only touch higher cols incl. their
Z-row adds), so only [256:512] rides each pair's freeing chain -- the
psC rotation unblocks the next pair's AV ~1us earlier, and the tail
chain shortens. 3-way splits regress (per-chain overhead). Remaining
idle: ~5us startup (DMA-bus chain), ~6us span-0/1 boundary (ss-pool
bufs=2 serializes the exp restart; psum full at 8 banks), tail drain.
"""

import os
import sys

try:
    import concourse.bass  # noqa: F401
except ImportError:  # pragma: no cover
    sys.path.insert(0, "/opt/trn_rl_repo")

import numpy as np

B, T, D = 4, 2048, 1024
H, HD = 16, 64
NCORES = 8
NPAIR = 4
NSPAN = 4
SPAN = 512
KC = 128
P = 128
LAG = 3

_CACHE = {}


def _build():
    import concourse.bacc as bacc
    import concourse.mybir as mybir
    import concourse.tile as tile

    f32 = mybir.dt.float32
    f16 = mybir.dt.float16
    bf16 = mybir.dt.bfloat16
    fp8 = mybir.dt.float8e4
    u8 = mybir.dt.uint8
    Exp = mybir.ActivationFunctionType.Exp

    DR = mybir.MatmulPerfMode.DoubleRow

    nc = bacc.Bacc("TRN2", target_bir_lowering=False, debug=False,
                   num_devices=NCORES)

    x8_h = nc.dram_tensor("x8", (D, T), u8, kind="ExternalInput")
    x8l_h = nc.dram_tensor("x8l", (D, T), u8, kind="ExternalInput")
    wq8_h = nc.dram_tensor("wq8", (D, 512), u8, kind="ExternalInput")
    wk8_h = nc.dram_tensor("wk8", (D, 512), u8, kind="ExternalInput")
    wv8h_h = nc.dram_tensor("wv8h", (D, 512), u8, kind="ExternalInput")
    wv8l_h = nc.dram_tensor("wv8l", (D, 512), u8, kind="ExternalInput")
    woh_h = nc.dram_tensor("woh", (512, D), f16, kind="ExternalInput")
    out_h = nc.dram_tensor("out", (T, D), f16, kind="ExternalOutput")

    # x8: D = (dcp 4, k 2, p 128); per span slice on T
    x8_d = x8_h.ap().rearrange("(dcp k p) t -> p dcp k t", p=P, k=2)
    x8l_d = x8l_h.ap().rearrange("(dcp k p) t -> p dcp k t", p=P, k=2)
    wq8_d = wq8_h.ap().rearrange("(dcp k p) (t h m) -> p dcp k t h m",
                                 p=P, k=2, t=2, h=2)
    wk8_d = wk8_h.ap().rearrange("(dcp k p) (t h m) -> p dcp k t h m",
                                 p=P, k=2, t=2, h=2)
    wv8h_d = wv8h_h.ap().rearrange("(dcp k p) f -> p dcp k f", p=P, k=2)
    wv8l_d = wv8l_h.ap().rearrange("(dcp k p) f -> p dcp k f", p=P, k=2)
    woh_d = woh_h.ap().rearrange("(pc p) f -> p pc f", p=P)

    with tile.TileContext(nc) as tc:
        with (
            tc.tile_pool(name="persist", bufs=1) as persist,
            tc.tile_pool(name="x8p", bufs=2) as x8pool,
            tc.tile_pool(name="xlp", bufs=2) as xlpool,
            tc.tile_pool(name="qp", bufs=4) as qpool,
            tc.tile_pool(name="ptp", bufs=6) as ptpool,
            tc.tile_pool(name="ptb", bufs=6) as ptbpool,
            tc.tile_pool(name="rzp", bufs=12) as rzpool,
            tc.tile_pool(name="cp", bufs=8) as cpool,
            tc.tile_pool(name="stg", bufs=6) as stgpool,
            tc.tile_pool(name="psS", bufs=2, space="PSUM") as psS,
            tc.tile_pool(name="psC", bufs=2, space="PSUM") as psC,
            tc.tile_pool(name="psT", bufs=2, space="PSUM") as psT,
        ):
            wq8 = persist.tile([P, 4, 2, 2, 2, P], u8, tag="wq8", name="wq8")
            wk8 = persist.tile([P, 4, 2, 2, 2, P], u8, tag="wk8", name="wk8")
            wv8 = persist.tile([P, 4, 2, 2, 512], u8, tag="wv8", name="wv8")
            wot = persist.tile([P, 4, D], f16, tag="wot", name="wot")
            kT8 = [persist.tile([P, 2, T], fp8, tag=f"kT8{t}", name=f"kT8{t}")
                   for t in range(2)]
            # Vb: [p, kjp 8, kj 2, pr 4, u 2, 96]: 64 v-dims + ones col at
            # 64 + zero pad to 96 (dual-fp8 ldweights needs cols % 32 == 0,
            # >= 64; psum rows 65:96 are dead)
            VW = 96
            Vbh = persist.tile([P, 8, 2, NPAIR, 2, VW], fp8, tag="Vbh",
                               name="Vbh")
            # diag-chunk AV runs in bf16 (pt quantization error bites only
            # concentrated near-diagonal attention rows): [p, kc, pr, u, 65]
            Vbb = persist.tile([P, 16, NPAIR, 2, HD + 1], bf16, tag="Vbb",
                               name="Vbb")
            mask01 = persist.tile([P, KC], bf16, tag="mask01", name="mask01")
            one = nc.const_aps.tensor(1.0, (P, 1))

            # ---- initial DMAs ----
            x8ts = {0: x8pool.tile([P, 4, 2, SPAN], u8, tag="x8t", name="x8t0")}
            x8ls = {0: xlpool.tile([P, 4, 2, SPAN], u8, tag="x8l", name="x8l0")}
            nc.sync.dma_start(wq8[:], wq8_d)
            nc.sync.dma_start(x8ts[0][:], x8_d[:, :, :, 0:SPAN])
            nc.sync.dma_start(wk8[:], wk8_d)
            nc.sync.dma_start(wv8[:, :, :, 0, :], wv8h_d)
            nc.sync.dma_start(x8ls[0][:], x8l_d[:, :, :, 0:SPAN])
            nc.sync.dma_start(wv8[:, :, :, 1, :], wv8l_d)
            nc.sync.dma_start(wot[:], woh_d)


            # mask01[p, f] = 1 if p <= f else 0 (post-exp diag pt mask)
            nc.gpsimd.memset(mask01[:], 1.0)
            nc.gpsimd.affine_select(
                out=mask01[:], in_=mask01[:],
                compare_op=mybir.AluOpType.is_ge, fill=0.0,
                base=0, channel_multiplier=-1, pattern=[[1, KC]],
            )
            # Vbh pad cols [65:96] must be zero (read by every off-diag
            # AV); data cols are always written before first read
            nslots = 8 * 2 * NPAIR * 2
            nc.gpsimd.memset(
                Vbh[:].rearrange("p a b c d e -> p (a b c d) e")
                [:, :, HD + 1:], 0.0)
            nc.vector.tensor_copy(
                Vbh[:].rearrange("p a b c d e -> p (a b c d) e")
                [:, :, HD:HD + 1],
                one.to_broadcast((P, nslots, 1)))
            nc.vector.tensor_copy(
                Vbb[:].rearrange("p a c d e -> p (a c d) e")
                [:, :, HD:HD + 1],
                one.to_broadcast((P, 16 * NPAIR * 2, 1)))

            qts = {}    # (sp, t) -> [P, 2, SPAN] fp8 tile
            ctxs = {}   # (sp, pr) -> [P, SPAN] f16 tile

            # ---------- emission helpers ----------
            def proj_qk(w8, pr_half, h, sp, x8t, scale, isq, on_act=False):
                t = pr_half

                def emit():
                    ps = psT.tile([P, SPAN], f32, tag="tr", name="psqk")
                    for dcp in range(4):
                        nc.tensor.matmul(
                            ps[:], w8[:, dcp, :, t, h, :].bitcast(fp8),
                            x8t[:, dcp, :, :].bitcast(fp8),
                            start=(dcp == 0), stop=(dcp == 3), perf_mode=DR)
                    if isq:
                        if (sp, t) not in qts:
                            qts[(sp, t)] = qpool.tile(
                                [P, 2, SPAN], fp8, tag=f"qT{t}",
                                name=f"qT{t}_{sp}")
                        dest = qts[(sp, t)][:, h, :]
                    else:
                        dest = kT8[t][:, h, sp * SPAN:(sp + 1) * SPAN]
                    if on_act:
                        # span-0 era: ACT is idle and DVE is the choke --
                        # the psum->fp8 scale-copy runs as activation Copy
                        nc.scalar.activation(
                            dest, ps[:], mybir.ActivationFunctionType.Copy,
                            scale=scale)
                    else:
                        nc.vector.tensor_scalar_mul(dest, ps[:], scale)
                return emit

            def proj_v(sp, tb, x8t, x8lt):
                # 3-pass fp8 DR: 16v = x_hi@w_hi + x_lo@w_hi + x_hi@w_lo
                # (w residual split keeps the value path at ~bf16 quality)
                def emit():
                    ps = psT.tile([P, SPAN], f32, tag="tr", name="psv")
                    ts = slice(tb * P, (tb + 1) * P)
                    passes = [(x8t, 0), (x8lt, 0), (x8t, 1)]
                    for pi, (xt, wl) in enumerate(passes):
                        for dcp in range(4):
                            nc.tensor.matmul(
                                ps[:], xt[:, dcp, :, ts].bitcast(fp8),
                                wv8[:, dcp, :, wl, :].bitcast(fp8),
                                start=(pi == 0 and dcp == 0),
                                stop=(pi == 2 and dcp == 3), perf_mode=DR)
                    kc = sp * 4 + tb
                    psv = ps[:].rearrange("p (pr u v) -> p pr u v",
                                          pr=NPAIR, v=HD)
                    nc.vector.tensor_scalar_mul(
                        Vbh[:, kc // 2, kc % 2, :, :, 0:HD], psv, 1.0 / 16)
                    nc.vector.tensor_scalar_mul(
                        Vbb[:, kc, :, :, 0:HD], psv, 1.0 / 16)
                return emit

            def q_groups(sp, x8t, on_act=False):
                return [proj_qk(wq8, t, h, sp, x8t, 1.0 / 32, True, on_act)
                        for t in range(2) for h in range(2)]

            def k_groups(sp, x8t, on_act=False):
                return [proj_qk(wk8, t, h, sp, x8t, 1.0 / 16, False, on_act)
                        for t in range(2) for h in range(2)]

            def v_groups(sp, x8t, x8lt):
                return [proj_v(sp, tb, x8t, x8lt) for tb in range(4)]

            ostages = {}  # tb -> [P, 2, SPAN] f16 stage (one DMA per tb)

            def outproj_group(sp, tb, os_, dma_q, ps_ap=None, act_copy=False):
                def emit():
                    if ps_ap is None:
                        ps = psT.tile([P, SPAN], f32, tag="tr", name="pso")[:]
                    else:
                        ps = ps_ap
                    for pc in range(NPAIR):
                        nc.tensor.matmul(
                            ps,
                            ctxs[(sp, pc)][:, (tb - sp * 4) * P:
                                           (tb - sp * 4 + 1) * P],
                            wot[:, pc, os_ * SPAN:(os_ + 1) * SPAN],
                            start=(pc == 0), stop=(pc == NPAIR - 1))
                    if os_ == 0:
                        ostages[tb] = stgpool.tile([P, 2, SPAN], f16,
                                                   tag="st", name="stage")
                    stage = ostages[tb]
                    if act_copy:
                        nc.scalar.activation(
                            stage[:, os_, :], ps,
                            mybir.ActivationFunctionType.Copy)
                    else:
                        nc.vector.tensor_copy(stage[:, os_, :], ps)
                    if os_ == 1:
                        # one merged 1024-col DMA per tb: fewer bus slots
                        dma_q.dma_start(
                            out_h.ap()[tb * P:(tb + 1) * P, :],
                            ostages.pop(tb)[:].rearrange("p a f -> p (a f)"))
                return emit

            def outproj_groups(sp):
                # all out-DMAs on SP: a Pool-queue DMACopy hold would block
                # partition_broadcast (evict chain)
                return [outproj_group(sp, tb, os_, nc.sync)
                        for tb in range(sp * 4, (sp + 1) * 4)
                        for os_ in range(2)]

            # ---------- attention for one span ----------
            deferred = []

            def attn_span(s, fillers, vgs=(), kgs=()):
                K = 4 * (s + 1)
                nslot = (K + 4) * NPAIR
                state = {"slot": 0, "fi": 0}

                def pace():
                    tgt = min(len(fillers),
                              len(fillers) * (state["slot"] + 1) // nslot)
                    while state["fi"] < tgt:
                        fillers[state["fi"]]()
                        state["fi"] += 1

                def tick():
                    state["slot"] += 1
                    pace()

                for pr in range(NPAIR):
                    t = pr // 2
                    ctxp = [psC.tile([96, SPAN], f32, tag="ctx",
                                     name=f"ctx{u}") for u in range(2)]
                    ct = cpool.tile([P, SPAN], f16, tag=f"cT{pr}",
                                    name=f"cT{pr}_{s}")
                    ctxs[(s, pr)] = ct
                    pts = {}
                    qt = qts[(s, t)]

                    avn = [0, 0]  # AV matmuls emitted per u (K total each)

                    def emit_qk(kj, pr=pr, t=t, pts=pts, qt=qt):
                        m = kj - 4 * s
                        c0 = 0 if m < 0 else m * KC
                        if m == 2:
                            # m2+m3 share one ss tile and one exp: m2 scores
                            # at cols [256:512], m3's 128 live cols parked in
                            # the unused [128:256]; exp'd together at m3
                            ss = psS.tile([P, 2, SPAN], f32, tag="psS",
                                          name="ss23")
                            pts[("ss23",)] = ss
                        elif m == 3:
                            ss = pts[("ss23",)]
                        else:
                            ss = psS.tile([P, 2, SPAN], f32, tag="psS",
                                          name="ss")
                        dst = slice(KC, 2 * KC) if m == 3 else slice(c0, SPAN)
                        for u in range(2):
                            b32 = 32 * (2 * (pr % 2) + u)
                            r = slice(b32, b32 + 32)
                            nc.tensor.matmul(
                                ss[:, u, dst],
                                kT8[t][r, :, kj * KC:(kj + 1) * KC],
                                qt[r, :, c0:],
                                start=True, stop=True, perf_mode=DR,
                                tile_position=(b32, 0))
                        if m < 0:
                            # off-diagonal: fp8 pt pair for DR AV
                            kjp, sl = kj // 2, kj % 2
                            if sl == 0:
                                pts[kjp] = ptpool.tile([P, 2, 2, SPAN], fp8,
                                                       tag="pt", name="pt")
                            nc.scalar.activation(pts[kjp][:, sl, :, :],
                                                 ss[:, :, :], Exp, scale=0.25)
                        elif m == 2:
                            # defer: exp'd together with m3
                            ptb = ptbpool.tile([P, 2, SPAN], bf16,
                                               tag="ptb", name="ptb")
                            pts[("d", kj)] = ptb
                            pts[("d", kj + 1)] = ptb
                        elif m == 3:
                            ptb = pts[("d", kj)]
                            del pts[("ss23",)]
                            nc.scalar.activation(ptb[:, :, KC:],
                                                 ss[:, :, KC:], Exp,
                                                 scale=0.25)
                            # both triangles ([128:256] = m3, [256:384] = m2)
                            # use the same 128x128 mask: one DVE mul
                            nc.vector.tensor_mul(
                                ptb[:, :, KC:3 * KC].rearrange(
                                    "p u (b f) -> p u b f", b=2),
                                ptb[:, :, KC:3 * KC].rearrange(
                                    "p u (b f) -> p u b f", b=2),
                                mask01[:].rearrange("p (u b f) -> p u b f",
                                                    u=1, b=1)
                                .to_broadcast((P, 2, 2, KC)))
                        else:
                            # diagonal m0/m1: own tile + exp + triangle mask
                            ptb = ptbpool.tile([P, 2, SPAN], bf16,
                                               tag="ptb", name="ptb")
                            pts[("d", kj)] = ptb
                            nc.scalar.activation(ptb[:, :, c0:],
                                                 ss[:, :, c0:], Exp,
                                                 scale=0.25)
                            nc.vector.tensor_mul(
                                ptb[:, :, c0:c0 + KC], ptb[:, :, c0:c0 + KC],
                                mask01[:].rearrange("p (u f) -> p u f", u=1)
                                .to_broadcast((P, 2, KC)))

                    NU = 2 * s + 4  # AV matmuls per u-chain

                    def emit_av8(kjp, pr=pr, pts=pts, ctxp=ctxp, NU=NU):
                        pt = pts.pop(kjp)
                        for u in range(2):
                            nc.tensor.matmul(
                                ctxp[u][:],
                                Vbh[:, kjp, :, pr, u, :],
                                pt[:, :, u, :],
                                start=(avn[u] == 0),
                                stop=(avn[u] == NU - 1),
                                perf_mode=DR, skip_group_check=True)
                            avn[u] += 1

                    def emit_avd(kj, pr=pr, pts=pts, ctxp=ctxp, NU=NU):
                        ptb = pts.pop(("d", kj))
                        m = kj - 4 * s
                        c0 = m * KC
                        # m3's pt lives at stored cols [128:256] (positional
                        # matmul mapping: out offset is independent of rhs)
                        rsl = slice(KC, 2 * KC) if m == 3 else slice(c0, SPAN)
                        for u in range(2):
                            nc.tensor.matmul(
                                ctxp[u][0:HD + 1, c0:],
                                Vbb[:, kj, pr, u, :],
                                ptb[:, u, rsl],
                                start=(avn[u] == 0), stop=(avn[u] == NU - 1),
                                skip_group_check=True)
                            avn[u] += 1

                    rzt = [rzpool.tile([P, 2, SPAN], bf16, tag="rz",
                                       name="rz"),
                           rzpool.tile([P, 2, SPAN], bf16, tag="rz",
                                       name="rzbc")]

                    def fin(u, cs=slice(0, SPAN), ctxp=ctxp, ct=ct,
                            rzt=rzt):
                        rz, rzbc = rzt

                        def run():
                            # 1/Z row: psum row 64 -> sbuf row 0
                            # (cross-partition DVE write, hw-verified)
                            with nc.allow_low_precision(
                                    reason="1/Z bf16: 0.4% on ctx"):
                                nc.vector.reciprocal(
                                    rz[0:1, u, cs], ctxp[u][64:65, cs])
                            nc.gpsimd.partition_broadcast(
                                rzbc[:, u, cs], rz[0:1, u, cs])
                            nc.vector.tensor_mul(
                                ct[u * HD:(u + 1) * HD, cs],
                                ctxp[u][0:HD, cs], rzbc[0:HD, u, cs])
                        return run

                    def evict(pr=pr, s=s):
                        # [0:256] was evicted early (see units)
                        deferred.append(fin(0, slice(256, SPAN)))
                        deferred.append(fin(1, slice(256, SPAN)))

                    # AV work units: (ready_kj, emit_fn); off-diag kjp ready
                    # at its odd kj, diag kj ready at kj
                    units = []
                    for kj2 in range(K):
                        if kj2 < 4 * s:
                            if kj2 % 2 == 1:
                                units.append((kj2, kj2 // 2, emit_av8))
                        else:
                            units.append((kj2, kj2, emit_avd))
                            if kj2 == 4 * s + 1:
                                # ctx cols [0:256] are final after avd(m1):
                                # evict them under the remaining diag AVs
                                # (frees the psC tile ~1us earlier for the
                                # next pair's AV chain)
                                units.append(
                                    (kj2, None,
                                     lambda _, f0=fin(0, slice(0, 256)),
                                     f1=fin(1, slice(0, 256)):
                                     (f0(), f1())))
                    # JIT placement: V(s) spread across pr0's whole kj
                    # range (consumers: pr0 diag AV); K(s) t0-groups in pr0,
                    # t1-groups in pr1 (consumers: pr0/pr2 diag QK)
                    jit = {}
                    if pr == 0:
                        for i in range(len(vgs)):
                            jit.setdefault(max(i * K // 4, i), []).append(
                                vgs[i])
                        # K(s) t0 keys must precede the diag QK at kj=4s
                        # (jit runs after emit_qk in the same iteration)
                        for i in (0, 1) if kgs else ():
                            jit.setdefault(min(1 + i * (K // 3 + 1),
                                               4 * s - 1), []).append(kgs[i])
                    elif pr == 1:
                        for i in (2, 3) if kgs else ():
                            jit.setdefault(1 + (i - 2) * (K // 3 + 1),
                                           []).append(kgs[i])
                    ui = [0]
                    for kj in range(K):
                        emit_qk(kj)
                        for g in jit.get(kj, ()):
                            g()
                        if deferred:
                            deferred.pop(0)()
                        tick()
                        while (ui[0] < len(units)
                               and units[ui[0]][0] + LAG <= kj):
                            units[ui[0]][2](units[ui[0]][1])
                            ui[0] += 1
                    while ui[0] < len(units):
                        if deferred:
                            deferred.pop(0)()
                        tick()
                        units[ui[0]][2](units[ui[0]][1])
                        ui[0] += 1
                    evict()
                while state["fi"] < len(fillers):
                    fillers[state["fi"]]()
                    state["fi"] += 1

            # ---------- main schedule ----------
            # V(s) and K(s) run inside span s itself (their first consumers
            # are span s's own diag AV / diag QK) -- keeps earlier spans off
            # the PE; only Q(s+1) must finish during span s
            # pr0's attention needs (q,k) t0 groups first: interleave
            qg0 = q_groups(0, x8ts[0], on_act=True)
            kg0 = k_groups(0, x8ts[0], on_act=True)
            for i in range(4):
                qg0[i]()
                kg0[i]()
            for s in range(NSPAN):
                vgs = v_groups(s, x8ts[s], x8ls[s])
                kgs = k_groups(s, x8ts[s]) if s >= 1 else ()
                fillers = []
                if s + 1 < NSPAN:
                    x8t = x8pool.tile([P, 4, 2, SPAN], u8, tag="x8t",
                                      name=f"x8t{s + 1}")
                    x8lt = xlpool.tile([P, 4, 2, SPAN], u8, tag="x8l",
                                       name=f"x8l{s + 1}")
                    x8ts[s + 1], x8ls[s + 1] = x8t, x8lt
                    sl = slice((s + 1) * SPAN, (s + 2) * SPAN)
                    nc.sync.dma_start(x8t[:], x8_d[:, :, :, sl])
                    nc.sync.dma_start(x8lt[:], x8l_d[:, :, :, sl])
                    fillers += q_groups(s + 1, x8t, on_act=(s == 0))
                if s == 2:
                    fillers += outproj_groups(0)
                elif s == 3:
                    fillers += outproj_groups(1) + outproj_groups(2)
                attn_span(s, fillers, vgs, kgs)
            while deferred:
                deferred.pop(0)()
            # tail outproj(3): attention is done, so psS's 4 banks are free
            # scratch -- 6 parallel psums let all 32 matmuls run back-to-back
            # (no psT-rotation stalls, PE stays ramped); stage copies split
            # DVE/ACT (both idle at the tail)
            sc = [psT.tile([P, SPAN], f32, tag="tr", name=f"ost{i}")[:]
                  for i in range(2)]
            for i in range(2):
                t = psS.tile([P, 2, SPAN], f32, tag="psS", name=f"osc{i}")
                sc += [t[:, 0, :], t[:, 1, :]]
            for i in range(2):
                sc.append(psC.tile([P, SPAN], f32, tag="ctx",
                                   name=f"oscc{i}")[:])
            tails = [(tb, os_) for tb in range(12, 16) for os_ in range(2)]
            # pc-interleaved: pc0..2 matmuls only need the earlier pairs'
            # ctx and run under the last evict chain; only pc3 gates on it
            for pc in range(NPAIR):
                for gi, (tb, os_) in enumerate(tails):
                    nc.tensor.matmul(
                        sc[gi],
                        ctxs[(3, pc)][:, (tb - 12) * P:(tb - 11) * P],
                        wot[:, pc, os_ * SPAN:(os_ + 1) * SPAN],
                        start=(pc == 0), stop=(pc == NPAIR - 1),
                        skip_group_check=True)
            tstg = {}
            for gi, (tb, os_) in enumerate(tails):
                if os_ == 0:
                    tstg[tb] = stgpool.tile([P, 2, SPAN], f16, tag="st",
                                            name="tstage")
                if gi % 2 == 1:
                    nc.scalar.activation(
                        tstg[tb][:, os_, :], sc[gi],
                        mybir.ActivationFunctionType.Copy)
                else:
                    nc.vector.tensor_copy(tstg[tb][:, os_, :], sc[gi])
                if os_ == 1:
                    nc.sync.dma_start(
                        out_h.ap()[tb * P:(tb + 1) * P, :],
                        tstg.pop(tb)[:].rearrange("p a f -> p (a f)"))

    nc.compile()
    return nc


def get_nc():
    if "nc" not in _CACHE:
        _CACHE["nc"] = _build()
    return _CACHE["nc"]


def _perm512():
    perm = np.empty(512, np.int64)
    i = 0
    for t in range(2):
        for h in range(2):
            for beta in range(4):
                pr = 2 * t + beta // 2
                u = beta % 2
                for dd in range(32):
                    perm[i] = pr * 128 + u * 64 + h * 32 + dd
                    i += 1
    return perm


def kernel(x, Wq, Wk, Wv, Wo, bo):
    import ml_dtypes
    from concourse import bass_utils

    e4 = ml_dtypes.float8_e4m3

    x = np.asarray(x, dtype=np.float32)
    Wq, Wk, Wv = (np.asarray(w, dtype=np.float32) for w in (Wq, Wk, Wv))
    Wo = np.asarray(Wo, dtype=np.float32)
    bo = np.asarray(bo, dtype=np.float32)
    perm = _perm512()

    in_maps = []
    for c in range(NCORES):
        b, g = c // 2, c % 2
        gsl = slice(g * 512, (g + 1) * 512)
        xT = np.ascontiguousarray(x[b].T)
        x8 = xT.astype(e4)
        wv16 = np.ascontiguousarray(16.0 * Wv[gsl].T)
        wv8h = wv16.astype(e4)
        in_maps.append({
            "x8": x8.view(np.uint8),
            # residual of the e4m3 x quantization (V-proj pass 2)
            "x8l": (xT - x8.astype(np.float32)).astype(e4).view(np.uint8),
            # w quantized x16 to dodge e4m3 subnormals; unscaled on-chip
            "wq8": np.ascontiguousarray((16.0 * Wq[gsl].T)[:, perm])
            .astype(e4).view(np.uint8),
            "wk8": np.ascontiguousarray((16.0 * Wk[gsl].T)[:, perm])
            .astype(e4).view(np.uint8),
            "wv8h": wv8h.view(np.uint8),
            "wv8l": (wv16 - wv8h.astype(np.float32)).astype(e4)
            .view(np.uint8),
            "woh": np.ascontiguousarray(Wo[:, gsl].T).astype(np.float16),
        })

    nc = get_nc()
    res = bass_utils.run_bass_kernel_spmd(nc, in_maps,
                                          core_ids=list(range(NCORES)))
    parts = [res.results[c]["out"].astype(np.float32) for c in range(NCORES)]
    out = np.stack([parts[2 * b] + parts[2 * b + 1] + bo for b in range(B)])
    return out.astype(np.float32)


# revision 77
# speedup vs baseline: 1.1066x; 1.0063x over previous
"""Multi-head causal attention (B=4, T=2048, D=1024, H=16) on 8 Trainium2 cores.

Sharding: core c = (b, g) with b = c//2 (batch), g = c%2 (head-group of 8 heads).
Each core: Q/K/V projections for its 8 heads (column-parallel), causal attention,
row-parallel partial output projection. Host sums the g=0/g=1 partials + bias.

v4 design (fp8 DoubleRow + streaming AV; cost model: matmul = out-free-rows x
0.4167ns x cpr, fp8e4 DoubleRow cpr=0.5 contracting 2x128/instr; ACT exp at
0.8333ns/free-elem = ~147us busy is the floor):
  - Q/K proj: fp8 DR, x8 moving [128,2,512], w8 stationary [128,2,128] in 4
    col-groups (t=pair-half, h=dim-half) so psum partitions land as
    (beta=2*(pr%2)+u, dd) blocks of 32 -> qT8/kT8 [32-blocks, 2h, T] fp8 for
    2x32-contraction DR score matmuls. w quantized x16 (e4m3 subnormal
    dodge), unscaled in the DVE psum->fp8 copy; score scale 1/8 folded as
    qT8 = e4m3(q/2) + exp(scale=0.25). Sub-128 DR needs explicit
    tile_position (base-96 slices break base_partition inference).
  - scores: fp8 DR per (pr,u,kj): out ss [128 keys, 2u, 512-c0] psum.
  - exp: ACT psum -> sbuf, one instr per kj covering both heads. Off-diag
    chunks -> fp8 pt pairs [128, 2kj, 2u, 512]; diag chunks -> bf16 pt
    (pt-quant error only bites concentrated near-diagonal rows) with
    post-exp mask01 multiply on DVE (off ACT's critical path). Diag m2+m3
    share one ss tile (m3's 128 live cols parked at [128:256]) and one
    exp + one mask-mul; the AV remaps positionally (out col offset is
    independent of rhs offset) -- same elements, 16 fewer ACT instrs.
  - AV streaming into ctx psum [96, 512] per (pr,u), accumulated across the
    span: off-diag = single-fp8 DR (lhsT = Vbh [128, 2kj, 96]: 64 v-dims +
    ones col at 64 -> Z at psum row 64 + zero pad -- dual-fp8 ldweights
    needs cols%32==0, >=64); diag = bf16 non-DR (lhsT = Vbb [128, 65]).
    Diffuse off-diag rows average out single-fp8 V error; vonly ~1e-3.
  - evict per (pr,u): DVE reciprocal of Z row -> bf16 rz at partition 0
    (cross-partition psum read, hw-verified), gpsimd partition_broadcast ->
    rzbc sbuf, DVE mul ctx*rzbc -> ctx_sb f16 [128=(u,vd), 512] (u1 written
    cross-partition to rows 64:128). Evicts deferred into the next pair's /
    span's kj loop so the PE never stalls on them.
  - outproj: f16, 4x128-contraction per [128q, 512od] psum group; DVE f16
    stage -> DMA (psum DMA is forbidden); host sums g-partials + bias f32.
  - V proj: 3-pass fp8 DR (16v = xh@wh + xl@wh + xh@wl; w x16-scaled
    hi/lo split keeps the value path ~bf16 quality; psum/16 on copy).
  - schedule: only Q(s+1) is paced as filler through attention(s); K(s) and
    V(s) are JIT-emitted inside span s itself (first consumers are its own
    diag QK/AV), spread across pr0/pr1 kj iterations AFTER each emit_qk --
    K keys must land strictly before kj=4s or the diag QK reads stale kT
    (caused a NaN once). outproj(0)@s2, outproj(1,2)@s3, outproj(3) tail.
    ALL DMAs on the SP queue (the DMA bus serializes at ~1.45us/512KB; a
    DMACopy SEQ-hold on ACT blocks exp dispatch, on Pool it blocks
    partition_broadcast). Early proj psum->fp8 copies (span-0 era) run as
    ACT activation-Copy w/ scale (ACT idle there, DVE is the choke) -- but
    ONLY ops whose inputs are ready early (ACT is in-order; late-input
    copies head-of-line block exps). Tail outproj pc-interleaved over 8
    scratch psums; per-tb merged 1024-col out DMAs.
Measured: 184418 ns (TimelineSim), rel err 1.21e-2 (gate 2e-2). Baseline
v3 was 240443 ns. ACT exp busy 146.9us (73%), PE 129us, DVE 118us.
Every pair's evict is split: cols [0:256] normalize under the remaining
diag AVs (final after avd(m1); m2/m3 only touch higher cols incl. their
Z-row adds), so only [256:512] rides each pair's freeing chain -- the
psC rotation unblocks the next pair's AV ~1us earlier, and the tail
chain shortens. 3-way splits regress (per-chain overhead). Remaining
idle: ~5us startup (DMA-bus chain), ~6us span-0/1 boundary (ss-pool
bufs=2 serializes the exp restart; psum full at 8 banks), tail drain.
"""

import os
import sys

try:
    import concourse.bass  # noqa: F401
except ImportError:  # pragma: no cover
    sys.path.insert(0, "/opt/trn_rl_repo")

import numpy as np

B, T, D = 4, 2048, 1024
H, HD = 16, 64
NCORES = 8
NPAIR = 4
NSPAN = 4
SPAN = 512
KC = 128
P = 128
LAG = 3

_CACHE = {}


def _build():
    import concourse.bacc as bacc
    import concourse.mybir as mybir
    import concourse.tile as tile

    f32 = mybir.dt.float32
    f16 = mybir.dt.float16
    bf16 = mybir.dt.bfloat16
    fp8 = mybir.dt.float8e4
    u8 = mybir.dt.uint8
    Exp = mybir.ActivationFunctionType.Exp

    DR = mybir.MatmulPerfMode.DoubleRow

    nc = bacc.Bacc("TRN2", target_bir_lowering=False, debug=False,
                   num_devices=NCORES)

    x8_h = nc.dram_tensor("x8", (D, T), u8, kind="ExternalInput")
    x8l_h = nc.dram_tensor("x8l", (D, T), u8, kind="ExternalInput")
    wq8_h = nc.dram_tensor("wq8", (D, 512), u8, kind="ExternalInput")
    wk8_h = nc.dram_tensor("wk8", (D, 512), u8, kind="ExternalInput")
    wv8h_h = nc.dram_tensor("wv8h", (D, 512), u8, kind="ExternalInput")
    wv8l_h = nc.dram_tensor("wv8l", (D, 512), u8, kind="ExternalInput")
    woh_h = nc.dram_tensor("woh", (512, D), f16, kind="ExternalInput")
    out_h = nc.dram_tensor("out", (T, D), f16, kind="ExternalOutput")

    # x8: D = (dcp 4, k 2, p 128); per span slice on T
    x8_d = x8_h.ap().rearrange("(dcp k p) t -> p dcp k t", p=P, k=2)
    x8l_d = x8l_h.ap().rearrange("(dcp k p) t -> p dcp k t", p=P, k=2)
    wq8_d = wq8_h.ap().rearrange("(dcp k p) (t h m) -> p dcp k t h m",
                                 p=P, k=2, t=2, h=2)
    wk8_d = wk8_h.ap().rearrange("(dcp k p) (t h m) -> p dcp k t h m",
                                 p=P, k=2, t=2, h=2)
    wv8h_d = wv8h_h.ap().rearrange("(dcp k p) f -> p dcp k f", p=P, k=2)
    wv8l_d = wv8l_h.ap().rearrange("(dcp k p) f -> p dcp k f", p=P, k=2)
    woh_d = woh_h.ap().rearrange("(pc p) f -> p pc f", p=P)

    with tile.TileContext(nc) as tc:
        with (
            tc.tile_pool(name="persist", bufs=1) as persist,
            tc.tile_pool(name="x8p", bufs=2) as x8pool,
            tc.tile_pool(name="xlp", bufs=2) as xlpool,
            tc.tile_pool(name="qp", bufs=4) as qpool,
            tc.tile_pool(name="ptp", bufs=6) as ptpool,
            tc.tile_pool(name="ptb", bufs=6) as ptbpool,
            tc.tile_pool(name="rzp", bufs=12) as rzpool,
            tc.tile_pool(name="cp", bufs=8) as cpool,
            tc.tile_pool(name="stg", bufs=6) as stgpool,
            tc.tile_pool(name="psS", bufs=2, space="PSUM") as psS,
            tc.tile_pool(name="psC", bufs=2, space="PSUM") as psC,
            tc.tile_pool(name="psT", bufs=2, space="PSUM") as psT,
        ):
            wq8 = persist.tile([P, 4, 2, 2, 2, P], u8, tag="wq8", name="wq8")
            wk8 = persist.tile([P, 4, 2, 2, 2, P], u8, tag="wk8", name="wk8")
            wv8 = persist.tile([P, 4, 2, 2, 512], u8, tag="wv8", name="wv8")
            wot = persist.tile([P, 4, D], f16, tag="wot", name="wot")
            kT8 = [persist.tile([P, 2, T], fp8, tag=f"kT8{t}", name=f"kT8{t}")
                   for t in range(2)]
            # Vb: [p, kjp 8, kj 2, pr 4, u 2, 96]: 64 v-dims + ones col at
            # 64 + zero pad to 96 (dual-fp8 ldweights needs cols % 32 == 0,
            # >= 64; psum rows 65:96 are dead)
            VW = 96
            Vbh = persist.tile([P, 8, 2, NPAIR, 2, VW], fp8, tag="Vbh",
                               name="Vbh")
            # diag-chunk AV runs in bf16 (pt quantization error bites only
            # concentrated near-diagonal attention rows): [p, kc, pr, u, 65]
            Vbb = persist.tile([P, 16, NPAIR, 2, HD + 1], bf16, tag="Vbb",
                               name="Vbb")
            mask01 = persist.tile([P, KC], bf16, tag="mask01", name="mask01")
            one = nc.const_aps.tensor(1.0, (P, 1))

            # ---- initial DMAs ----
            x8ts = {0: x8pool.tile([P, 4, 2, SPAN], u8, tag="x8t", name="x8t0")}
            x8ls = {0: xlpool.tile([P, 4, 2, SPAN], u8, tag="x8l", name="x8l0")}
            nc.sync.dma_start(wq8[:], wq8_d)
            nc.sync.dma_start(x8ts[0][:], x8_d[:, :, :, 0:SPAN])
            nc.sync.dma_start(wk8[:], wk8_d)
            nc.sync.dma_start(wv8[:, :, :, 0, :], wv8h_d)
            nc.sync.dma_start(x8ls[0][:], x8l_d[:, :, :, 0:SPAN])
            nc.sync.dma_start(wv8[:, :, :, 1, :], wv8l_d)
            nc.sync.dma_start(wot[:], woh_d)


            # mask01[p, f] = 1 if p <= f else 0 (post-exp diag pt mask)
            nc.gpsimd.memset(mask01[:], 1.0)
            nc.gpsimd.affine_select(
                out=mask01[:], in_=mask01[:],
                compare_op=mybir.AluOpType.is_ge, fill=0.0,
                base=0, channel_multiplier=-1, pattern=[[1, KC]],
            )
            # Vbh pad cols [65:96] must be zero (read by every off-diag
            # AV); data cols are always written before first read
            nslots = 8 * 2 * NPAIR * 2
            nc.gpsimd.memset(
                Vbh[:].rearrange("p a b c d e -> p (a b c d) e")
                [:, :, HD + 1:], 0.0)
            nc.vector.tensor_copy(
                Vbh[:].rearrange("p a b c d e -> p (a b c d) e")
                [:, :, HD:HD + 1],
                one.to_broadcast((P, nslots, 1)))
            nc.vector.tensor_copy(
                Vbb[:].rearrange("p a c d e -> p (a c d) e")
                [:, :, HD:HD + 1],
                one.to_broadcast((P, 16 * NPAIR * 2, 1)))

            qts = {}    # (sp, t) -> [P, 2, SPAN] fp8 tile
            ctxs = {}   # (sp, pr) -> [P, SPAN] f16 tile

            # ---------- emission helpers ----------
            def proj_qk(w8, pr_half, h, sp, x8t, scale, isq, on_act=False):
                t = pr_half

                def emit():
                    ps = psT.tile([P, SPAN], f32, tag="tr", name="psqk")
                    for dcp in range(4):
                        nc.tensor.matmul(
                            ps[:], w8[:, dcp, :, t, h, :].bitcast(fp8),
                            x8t[:, dcp, :, :].bitcast(fp8),
                            start=(dcp == 0), stop=(dcp == 3), perf_mode=DR)
                    if isq:
                        if (sp, t) not in qts:
                            qts[(sp, t)] = qpool.tile(
                                [P, 2, SPAN], fp8, tag=f"qT{t}",
                                name=f"qT{t}_{sp}")
                        dest = qts[(sp, t)][:, h, :]
                    else:
                        dest = kT8[t][:, h, sp * SPAN:(sp + 1) * SPAN]
                    if on_act:
                        # span-0 era: ACT is idle and DVE is the choke --
                        # the psum->fp8 scale-copy runs as activation Copy
                        nc.scalar.activation(
                            dest, ps[:], mybir.ActivationFunctionType.Copy,
                            scale=scale)
                    else:
                        nc.vector.tensor_scalar_mul(dest, ps[:], scale)
                return emit

            def proj_v(sp, tb, x8t, x8lt):
                # 3-pass fp8 DR: 16v = x_hi@w_hi + x_lo@w_hi + x_hi@w_lo
                # (w residual split keeps the value path at ~bf16 quality)
                def emit():
                    ps = psT.tile([P, SPAN], f32, tag="tr", name="psv")
                    ts = slice(tb * P, (tb + 1) * P)
                    passes = [(x8t, 0), (x8lt, 0), (x8t, 1)]
                    for pi, (xt, wl) in enumerate(passes):
                        for dcp in range(4):
                            nc.tensor.matmul(
                                ps[:], xt[:, dcp, :, ts].bitcast(fp8),
                                wv8[:, dcp, :, wl, :].bitcast(fp8),
                                start=(pi == 0 and dcp == 0),
                                stop=(pi == 2 and dcp == 3), perf_mode=DR)
                    kc = sp * 4 + tb
                    psv = ps[:].rearrange("p (pr u v) -> p pr u v",
                                          pr=NPAIR, v=HD)
                    nc.vector.tensor_scalar_mul(
                        Vbh[:, kc // 2, kc % 2, :, :, 0:HD], psv, 1.0 / 16)
                    nc.vector.tensor_scalar_mul(
                        Vbb[:, kc, :, :, 0:HD], psv, 1.0 / 16)
                return emit

            def q_groups(sp, x8t, on_act=False):
                return [proj_qk(wq8, t, h, sp, x8t, 1.0 / 32, True, on_act)
                        for t in range(2) for h in range(2)]

            def k_groups(sp, x8t, on_act=False):
                return [proj_qk(wk8, t, h, sp, x8t, 1.0 / 16, False, on_act)
                        for t in range(2) for h in range(2)]

            def v_groups(sp, x8t, x8lt):
                return [proj_v(sp, tb, x8t, x8lt) for tb in range(4)]

            ostages = {}  # tb -> [P, 2, SPAN] f16 stage (one DMA per tb)

            def outproj_group(sp, tb, os_, dma_q, ps_ap=None, act_copy=False):
                def emit():
                    if ps_ap is None:
                        ps = psT.tile([P, SPAN], f32, tag="tr", name="pso")[:]
                    else:
                        ps = ps_ap
                    for pc in range(NPAIR):
                        nc.tensor.matmul(
                            ps,
                            ctxs[(sp, pc)][:, (tb - sp * 4) * P:
                                           (tb - sp * 4 + 1) * P],
                            wot[:, pc, os_ * SPAN:(os_ + 1) * SPAN],
                            start=(pc == 0), stop=(pc == NPAIR - 1))
                    if os_ == 0:
                        ostages[tb] = stgpool.tile([P, 2, SPAN], f16,
                                                   tag="st", name="stage")
                    stage = ostages[tb]
                    if act_copy:
                        nc.scalar.activation(
                            stage[:, os_, :], ps,
                            mybir.ActivationFunctionType.Copy)
                    else:
                        nc.vector.tensor_copy(stage[:, os_, :], ps)
                    if os_ == 1:
                        # one merged 1024-col DMA per tb: fewer bus slots
                        dma_q.dma_start(
                            out_h.ap()[tb * P:(tb + 1) * P, :],
                            ostages.pop(tb)[:].rearrange("p a f -> p (a f)"))
                return emit

            def outproj_groups(sp):
                # all out-DMAs on SP: a Pool-queue DMACopy hold would block
                # partition_broadcast (evict chain)
                return [outproj_group(sp, tb, os_, nc.sync)
                        for tb in range(sp * 4, (sp + 1) * 4)
                        for os_ in range(2)]

            # ---------- attention for one span ----------
            deferred = []

            def attn_span(s, fillers, vgs=(), kgs=()):
                K = 4 * (s + 1)
                nslot = (K + 4) * NPAIR
                state = {"slot": 0, "fi": 0}

                def pace():
                    tgt = min(len(fillers),
                              len(fillers) * (state["slot"] + 1) // nslot)
                    while state["fi"] < tgt:
                        fillers[state["fi"]]()
                        state["fi"] += 1

                def tick():
                    state["slot"] += 1
                    pace()

                for pr in range(NPAIR):
                    t = pr // 2
                    ctxp = [psC.tile([96, SPAN], f32, tag="ctx",
                                     name=f"ctx{u}") for u in range(2)]
                    ct = cpool.tile([P, SPAN], f16, tag=f"cT{pr}",
                                    name=f"cT{pr}_{s}")
                    ctxs[(s, pr)] = ct
                    pts = {}
                    qt = qts[(s, t)]

                    avn = [0, 0]  # AV matmuls emitted per u (K total each)

                    def emit_qk(kj, pr=pr, t=t, pts=pts, qt=qt):
                        m = kj - 4 * s
                        c0 = 0 if m < 0 else m * KC
                        if m == 2:
                            # m2+m3 share one ss tile and one exp: m2 scores
                            # at cols [256:512], m3's 128 live cols parked in
                            # the unused [128:256]; exp'd together at m3
                            ss = psS.tile([P, 2, SPAN], f32, tag="psS",
                                          name="ss23")
                            pts[("ss23",)] = ss
                        elif m == 3:
                            ss = pts[("ss23",)]
                        else:
                            ss = psS.tile([P, 2, SPAN], f32, tag="psS",
                                          name="ss")
                        dst = slice(KC, 2 * KC) if m == 3 else slice(c0, SPAN)
                        for u in range(2):
                            b32 = 32 * (2 * (pr % 2) + u)
                            r = slice(b32, b32 + 32)
                            nc.tensor.matmul(
                                ss[:, u, dst],
                                kT8[t][r, :, kj * KC:(kj + 1) * KC],
                                qt[r, :, c0:],
                                start=True, stop=True, perf_mode=DR,
                                tile_position=(b32, 0))
                        if m < 0:
                            # off-diagonal: fp8 pt pair for DR AV
                            kjp, sl = kj // 2, kj % 2
                            if sl == 0:
                                pts[kjp] = ptpool.tile([P, 2, 2, SPAN], fp8,
                                                       tag="pt", name="pt")
                            nc.scalar.activation(pts[kjp][:, sl, :, :],
                                                 ss[:, :, :], Exp, scale=0.25)
                        elif m == 2:
                            # defer: exp'd together with m3
                            ptb = ptbpool.tile([P, 2, SPAN], bf16,
                                               tag="ptb", name="ptb")
                            pts[("d", kj)] = ptb
                            pts[("d", kj + 1)] = ptb
                        elif m == 3:
                            ptb = pts[("d", kj)]
                            del pts[("ss23",)]
                            nc.scalar.activation(ptb[:, :, KC:],
                                                 ss[:, :, KC:], Exp,
                                                 scale=0.25)
                            # both triangles ([128:256] = m3, [256:384] = m2)
                            # use the same 128x128 mask: one DVE mul
                            nc.vector.tensor_mul(
                                ptb[:, :, KC:3 * KC].rearrange(
                                    "p u (b f) -> p u b f", b=2),
                                ptb[:, :, KC:3 * KC].rearrange(
                                    "p u (b f) -> p u b f", b=2),
                                mask01[:].rearrange("p (u b f) -> p u b f",
                                                    u=1, b=1)
                                .to_broadcast((P, 2, 2, KC)))
                        else:
                            # diagonal m0/m1: own tile + exp + triangle mask
                            ptb = ptbpool.tile([P, 2, SPAN], bf16,
                                               tag="ptb", name="ptb")
                            pts[("d", kj)] = ptb
                            nc.scalar.activation(ptb[:, :, c0:],
                                                 ss[:, :, c0:], Exp,
                                                 scale=0.25)
                            nc.vector.tensor_mul(
                                ptb[:, :, c0:c0 + KC], ptb[:, :, c0:c0 + KC],
                                mask01[:].rearrange("p (u f) -> p u f", u=1)
                                .to_broadcast((P, 2, KC)))

                    NU = 2 * s + 4  # AV matmuls per u-chain

                    def emit_av8(kjp, pr=pr, pts=pts, ctxp=ctxp, NU=NU):
                        pt = pts.pop(kjp)
                        for u in range(2):
                            nc.tensor.matmul(
                                ctxp[u][:],
                                Vbh[:, kjp, :, pr, u, :],
                                pt[:, :, u, :],
                                start=(avn[u] == 0),
                                stop=(avn[u] == NU - 1),
                                perf_mode=DR, skip_group_check=True)
                            avn[u] += 1

                    def emit_avd(kj, pr=pr, pts=pts, ctxp=ctxp, NU=NU):
                        ptb = pts.pop(("d", kj))
                        m = kj - 4 * s
                        c0 = m * KC
                        # m3's pt lives at stored cols [128:256] (positional
                        # matmul mapping: out offset is independent of rhs)
                        rsl = slice(KC, 2 * KC) if m == 3 else slice(c0, SPAN)
                        for u in range(2):
                            nc.tensor.matmul(
                                ctxp[u][0:HD + 1, c0:],
                                Vbb[:, kj, pr, u, :],
                                ptb[:, u, rsl],
                                start=(avn[u] == 0), stop=(avn[u] == NU - 1),
                                skip_group_check=True)
                            avn[u] += 1

                    rzt = [rzpool.tile([P, 2, SPAN], bf16, tag="rz",
                                       name="rz"),
                           rzpool.tile([P, 2, SPAN], bf16, tag="rz",
                                       name="rzbc")]

                    def fin(u, cs=slice(0, SPAN), ctxp=ctxp, ct=ct,
                            rzt=rzt):
                        rz, rzbc = rzt

                        def run():
                            # 1/Z row: psum row 64 -> sbuf row 0
                            # (cross-partition DVE write, hw-verified)
                            with nc.allow_low_precision(
                                    reason="1/Z bf16: 0.4% on ctx"):
                                nc.vector.reciprocal(
                                    rz[0:1, u, cs], ctxp[u][64:65, cs])
                            nc.gpsimd.partition_broadcast(
                                rzbc[:, u, cs], rz[0:1, u, cs])
                            nc.vector.tensor_mul(
                                ct[u * HD:(u + 1) * HD, cs],
                                ctxp[u][0:HD, cs], rzbc[0:HD, u, cs])
                        return run

                    def evict(pr=pr, s=s):
                        # [0:256] was evicted early (see units)
                        deferred.append(fin(0, slice(256, SPAN)))
                        deferred.append(fin(1, slice(256, SPAN)))

                    # AV work units: (ready_kj, emit_fn); off-diag kjp ready
                    # at its odd kj, diag kj ready at kj
                    units = []
                    for kj2 in range(K):
                        if kj2 < 4 * s:
                            if kj2 % 2 == 1:
                                units.append((kj2, kj2 // 2, emit_av8))
                        else:
                            units.append((kj2, kj2, emit_avd))
                            if kj2 == 4 * s + 1:
                                # ctx cols [0:256] are final after avd(m1):
                                # evict them under the remaining diag AVs
                                # (frees the psC tile ~1us earlier for the
                                # next pair's AV chain)
                                units.append(
                                    (kj2, None,
                                     lambda _, f0=fin(0, slice(0, 256)),
                                     f1=fin(1, slice(0, 256)):
                                     (f0(), f1())))
                    # JIT placement: V(s) spread across pr0's whole kj
                    # range (consumers: pr0 diag AV); K(s) t0-groups in pr0,
                    # t1-groups in pr1 (consumers: pr0/pr2 diag QK)
                    jit = {}
                    if pr == 0:
                        for i in range(len(vgs)):
                            jit.setdefault(max(i * K // 4, i), []).append(
                                vgs[i])
                        # K(s) t0 keys must precede the diag QK at kj=4s
                        # (jit runs after emit_qk in the same iteration)
                        for i in (0, 1) if kgs else ():
                            jit.setdefault(min(1 + i * (K // 3 + 1),
                                               4 * s - 1), []).append(kgs[i])
                    elif pr == 1:
                        for i in (2, 3) if kgs else ():
                            jit.setdefault(1 + (i - 2) * (K // 3 + 1),
                                           []).append(kgs[i])
                    ui = [0]
                    for kj in range(K):
                        emit_qk(kj)
                        for g in jit.get(kj, ()):
                            g()
                        if deferred:
                            deferred.pop(0)()
                        tick()
                        while (ui[0] < len(units)
                               and units[ui[0]][0] + LAG <= kj):
                            units[ui[0]][2](units[ui[0]][1])
                            ui[0] += 1
                    while ui[0] < len(units):
                        if deferred:
                            deferred.pop(0)()
                        tick()
                        units[ui[0]][2](units[ui[0]][1])
                        ui[0] += 1
                    evict()
                while state["fi"] < len(fillers):
                    fillers[state["fi"]]()
                    state["fi"] += 1

            # ---------- main schedule ----------
            # V(s) and K(s) run inside span s itself (their first consumers
            # are span s's own diag AV / diag QK) -- keeps earlier spans off
            # the PE; only Q(s+1) must finish during span s
            # pr0's attention needs (q,k) t0 groups first: interleave
            qg0 = q_groups(0, x8ts[0], on_act=True)
            kg0 = k_groups(0, x8ts[0], on_act=True)
            for i in range(4):
                qg0[i]()
                kg0[i]()
            for s in range(NSPAN):
                vgs = v_groups(s, x8ts[s], x8ls[s])
                kgs = k_groups(s, x8ts[s]) if s >= 1 else ()
                fillers = []
                if s + 1 < NSPAN:
                    x8t = x8pool.tile([P, 4, 2, SPAN], u8, tag="x8t",
                                      name=f"x8t{s + 1}")
                    x8lt = xlpool.tile([P, 4, 2, SPAN], u8, tag="x8l",
                                       name=f"x8l{s + 1}")
                    x8ts[s + 1], x8ls[s + 1] = x8t, x8lt
                    sl = slice((s + 1) * SPAN, (s + 2) * SPAN)
                    nc.sync.dma_start(x8t[:], x8_d[:, :, :, sl])
                    nc.sync.dma_start(x8lt[:], x8l_d[:, :, :, sl])
                    fillers += q_groups(s + 1, x8t, on_act=(s == 0))
                if s == 2:
                    fillers += outproj_groups(0)
                elif s == 3:
                    fillers += outproj_groups(1) + outproj_groups(2)
                attn_span(s, fillers, vgs, kgs)
            while deferred:
                deferred.pop(0)()
            # tail outproj(3): attention is done, so psS's 4 banks are free
            # scratch -- 6 parallel psums let all 32 matmuls run back-to-back
            # (no psT-rotation stalls, PE stays ramped); stage copies split
            # DVE/ACT (both idle at the tail)
            sc = [psT.tile([P, SPAN], f32, tag="tr", name=f"ost{i}")[:]
                  for i in range(2)]
            for i in range(2):
                t = psS.tile([P, 2, SPAN], f32, tag="psS", name=f"osc{i}")
                sc += [t[:, 0, :], t[:, 1, :]]
            for i in range(2):
                sc.append(psC.tile([P, SPAN], f32, tag="ctx",
                                   name=f"oscc{i}")[:])
            tails = [(tb, os_) for tb in range(12, 16) for os_ in range(2)]
            # pc-interleaved: pc0..2 matmuls only need the earlier pairs'
            # ctx and run under the last evict chain; only pc3 gates on it
            for pc in range(NPAIR):
                for gi, (tb, os_) in enumerate(tails):
                    nc.tensor.matmul(
                        sc[gi],
                        ctxs[(3, pc)][:, (tb - 12) * P:(tb - 11) * P],
                        wot[:, pc, os_ * SPAN:(os_ + 1) * SPAN],
                        start=(pc == 0), stop=(pc == NPAIR - 1),
                        skip_group_check=True)
            tstg = {}
            for gi, (tb, os_) in enumerate(tails):
                if os_ == 0:
                    tstg[tb] = stgpool.tile([P, 2, SPAN], f16, tag="st",
                                            name="tstage")
                if gi % 2 == 1:
                    nc.scalar.activation(
                        tstg[tb][:, os_, :], sc[gi],
                        mybir.ActivationFunctionType.Copy)
                else:
                    nc.vector.tensor_copy(tstg[tb][:, os_, :], sc[gi])
                if os_ == 1:
                    nc.sync.dma_start(
                        out_h.ap()[tb * P:(tb + 1) * P, :],
                        tstg.pop(tb)[:].rearrange("p a f -> p (a f)"))

    nc.compile()
    return nc


def get_nc():
    if "nc" not in _CACHE:
        _CACHE["nc"] = _build()
    return _CACHE["nc"]


def _perm512():
    perm = np.empty(512, np.int64)
    i = 0
    for t in range(2):
        for h in range(2):
            for beta in range(4):
                pr = 2 * t + beta // 2
                u = beta % 2
                for dd in range(32):
                    perm[i] = pr * 128 + u * 64 + h * 32 + dd
                    i += 1
    return perm


def kernel(x, Wq, Wk, Wv, Wo, bo):
    import ml_dtypes
    from concourse import bass_utils

    e4 = ml_dtypes.float8_e4m3

    x = np.asarray(x, dtype=np.float32)
    Wq, Wk, Wv = (np.asarray(w, dtype=np.float32) for w in (Wq, Wk, Wv))
    Wo = np.asarray(Wo, dtype=np.float32)
    bo = np.asarray(bo, dtype=np.float32)
    perm = _perm512()

    in_maps = []
    for c in range(NCORES):
        b, g = c // 2, c % 2
        gsl = slice(g * 512, (g + 1) * 512)
        xT = np.ascontiguousarray(x[b].T)
        x8 = xT.astype(e4)
        wv16 = np.ascontiguousarray(16.0 * Wv[gsl].T)
        wv8h = wv16.astype(e4)
        in_maps.append({
            "x8": x8.view(np.uint8),
            # residual of the e4m3 x quantization (V-proj pass 2)
            "x8l": (xT - x8.astype(np.float32)).astype(e4).view(np.uint8),
            # w quantized x16 to dodge e4m3 subnormals; unscaled on-chip
            "wq8": np.ascontiguousarray((16.0 * Wq[gsl].T)[:, perm])
            .astype(e4).view(np.uint8),
            "wk8": np.ascontiguousarray((16.0 * Wk[gsl].T)[:, perm])
            .astype(e4).view(np.uint8),
            "wv8h": wv8h.view(np.uint8),
            "wv8l": (wv16 - wv8h.astype(np.float32)).astype(e4)
            .view(np.uint8),
            "woh": np.ascontiguousarray(Wo[:, gsl].T).astype(np.float16),
        })

    nc = get_nc()
    res = bass_utils.run_bass_kernel_spmd(nc, in_maps,
                                          core_ids=list(range(NCORES)))
    parts = [res.results[c]["out"].astype(np.float32) for c in range(NCORES)]
    out = np.stack([parts[2 * b] + parts[2 * b + 1] + bo for b in range(B)])
    return out.astype(np.float32)


# revision 82
# speedup vs baseline: 1.1067x; 1.0001x over previous
"""Multi-head causal attention (B=4, T=2048, D=1024, H=16) on 8 Trainium2 cores.

Sharding: core c = (b, g) with b = c//2 (batch), g = c%2 (head-group of 8 heads).
Each core: Q/K/V projections for its 8 heads (column-parallel), causal attention,
row-parallel partial output projection. Host sums the g=0/g=1 partials + bias.

v4 design (fp8 DoubleRow + streaming AV; cost model: matmul = out-free-rows x
0.4167ns x cpr, fp8e4 DoubleRow cpr=0.5 contracting 2x128/instr; ACT exp at
0.8333ns/free-elem = ~147us busy is the floor):
  - Q/K proj: fp8 DR, x8 moving [128,2,512], w8 stationary [128,2,128] in 4
    col-groups (t=pair-half, h=dim-half) so psum partitions land as
    (beta=2*(pr%2)+u, dd) blocks of 32 -> qT8/kT8 [32-blocks, 2h, T] fp8 for
    2x32-contraction DR score matmuls. w quantized x16 (e4m3 subnormal
    dodge), unscaled in the DVE psum->fp8 copy; score scale 1/8 folded as
    qT8 = e4m3(q/2) + exp(scale=0.25). Sub-128 DR needs explicit
    tile_position (base-96 slices break base_partition inference).
  - scores: fp8 DR per (pr,u,kj): out ss [128 keys, 2u, 512-c0] psum.
  - exp: ACT psum -> sbuf, one instr per kj covering both heads. Off-diag
    chunks -> fp8 pt pairs [128, 2kj, 2u, 512]; diag chunks -> bf16 pt
    (pt-quant error only bites concentrated near-diagonal rows) with
    post-exp mask01 multiply on DVE (off ACT's critical path). Diag m2+m3
    share one ss tile (m3's 128 live cols parked at [128:256]) and one
    exp + one mask-mul; the AV remaps positionally (out col offset is
    independent of rhs offset) -- same elements, 16 fewer ACT instrs.
  - AV streaming into ctx psum [96, 512] per (pr,u), accumulated across the
    span: off-diag = single-fp8 DR (lhsT = Vbh [128, 2kj, 96]: 64 v-dims +
    ones col at 64 -> Z at psum row 64 + zero pad -- dual-fp8 ldweights
    needs cols%32==0, >=64); diag = bf16 non-DR (lhsT = Vbb [128, 65]).
    Diffuse off-diag rows average out single-fp8 V error; vonly ~1e-3.
  - evict per (pr,u): DVE reciprocal of Z row -> bf16 rz at partition 0
    (cross-partition psum read, hw-verified), gpsimd partition_broadcast ->
    rzbc sbuf, DVE mul ctx*rzbc -> ctx_sb f16 [128=(u,vd), 512] (u1 written
    cross-partition to rows 64:128). Evicts deferred into the next pair's /
    span's kj loop so the PE never stalls on them.
  - outproj: f16, 4x128-contraction per [128q, 512od] psum group; DVE f16
    stage -> DMA (psum DMA is forbidden); host sums g-partials + bias f32.
  - V proj: 3-pass fp8 DR (16v = xh@wh + xl@wh + xh@wl; w x16-scaled
    hi/lo split keeps the value path ~bf16 quality; psum/16 on copy).
  - schedule: only Q(s+1) is paced as filler through attention(s); K(s) and
    V(s) are JIT-emitted inside span s itself (first consumers are its own
    diag QK/AV), spread across pr0/pr1 kj iterations AFTER each emit_qk --
    K keys must land strictly before kj=4s or the diag QK reads stale kT
    (caused a NaN once). outproj(0)@s2, outproj(1,2)@s3, outproj(3) tail.
    ALL DMAs on the SP queue (the DMA bus serializes at ~1.45us/512KB; a
    DMACopy SEQ-hold on ACT blocks exp dispatch, on Pool it blocks
    partition_broadcast). Early proj psum->fp8 copies (span-0 era) run as
    ACT activation-Copy w/ scale (ACT idle there, DVE is the choke) -- but
    ONLY ops whose inputs are ready early (ACT is in-order; late-input
    copies head-of-line block exps). Tail outproj pc-interleaved over 8
    scratch psums; per-tb merged 1024-col out DMAs.
Measured: 183265 ns (TimelineSim), rel err 1.21e-2 (gate 2e-2). Baseline
v3 was 240443 ns. ACT exp busy 146.9us (73%), PE 129us, DVE 118us.
Every pair's evict is split: cols [0:256] normalize under the remaining
diag AVs (final after avd(m1); m2/m3 only touch higher cols incl. their
Z-row adds), so only [256:512] rides each pair's freeing chain -- the
psC rotation unblocks the next pair's AV ~1us earlier, and the tail
chain shortens. 3-way splits regress (per-chain overhead). Remaining
idle: ~5us startup (DMA-bus chain), ~6us span-0/1 boundary (ss-pool
bufs=2 serializes the exp restart; psum full at 8 banks), tail drain.
"""

import os
import sys

try:
    import concourse.bass  # noqa: F401
except ImportError:  # pragma: no cover
    sys.path.insert(0, "/opt/trn_rl_repo")

import numpy as np

B, T, D = 4, 2048, 1024
H, HD = 16, 64
NCORES = 8
NPAIR = 4
NSPAN = 4
SPAN = 512
KC = 128
P = 128
LAG = 3

_CACHE = {}


def _build():
    import concourse.bacc as bacc
    import concourse.mybir as mybir
    import concourse.tile as tile

    f32 = mybir.dt.float32
    f16 = mybir.dt.float16
    bf16 = mybir.dt.bfloat16
    fp8 = mybir.dt.float8e4
    u8 = mybir.dt.uint8
    Exp = mybir.ActivationFunctionType.Exp

    DR = mybir.MatmulPerfMode.DoubleRow

    nc = bacc.Bacc("TRN2", target_bir_lowering=False, debug=False,
                   num_devices=NCORES)

    x8_h = nc.dram_tensor("x8", (D, T), u8, kind="ExternalInput")
    x8l_h = nc.dram_tensor("x8l", (D, T), u8, kind="ExternalInput")
    wq8_h = nc.dram_tensor("wq8", (D, 512), u8, kind="ExternalInput")
    wk8_h = nc.dram_tensor("wk8", (D, 512), u8, kind="ExternalInput")
    wv8h_h = nc.dram_tensor("wv8h", (D, 512), u8, kind="ExternalInput")
    wv8l_h = nc.dram_tensor("wv8l", (D, 512), u8, kind="ExternalInput")
    woh_h = nc.dram_tensor("woh", (512, D), f16, kind="ExternalInput")
    out_h = nc.dram_tensor("out", (T, D), f16, kind="ExternalOutput")

    # x8: D = (dcp 4, k 2, p 128); per span slice on T
    x8_d = x8_h.ap().rearrange("(dcp k p) t -> p dcp k t", p=P, k=2)
    x8l_d = x8l_h.ap().rearrange("(dcp k p) t -> p dcp k t", p=P, k=2)
    wq8_d = wq8_h.ap().rearrange("(dcp k p) (t h m) -> p dcp k t h m",
                                 p=P, k=2, t=2, h=2)
    wk8_d = wk8_h.ap().rearrange("(dcp k p) (t h m) -> p dcp k t h m",
                                 p=P, k=2, t=2, h=2)
    wv8h_d = wv8h_h.ap().rearrange("(dcp k p) f -> p dcp k f", p=P, k=2)
    wv8l_d = wv8l_h.ap().rearrange("(dcp k p) f -> p dcp k f", p=P, k=2)
    woh_d = woh_h.ap().rearrange("(pc p) f -> p pc f", p=P)

    with tile.TileContext(nc) as tc:
        with (
            tc.tile_pool(name="persist", bufs=1) as persist,
            tc.tile_pool(name="x8p", bufs=2) as x8pool,
            tc.tile_pool(name="xlp", bufs=2) as xlpool,
            tc.tile_pool(name="qp", bufs=4) as qpool,
            tc.tile_pool(name="ptp", bufs=7) as ptpool,
            tc.tile_pool(name="ptb", bufs=6) as ptbpool,
            tc.tile_pool(name="rzp", bufs=12) as rzpool,
            tc.tile_pool(name="cp", bufs=10) as cpool,
            tc.tile_pool(name="stg", bufs=8) as stgpool,
            tc.tile_pool(name="psS", bufs=2, space="PSUM") as psS,
            tc.tile_pool(name="psC", bufs=2, space="PSUM") as psC,
            tc.tile_pool(name="psT", bufs=2, space="PSUM") as psT,
        ):
            wq8 = persist.tile([P, 4, 2, 2, 2, P], u8, tag="wq8", name="wq8")
            wk8 = persist.tile([P, 4, 2, 2, 2, P], u8, tag="wk8", name="wk8")
            wv8 = persist.tile([P, 4, 2, 2, 512], u8, tag="wv8", name="wv8")
            wot = persist.tile([P, 4, D], f16, tag="wot", name="wot")
            kT8 = [persist.tile([P, 2, T], fp8, tag=f"kT8{t}", name=f"kT8{t}")
                   for t in range(2)]
            # Vb: [p, kjp 8, kj 2, pr 4, u 2, 96]: 64 v-dims + ones col at
            # 64 + zero pad to 96 (dual-fp8 ldweights needs cols % 32 == 0,
            # >= 64; psum rows 65:96 are dead)
            VW = 96
            Vbh = persist.tile([P, 8, 2, NPAIR, 2, VW], fp8, tag="Vbh",
                               name="Vbh")
            # diag-chunk AV runs in bf16 (pt quantization error bites only
            # concentrated near-diagonal attention rows): [p, kc, pr, u, 65]
            Vbb = persist.tile([P, 16, NPAIR, 2, HD + 1], bf16, tag="Vbb",
                               name="Vbb")
            mask01 = persist.tile([P, KC], bf16, tag="mask01", name="mask01")
            one = nc.const_aps.tensor(1.0, (P, 1))

            # ---- initial DMAs ----
            x8ts = {0: x8pool.tile([P, 4, 2, SPAN], u8, tag="x8t", name="x8t0")}
            x8ls = {0: xlpool.tile([P, 4, 2, SPAN], u8, tag="x8l", name="x8l0")}
            nc.sync.dma_start(wq8[:], wq8_d)
            nc.sync.dma_start(x8ts[0][:], x8_d[:, :, :, 0:SPAN])
            nc.sync.dma_start(wk8[:], wk8_d)
            nc.sync.dma_start(wv8[:, :, :, 0, :], wv8h_d)
            nc.sync.dma_start(x8ls[0][:], x8l_d[:, :, :, 0:SPAN])
            nc.sync.dma_start(wv8[:, :, :, 1, :], wv8l_d)
            nc.sync.dma_start(wot[:], woh_d)


            # mask01[p, f] = 1 if p <= f else 0 (post-exp diag pt mask)
            nc.gpsimd.memset(mask01[:], 1.0)
            nc.gpsimd.affine_select(
                out=mask01[:], in_=mask01[:],
                compare_op=mybir.AluOpType.is_ge, fill=0.0,
                base=0, channel_multiplier=-1, pattern=[[1, KC]],
            )
            # Vbh pad cols [65:96] must be zero (read by every off-diag
            # AV); data cols are always written before first read
            nslots = 8 * 2 * NPAIR * 2
            nc.gpsimd.memset(
                Vbh[:].rearrange("p a b c d e -> p (a b c d) e")
                [:, :, HD + 1:], 0.0)
            nc.vector.tensor_copy(
                Vbh[:].rearrange("p a b c d e -> p (a b c d) e")
                [:, :, HD:HD + 1],
                one.to_broadcast((P, nslots, 1)))
            nc.vector.tensor_copy(
                Vbb[:].rearrange("p a c d e -> p (a c d) e")
                [:, :, HD:HD + 1],
                one.to_broadcast((P, 16 * NPAIR * 2, 1)))

            qts = {}    # (sp, t) -> [P, 2, SPAN] fp8 tile
            ctxs = {}   # (sp, pr) -> [P, SPAN] f16 tile

            # ---------- emission helpers ----------
            def proj_qk(w8, pr_half, h, sp, x8t, scale, isq, on_act=False):
                t = pr_half

                def emit():
                    ps = psT.tile([P, SPAN], f32, tag="tr", name="psqk")
                    for dcp in range(4):
                        nc.tensor.matmul(
                            ps[:], w8[:, dcp, :, t, h, :].bitcast(fp8),
                            x8t[:, dcp, :, :].bitcast(fp8),
                            start=(dcp == 0), stop=(dcp == 3), perf_mode=DR)
                    if isq:
                        if (sp, t) not in qts:
                            qts[(sp, t)] = qpool.tile(
                                [P, 2, SPAN], fp8, tag=f"qT{t}",
                                name=f"qT{t}_{sp}")
                        dest = qts[(sp, t)][:, h, :]
                    else:
                        dest = kT8[t][:, h, sp * SPAN:(sp + 1) * SPAN]
                    if on_act:
                        # span-0 era: ACT is idle and DVE is the choke --
                        # the psum->fp8 scale-copy runs as activation Copy
                        nc.scalar.activation(
                            dest, ps[:], mybir.ActivationFunctionType.Copy,
                            scale=scale)
                    else:
                        nc.vector.tensor_scalar_mul(dest, ps[:], scale)
                return emit

            def proj_v(sp, tb, x8t, x8lt):
                # 3-pass fp8 DR: 16v = x_hi@w_hi + x_lo@w_hi + x_hi@w_lo
                # (w residual split keeps the value path at ~bf16 quality)
                def emit():
                    ps = psT.tile([P, SPAN], f32, tag="tr", name="psv")
                    ts = slice(tb * P, (tb + 1) * P)
                    passes = [(x8t, 0), (x8lt, 0), (x8t, 1)]
                    for pi, (xt, wl) in enumerate(passes):
                        for dcp in range(4):
                            nc.tensor.matmul(
                                ps[:], xt[:, dcp, :, ts].bitcast(fp8),
                                wv8[:, dcp, :, wl, :].bitcast(fp8),
                                start=(pi == 0 and dcp == 0),
                                stop=(pi == 2 and dcp == 3), perf_mode=DR)
                    kc = sp * 4 + tb
                    psv = ps[:].rearrange("p (pr u v) -> p pr u v",
                                          pr=NPAIR, v=HD)
                    nc.vector.tensor_scalar_mul(
                        Vbh[:, kc // 2, kc % 2, :, :, 0:HD], psv, 1.0 / 16)
                    nc.vector.tensor_scalar_mul(
                        Vbb[:, kc, :, :, 0:HD], psv, 1.0 / 16)
                return emit

            def q_groups(sp, x8t, on_act=False):
                return [proj_qk(wq8, t, h, sp, x8t, 1.0 / 32, True, on_act)
                        for t in range(2) for h in range(2)]

            def k_groups(sp, x8t, on_act=False):
                return [proj_qk(wk8, t, h, sp, x8t, 1.0 / 16, False, on_act)
                        for t in range(2) for h in range(2)]

            def v_groups(sp, x8t, x8lt):
                return [proj_v(sp, tb, x8t, x8lt) for tb in range(4)]

            ostages = {}  # tb -> [P, 2, SPAN] f16 stage (one DMA per tb)

            def outproj_group(sp, tb, os_, dma_q, ps_ap=None, act_copy=False):
                def emit():
                    if ps_ap is None:
                        ps = psT.tile([P, SPAN], f32, tag="tr", name="pso")[:]
                    else:
                        ps = ps_ap
                    for pc in range(NPAIR):
                        nc.tensor.matmul(
                            ps,
                            ctxs[(sp, pc)][:, (tb - sp * 4) * P:
                                           (tb - sp * 4 + 1) * P],
                            wot[:, pc, os_ * SPAN:(os_ + 1) * SPAN],
                            start=(pc == 0), stop=(pc == NPAIR - 1))
                    if os_ == 0:
                        ostages[tb] = stgpool.tile([P, 2, SPAN], f16,
                                                   tag="st", name="stage")
                    stage = ostages[tb]
                    if act_copy:
                        nc.scalar.activation(
                            stage[:, os_, :], ps,
                            mybir.ActivationFunctionType.Copy)
                    else:
                        nc.vector.tensor_copy(stage[:, os_, :], ps)
                    if os_ == 1:
                        # one merged 1024-col DMA per tb: fewer bus slots
                        dma_q.dma_start(
                            out_h.ap()[tb * P:(tb + 1) * P, :],
                            ostages.pop(tb)[:].rearrange("p a f -> p (a f)"))
                return emit

            def outproj_groups(sp):
                # all out-DMAs on SP: a Pool-queue DMACopy hold would block
                # partition_broadcast (evict chain)
                return [outproj_group(sp, tb, os_, nc.sync)
                        for tb in range(sp * 4, (sp + 1) * 4)
                        for os_ in range(2)]

            # ---------- attention for one span ----------
            deferred = []

            def attn_span(s, fillers, vgs=(), kgs=()):
                K = 4 * (s + 1)
                nslot = (K + 4) * NPAIR
                state = {"slot": 0, "fi": 0}

                def pace():
                    tgt = min(len(fillers),
                              len(fillers) * (state["slot"] + 1) // nslot)
                    while state["fi"] < tgt:
                        fillers[state["fi"]]()
                        state["fi"] += 1

                def tick():
                    state["slot"] += 1
                    pace()

                for pr in range(NPAIR):
                    t = pr // 2
                    ctxp = [psC.tile([96, SPAN], f32, tag="ctx",
                                     name=f"ctx{u}") for u in range(2)]
                    ct = cpool.tile([P, SPAN], f16, tag=f"cT{pr}",
                                    name=f"cT{pr}_{s}")
                    ctxs[(s, pr)] = ct
                    pts = {}
                    qt = qts[(s, t)]

                    avn = [0, 0]  # AV matmuls emitted per u (K total each)

                    def emit_qk(kj, pr=pr, t=t, pts=pts, qt=qt):
                        m = kj - 4 * s
                        c0 = 0 if m < 0 else m * KC
                        if m == 2:
                            # m2+m3 share one ss tile and one exp: m2 scores
                            # at cols [256:512], m3's 128 live cols parked in
                            # the unused [128:256]; exp'd together at m3
                            ss = psS.tile([P, 2, SPAN], f32, tag="psS",
                                          name="ss23")
                            pts[("ss23",)] = ss
                        elif m == 3:
                            ss = pts[("ss23",)]
                        else:
                            ss = psS.tile([P, 2, SPAN], f32, tag="psS",
                                          name="ss")
                        dst = slice(KC, 2 * KC) if m == 3 else slice(c0, SPAN)
                        for u in range(2):
                            b32 = 32 * (2 * (pr % 2) + u)
                            r = slice(b32, b32 + 32)
                            nc.tensor.matmul(
                                ss[:, u, dst],
                                kT8[t][r, :, kj * KC:(kj + 1) * KC],
                                qt[r, :, c0:],
                                start=True, stop=True, perf_mode=DR,
                                tile_position=(b32, 0))
                        if m < 0:
                            # off-diagonal: fp8 pt pair for DR AV
                            kjp, sl = kj // 2, kj % 2
                            if sl == 0:
                                pts[kjp] = ptpool.tile([P, 2, 2, SPAN], fp8,
                                                       tag="pt", name="pt")
                            nc.scalar.activation(pts[kjp][:, sl, :, :],
                                                 ss[:, :, :], Exp, scale=0.25)
                        elif m == 2:
                            # defer: exp'd together with m3
                            ptb = ptbpool.tile([P, 2, SPAN], bf16,
                                               tag="ptb", name="ptb")
                            pts[("d", kj)] = ptb
                            pts[("d", kj + 1)] = ptb
                        elif m == 3:
                            ptb = pts[("d", kj)]
                            del pts[("ss23",)]
                            nc.scalar.activation(ptb[:, :, KC:],
                                                 ss[:, :, KC:], Exp,
                                                 scale=0.25)
                            # both triangles ([128:256] = m3, [256:384] = m2)
                            # use the same 128x128 mask: one DVE mul
                            nc.vector.tensor_mul(
                                ptb[:, :, KC:3 * KC].rearrange(
                                    "p u (b f) -> p u b f", b=2),
                                ptb[:, :, KC:3 * KC].rearrange(
                                    "p u (b f) -> p u b f", b=2),
                                mask01[:].rearrange("p (u b f) -> p u b f",
                                                    u=1, b=1)
                                .to_broadcast((P, 2, 2, KC)))
                        else:
                            # diagonal m0/m1: own tile + exp + triangle mask
                            ptb = ptbpool.tile([P, 2, SPAN], bf16,
                                               tag="ptb", name="ptb")
                            pts[("d", kj)] = ptb
                            nc.scalar.activation(ptb[:, :, c0:],
                                                 ss[:, :, c0:], Exp,
                                                 scale=0.25)
                            nc.vector.tensor_mul(
                                ptb[:, :, c0:c0 + KC], ptb[:, :, c0:c0 + KC],
                                mask01[:].rearrange("p (u f) -> p u f", u=1)
                                .to_broadcast((P, 2, KC)))

                    NU = 2 * s + 4  # AV matmuls per u-chain

                    def emit_av8(kjp, pr=pr, pts=pts, ctxp=ctxp, NU=NU):
                        pt = pts.pop(kjp)
                        for u in range(2):
                            nc.tensor.matmul(
                                ctxp[u][:],
                                Vbh[:, kjp, :, pr, u, :],
                                pt[:, :, u, :],
                                start=(avn[u] == 0),
                                stop=(avn[u] == NU - 1),
                                perf_mode=DR, skip_group_check=True)
                            avn[u] += 1

                    def emit_avd(kj, pr=pr, pts=pts, ctxp=ctxp, NU=NU):
                        ptb = pts.pop(("d", kj))
                        m = kj - 4 * s
                        c0 = m * KC
                        # m3's pt lives at stored cols [128:256] (positional
                        # matmul mapping: out offset is independent of rhs)
                        rsl = slice(KC, 2 * KC) if m == 3 else slice(c0, SPAN)
                        for u in range(2):
                            nc.tensor.matmul(
                                ctxp[u][0:HD + 1, c0:],
                                Vbb[:, kj, pr, u, :],
                                ptb[:, u, rsl],
                                start=(avn[u] == 0), stop=(avn[u] == NU - 1),
                                skip_group_check=True)
                            avn[u] += 1

                    rzt = [rzpool.tile([P, 2, SPAN], bf16, tag="rz",
                                       name="rz"),
                           rzpool.tile([P, 2, SPAN], bf16, tag="rz",
                                       name="rzbc")]

                    def fin(u, cs=slice(0, SPAN), ctxp=ctxp, ct=ct,
                            rzt=rzt):
                        rz, rzbc = rzt

                        def run():
                            # 1/Z row: psum row 64 -> sbuf row 0
                            # (cross-partition DVE write, hw-verified)
                            with nc.allow_low_precision(
                                    reason="1/Z bf16: 0.4% on ctx"):
                                nc.vector.reciprocal(
                                    rz[0:1, u, cs], ctxp[u][64:65, cs])
                            nc.gpsimd.partition_broadcast(
                                rzbc[:, u, cs], rz[0:1, u, cs])
                            nc.vector.tensor_mul(
                                ct[u * HD:(u + 1) * HD, cs],
                                ctxp[u][0:HD, cs], rzbc[0:HD, u, cs])
                        return run

                    def evict(pr=pr, s=s):
                        # [0:256] was evicted early (see units)
                        deferred.append(fin(0, slice(256, SPAN)))
                        deferred.append(fin(1, slice(256, SPAN)))

                    # AV work units: (ready_kj, emit_fn); off-diag kjp ready
                    # at its odd kj, diag kj ready at kj
                    units = []
                    for kj2 in range(K):
                        if kj2 < 4 * s:
                            if kj2 % 2 == 1:
                                units.append((kj2, kj2 // 2, emit_av8))
                        else:
                            units.append((kj2, kj2, emit_avd))
                            if kj2 == 4 * s + 1:
                                # ctx cols [0:256] are final after avd(m1):
                                # evict them under the remaining diag AVs
                                # (frees the psC tile ~1us earlier for the
                                # next pair's AV chain)
                                units.append(
                                    (kj2, None,
                                     lambda _, f0=fin(0, slice(0, 256)),
                                     f1=fin(1, slice(0, 256)):
                                     (f0(), f1())))
                    # JIT placement: V(s) spread across pr0's whole kj
                    # range (consumers: pr0 diag AV); K(s) t0-groups in pr0,
                    # t1-groups in pr1 (consumers: pr0/pr2 diag QK)
                    jit = {}
                    if pr == 0:
                        for i in range(len(vgs)):
                            jit.setdefault(max(i * K // 4, i), []).append(
                                vgs[i])
                        # K(s) t0 keys must precede the diag QK at kj=4s
                        # (jit runs after emit_qk in the same iteration)
                        for i in (0, 1) if kgs else ():
                            jit.setdefault(min(1 + i * (K // 3 + 1),
                                               4 * s - 1), []).append(kgs[i])
                    elif pr == 1:
                        for i in (2, 3) if kgs else ():
                            jit.setdefault(1 + (i - 2) * (K // 3 + 1),
                                           []).append(kgs[i])
                    ui = [0]
                    for kj in range(K):
                        emit_qk(kj)
                        for g in jit.get(kj, ()):
                            g()
                        if deferred:
                            deferred.pop(0)()
                        tick()
                        while (ui[0] < len(units)
                               and units[ui[0]][0] + LAG <= kj):
                            units[ui[0]][2](units[ui[0]][1])
                            ui[0] += 1
                    while ui[0] < len(units):
                        if deferred:
                            deferred.pop(0)()
                        tick()
                        units[ui[0]][2](units[ui[0]][1])
                        ui[0] += 1
                    evict()
                while state["fi"] < len(fillers):
                    fillers[state["fi"]]()
                    state["fi"] += 1

            # ---------- main schedule ----------
            # V(s) and K(s) run inside span s itself (their first consumers
            # are span s's own diag AV / diag QK) -- keeps earlier spans off
            # the PE; only Q(s+1) must finish during span s
            # pr0's attention needs (q,k) t0 groups first: interleave
            qg0 = q_groups(0, x8ts[0], on_act=True)
            kg0 = k_groups(0, x8ts[0], on_act=True)
            for i in range(4):
                qg0[i]()
                kg0[i]()
            for s in range(NSPAN):
                vgs = v_groups(s, x8ts[s], x8ls[s])
                kgs = k_groups(s, x8ts[s]) if s >= 1 else ()
                fillers = []
                if s + 1 < NSPAN:
                    x8t = x8pool.tile([P, 4, 2, SPAN], u8, tag="x8t",
                                      name=f"x8t{s + 1}")
                    x8lt = xlpool.tile([P, 4, 2, SPAN], u8, tag="x8l",
                                       name=f"x8l{s + 1}")
                    x8ts[s + 1], x8ls[s + 1] = x8t, x8lt
                    sl = slice((s + 1) * SPAN, (s + 2) * SPAN)
                    nc.sync.dma_start(x8t[:], x8_d[:, :, :, sl])
                    nc.sync.dma_start(x8lt[:], x8l_d[:, :, :, sl])
                    fillers += q_groups(s + 1, x8t, on_act=(s == 0))
                if s == 2:
                    fillers += outproj_groups(0)
                elif s == 3:
                    fillers += outproj_groups(1) + outproj_groups(2)
                attn_span(s, fillers, vgs, kgs)
            while deferred:
                deferred.pop(0)()
            # tail outproj(3): attention is done, so psS's 4 banks are free
            # scratch -- 6 parallel psums let all 32 matmuls run back-to-back
            # (no psT-rotation stalls, PE stays ramped); stage copies split
            # DVE/ACT (both idle at the tail)
            sc = [psT.tile([P, SPAN], f32, tag="tr", name=f"ost{i}")[:]
                  for i in range(2)]
            for i in range(2):
                t = psS.tile([P, 2, SPAN], f32, tag="psS", name=f"osc{i}")
                sc += [t[:, 0, :], t[:, 1, :]]
            for i in range(2):
                sc.append(psC.tile([P, SPAN], f32, tag="ctx",
                                   name=f"oscc{i}")[:])
            tails = [(tb, os_) for tb in range(12, 16) for os_ in range(2)]
            # pc-interleaved: pc0..2 matmuls only need the earlier pairs'
            # ctx and run under the last evict chain; only pc3 gates on it
            for pc in range(NPAIR):
                for gi, (tb, os_) in enumerate(tails):
                    nc.tensor.matmul(
                        sc[gi],
                        ctxs[(3, pc)][:, (tb - 12) * P:(tb - 11) * P],
                        wot[:, pc, os_ * SPAN:(os_ + 1) * SPAN],
                        start=(pc == 0), stop=(pc == NPAIR - 1),
                        skip_group_check=True)
            tstg = {}
            for gi, (tb, os_) in enumerate(tails):
                if os_ == 0:
                    tstg[tb] = stgpool.tile([P, 2, SPAN], f16, tag="st",
                                            name="tstage")
                if gi % 2 == 1:
                    nc.scalar.activation(
                        tstg[tb][:, os_, :], sc[gi],
                        mybir.ActivationFunctionType.Copy)
                else:
                    nc.vector.tensor_copy(tstg[tb][:, os_, :], sc[gi])
                if os_ == 1:
                    nc.sync.dma_start(
                        out_h.ap()[tb * P:(tb + 1) * P, :],
                        tstg.pop(tb)[:].rearrange("p a f -> p (a f)"))

    nc.compile()
    return nc


def get_nc():
    if "nc" not in _CACHE:
        _CACHE["nc"] = _build()
    return _CACHE["nc"]


def _perm512():
    perm = np.empty(512, np.int64)
    i = 0
    for t in range(2):
        for h in range(2):
            for beta in range(4):
                pr = 2 * t + beta // 2
                u = beta % 2
                for dd in range(32):
                    perm[i] = pr * 128 + u * 64 + h * 32 + dd
                    i += 1
    return perm


def kernel(x, Wq, Wk, Wv, Wo, bo):
    import ml_dtypes
    from concourse import bass_utils

    e4 = ml_dtypes.float8_e4m3

    x = np.asarray(x, dtype=np.float32)
    Wq, Wk, Wv = (np.asarray(w, dtype=np.float32) for w in (Wq, Wk, Wv))
    Wo = np.asarray(Wo, dtype=np.float32)
    bo = np.asarray(bo, dtype=np.float32)
    perm = _perm512()

    in_maps = []
    for c in range(NCORES):
        b, g = c // 2, c % 2
        gsl = slice(g * 512, (g + 1) * 512)
        xT = np.ascontiguousarray(x[b].T)
        x8 = xT.astype(e4)
        wv16 = np.ascontiguousarray(16.0 * Wv[gsl].T)
        wv8h = wv16.astype(e4)
        in_maps.append({
            "x8": x8.view(np.uint8),
            # residual of the e4m3 x quantization (V-proj pass 2)
            "x8l": (xT - x8.astype(np.float32)).astype(e4).view(np.uint8),
            # w quantized x16 to dodge e4m3 subnormals; unscaled on-chip
            "wq8": np.ascontiguousarray((16.0 * Wq[gsl].T)[:, perm])
            .astype(e4).view(np.uint8),
            "wk8": np.ascontiguousarray((16.0 * Wk[gsl].T)[:, perm])
            .astype(e4).view(np.uint8),
            "wv8h": wv8h.view(np.uint8),
            "wv8l": (wv16 - wv8h.astype(np.float32)).astype(e4)
            .view(np.uint8),
            "woh": np.ascontiguousarray(Wo[:, gsl].T).astype(np.float16),
        })

    nc = get_nc()
    res = bass_utils.run_bass_kernel_spmd(nc, in_maps,
                                          core_ids=list(range(NCORES)))
    parts = [res.results[c]["out"].astype(np.float32) for c in range(NCORES)]
    out = np.stack([parts[2 * b] + parts[2 * b + 1] + bo for b in range(B)])
    return out.astype(np.float32)
